# revision 19
# baseline (speedup 1.0000x reference)
"""Trainium2 Bass kernel for nn_Block_31954556682442 (spiking MoE-SSA block).

Sharding: pure data-parallel over batch B=8 -> one sample (4 LIF time steps)
per NeuronCore, zero collectives. v3 design:
  - weight matmuls in fp32r (PE rounds inputs to ~11 mantissa bits, exact
    fp32 accumulate, full bf16 throughput) -> no hi/lo splits needed
  - time steps batched into matmul free dims (N=512 covers 2 steps)
  - bf16 exact-integer attention core (spikes are {0,1})
  - LIF scans in 2^t-scaled form: membrane update = tensor_add on GPSIMD,
    spike/reset = tensor_scalar/scalar_tensor_tensor on DVE (threshold 2^t)
  - depthwise 3x3 conv t-batched: 9 shifted per-partition-scalar MACs over
    (128, 4*256) tiles on DVE, 2^t applied at the LIF add
  - PSUM evicts fused with BN scale+bias (+2^t*0.5) on ScalarE
Self-contained: hardcodes all shapes; no sibling imports.
"""
import numpy as np
import ml_dtypes

import concourse.bacc as bacc
import concourse.mybir as mybir
import concourse.tile as tile
from concourse.bass_utils import run_bass_kernel_spmd

F32 = mybir.dt.float32
F32R = mybir.dt.float32r
BF16 = mybir.dt.bfloat16
AL = mybir.AluOpType
AF = mybir.ActivationFunctionType

T, B, C, N = 4, 8, 384, 256
ED = 96
NE = 4
NU = 5
HID, HH = 2048, 1024
S = float(1.0 / np.sqrt(1.0 + 1e-5))
P = 128


def _r(ap):
    return ap.bitcast(F32R)


def _body(nc, tc, d):
    from contextlib import ExitStack
    VE = nc.vector
    GE = nc.gpsimd
    K_SG = float(2.0 ** 20)

    with ExitStack() as ctx:
        def pool(name, bufs, space="SBUF"):
            return ctx.enter_context(tc.tile_pool(name=name, bufs=bufs, space=space))

        wp = pool("wp", 1)
        mp = pool("mp", 1)
        ps_m = pool("ps_m", 2, "PSUM")
        xs_p = pool("xs_p", 3)       # (128,1024) f32, doubles as x_new
        xkq_p = pool("xkq_p", 2)     # (96,1280) f32
        xev_p = pool("xev_p", 4)     # (128,768) f32 evict/LIF targets
        sp_p = pool("sp_p", 4)       # (96,1280) bf16 kq spikes
        vsp_p = pool("vsp_p", 4)     # (128,768) bf16
        wsp_p = pool("wsp_p", 4)     # (128,8) f32
        at_p = pool("at_p", 3)       # (128,256) bf16
        rsp_p = pool("rsp_p", 2)     # (128,768) bf16
        y_p = pool("y_p", 8)         # (128,384) bf16
        ydn_p = pool("ydn_p", 3)     # (128,1024) bf16
        xh_p = pool("xh_p", 2)       # (128,2048) f32
        x2_p = pool("x2_p", 2)       # (128,1024) bf16
        acc_p = pool("acc_p", 2)     # (128,1024) f32
        mg_p = pool("mg_p", 8)       # (128,1024) bf16, all 8 persist for fc2
        mh_p = pool("mh_p", 1)       # (128,512) f32
        mdw_p = pool("mdw_p", 1)     # (128,256) f32

        # ---------------- weight loads ----------------
        def wload(name, shape, dt=F32, src=None, rr=False):
            w = wp.tile(shape, dt, name=name, tag=name)
            s_ = d[name] if src is None else src
            nc.sync.dma_start(out=_r(w) if rr else w, in_=_r(s_) if rr else s_)
            return w

        ident = wload('ident', [P, P], BF16)
        # PE warmup: ~60 dummy matmuls to ramp the PE pstate before phase A
        pwarm = ps_m.tile([P, P], F32, name="pwarm", tag="pm")
        for wi in range(60):
            nc.tensor.matmul(pwarm, ident, ident, start=True, stop=True)
        warm_sink = wp.tile([P, 1], F32, name="warm_sink", tag="warm_sink")
        nc.scalar.activation(warm_sink, pwarm[:, 0:1], AF.Copy)

        # xs first (A-phase starts on these)
        xs_kt = []
        for kt in range(3):
            x_ = xs_p.tile([P, 4 * N], F32, name=f"xs{kt}", tag="t")
            xs_kt.append(x_)
        for kt in range(3):
            nc.sync.dma_start(out=_r(xs_kt[kt]), in_=_r(d['xin'][kt*P:(kt+1)*P, :]))
        kqf, vf, f1f = [], [], []
        for kt in range(3):
            kqf.append(wload(f'kqf_{kt}', [P, 484], F32, d['kq_wf'][kt*P:(kt+1)*P, :], rr=True))
        skq = wload('s_kq', [100, 40])
        a_kq, b_kq = skq[:, 0:20], skq[:, 20:40]
        for kt in range(3):
            vf.append(wload(f'vf_{kt}', [P, 384], F32, d['v_wf'][kt*P:(kt+1)*P, :], rr=True))
        pj2 = []
        for kt in range(3):
            pj2.append(wload(f'pj2_{kt}', [P, 768], BF16, d['pj_w2'][kt*P:(kt+1)*P, :]))
        pjh = [w[:, 0:384] for w in pj2]; pjl = [w[:, 384:768] for w in pj2]
        spo = wload('s_po', [P, 48])
        a_p, b_p = spo[:, 0:12], spo[:, 12:24]
        a_o, b_o = spo[:, 24:36], spo[:, 36:48]
        for kt in range(3):
            f1f.append(wload(f'f1f_{kt}', [P, 2048], F32, d['f1_wf'][kt*P:(kt+1)*P, :], rr=True))
        sh = wload('s_h', [P, 132])
        a_h, b_h, sgb = sh[:, 0:64], sh[:, 64:128], sh[:, 128:132]
        sdw = wload('s_dw', [P, 32])
        bdw_t = sdw[:, 0:32]
        f22 = []
        for ch in range(8):
            f22.append(wload(f'f22_{ch}', [P, 768], BF16, d['f2_w2'][ch*P:(ch+1)*P, :]))
        f2h = [w[:, 0:384] for w in f22]; f2l = [w[:, 384:768] for w in f22]
        dwd_p = pool("dwd_p", 3)     # (128,1152) bf16 streamed diag taps
        dwd = {}

        def dwd_load(ch):
            w = dwd_p.tile([P, 1152], BF16, name=f"dwd{ch}", tag="t")
            nc.sync.dma_start(out=w, in_=d['dw_diag'][ch*P:(ch+1)*P, :])
            dwd[ch] = w
        # padded x1 spike tiles: [20 guard | 4 x (18x18) planes | 20 guard]
        x1p_ab = []
        for i in range(2):
            xp_ = wp.tile([P, 1336], BF16, name=f"x1p{i}", tag=f"x1p{i}")
            GE.memzero(xp_)
            x1p_ab.append(xp_)

        # ---------------- phase A: kq / v / router matmuls + evicts ----------------
        m_kq = mp.tile([100, 5 * N], F32, name="m_kq", tag="m_kq")
        m_vt = mp.tile([P, 768], F32, name="m_vt", tag="m_vt")
        m_p = mp.tile([P, 768], F32, name="m_p", tag="m_p")
        m_o = [mp.tile([P, N], F32, name=f"m_o{i}", tag=f"m_o{i}") for i in range(3)]

        xkq_t = [xkq_p.tile([100, 5 * N], F32, name=f"xkq{t}", tag="t") for t in range(T)]
        xvt_t = [xev_p.tile([P, 768], F32, name=f"xvt{t}", tag="t") for t in range(T)]

        for tp in range(2):
            for u in range(NU):
                # u0 block is 100 wide: 96 k-head rows + 4 router rows
                rw = 100 if u == 0 else 96
                w0 = 0 if u == 0 else 100 + 96 * (u - 1)
                pt = ps_m.tile([rw, 512], F32, name=f"pkq{u}_{tp}", tag="pm")
                for kt in range(3):
                    nc.tensor.matmul(pt, _r(kqf[kt][:, w0:w0+rw]),
                                     _r(xs_kt[kt][:, tp*512:(tp+1)*512]),
                                     start=(kt == 0), stop=(kt == 2))
                for ti in range(2):
                    t = tp * 2 + ti
                    c = u * 4 + t
                    nc.scalar.activation(xkq_t[t][0:rw, u*N:(u+1)*N], pt[:, ti*N:(ti+1)*N],
                                         AF.Identity, bias=b_kq[0:rw, c:c+1],
                                         scale=a_kq[0:rw, c:c+1])
        for t in range(T):
            for mt in range(2):
                pv = ps_m.tile([P, 384], F32, name=f"pvt{t}_{mt}", tag="pm")
                for kt in range(3):
                    nc.tensor.matmul(pv, _r(xs_kt[kt][:, t*N + mt*P: t*N + (mt+1)*P]),
                                     _r(vf[kt]), start=(kt == 0), stop=(kt == 2))
                nc.scalar.activation(xvt_t[t][:, mt*384:(mt+1)*384], pv, AF.Copy,
                                     bias=0.0, scale=0.5 * float(2.0 ** t))

        # ---------------- phase B: LIF scans for kq / v / r ----------------
        sp_t, v_sp, w_sp = [], [], []
        for t in range(T):
            thr = float(2.0 ** t)
            U = xkq_t[t]
            if t > 0:
                GE.tensor_add(U, m_kq, U)
            sp = sp_p.tile([100, 5 * N], BF16, name=f"sp{t}", tag="t")
            VE.tensor_single_scalar(sp, U, thr, AL.is_ge)
            if t < T - 1:
                VE.scalar_tensor_tensor(out=m_kq, in0=U, scalar=thr, in1=U,
                                        op0=AL.is_lt, op1=AL.mult)
            sp_t.append(sp)

            U = xvt_t[t]
            if t > 0:
                GE.tensor_add(U, m_vt, U)
            vs = vsp_p.tile([P, 768], BF16, name=f"vsp{t}", tag="t")
            nc.scalar.activation(vs, U, AF.Sigmoid, bias=sgb[:, t:t+1],
                                 scale=K_SG)
            if t < T - 1:
                VE.scalar_tensor_tensor(out=m_vt, in0=U, scalar=thr, in1=U,
                                        op0=AL.is_lt, op1=AL.mult)
            v_sp.append(vs)

            # router spikes ride in sp rows 96:100 of the u0 block; transpose
            # each position-half to (pos, 4) for the per-partition y weights
            ws = wsp_p.tile([P, 8], F32, name=f"wsp{t}", tag="t")
            ws4 = wsp_p.tile([4, N], BF16, name=f"ws4_{t}", tag="t")
            nc.sync.dma_start(out=ws4, in_=sp[96:100, 0:N])
            for mt in range(2):
                ptw = ps_m.tile([P, 4], BF16, name=f"ptw{t}{mt}", tag="pm")
                nc.tensor.transpose(ptw, ws4[:, mt*P:(mt+1)*P], ident[0:4, 0:4])
                nc.scalar.activation(ws[:, mt*4:(mt+1)*4], ptw, AF.Copy)
            w_sp.append(ws)

        # ---------------- phase C: experts ----------------
        y = [[None] * 2 for _ in range(T)]
        m_res_e = [mp.tile([P, 768], F32, name=f"m_res{e}", tag=f"m_res{e}")
                   for e in range(NE)]
        for t in range(T):
            thr = float(2.0 ** t)
            for e in range(NE):
                m_res = m_res_e[e]
                at_sb = []
                for mt in range(2):
                    pa = ps_m.tile([P, N], F32, name=f"pat{e}{t}{mt}", tag="pm")
                    nc.tensor.matmul(pa, sp_t[t][0:96, mt*P:(mt+1)*P],
                                     sp_t[t][0:96, (1+e)*N:(2+e)*N], start=True, stop=True)
                    ats = at_p.tile([P, N], BF16, name=f"at{e}{t}{mt}", tag="t")
                    nc.scalar.activation(ats, pa, AF.Copy)
                    at_sb.append(ats)
                xr = xev_p.tile([P, 768], F32, name=f"xres{e}{t}", tag="t")
                for mt in range(2):
                    pr_ = ps_m.tile([P, 384], F32, name=f"pres{e}{t}{mt}", tag="pm")
                    for mk in range(2):
                        nc.tensor.matmul(pr_, at_sb[mk][:, mt*P:(mt+1)*P],
                                         v_sp[t][:, mk*384:(mk+1)*384],
                                         start=(mk == 0), stop=(mk == 1))
                    nc.scalar.activation(xr[:, mt*384:(mt+1)*384], pr_, AF.Copy,
                                         bias=0.0, scale=0.5 * thr)
                U = xr
                if t > 0:
                    GE.tensor_add(U, m_res, U)
                rs = rsp_p.tile([P, 768], BF16, name=f"rsp{e}{t}", tag="t")
                VE.tensor_single_scalar(rs, U, thr, AL.is_ge)
                if t < T - 1:
                    VE.scalar_tensor_tensor(out=m_res, in0=U, scalar=thr, in1=U,
                                            op0=AL.is_lt, op1=AL.mult)
                for mt in range(2):
                    if e == 0:
                        yt = y_p.tile([P, 384], BF16, name=f"y{t}_{mt}", tag="t")
                        VE.scalar_tensor_tensor(
                            out=yt, in0=rs[:, mt*384:(mt+1)*384],
                            scalar=w_sp[t][:, mt*4:mt*4+1],
                            in1=rs[:, mt*384:(mt+1)*384], op0=AL.mult, op1=AL.bypass)
                        y[t][mt] = yt
                    else:
                        VE.scalar_tensor_tensor(
                            out=y[t][mt], in0=rs[:, mt*384:(mt+1)*384],
                            scalar=w_sp[t][:, mt*4+e:mt*4+e+1],
                            in1=y[t][mt], op0=AL.mult, op1=AL.add)

        # ---------------- phase D: transpose y, proj, LIF, residual ----------------
        ydn = [ydn_p.tile([P, 4 * N], BF16, name=f"ydn{dt}", tag="t") for dt in range(3)]
        xp_t = [xev_p.tile([P, 768], F32, name=f"xp{t}", tag="t") for t in range(T)]
        for tp in range(2):
            for t in (tp * 2, tp * 2 + 1):
                for mt in range(2):
                    for dt in range(3):
                        ptr = ps_m.tile([P, P], BF16, name=f"ptr{t}{mt}{dt}", tag="pm")
                        nc.tensor.transpose(ptr, y[t][mt][:, dt*P:(dt+1)*P], ident)
                        nc.scalar.activation(ydn[dt][:, t*N + mt*P: t*N + (mt+1)*P],
                                             ptr, AF.Copy)
            for mt in range(3):
                pp = ps_m.tile([P, 512], F32, name=f"pp{mt}_{tp}", tag="pm")
                first = True
                for kt in range(3):
                    r_ = ydn[kt][:, tp*512:(tp+1)*512]
                    nc.tensor.matmul(pp, pjh[kt][:, mt*P:(mt+1)*P], r_,
                                     start=first, stop=False)
                    first = False
                    nc.tensor.matmul(pp, pjl[kt][:, mt*P:(mt+1)*P], r_,
                                     start=False, stop=(kt == 2))
                for ti in range(2):
                    t = tp * 2 + ti
                    c = mt * 4 + t
                    nc.scalar.activation(xp_t[t][:, mt*N:(mt+1)*N], pp[:, ti*N:(ti+1)*N],
                                         AF.Identity, bias=b_p[:, c:c+1], scale=a_p[:, c:c+1])
            for t in (tp * 2, tp * 2 + 1):
                thr = float(2.0 ** t)
                U = xp_t[t]
                if t > 0:
                    GE.tensor_add(U, m_p, U)
                if t < T - 1:
                    VE.scalar_tensor_tensor(out=m_p, in0=U, scalar=thr, in1=U,
                                            op0=AL.is_lt, op1=AL.mult)
                for mt in range(3):
                    # x_new overwrites xs in place (residual add)
                    VE.scalar_tensor_tensor(
                        out=_r(xs_kt[mt][:, t*N:(t+1)*N]), in0=U[:, mt*N:(mt+1)*N],
                        scalar=thr, in1=xs_kt[mt][:, t*N:(t+1)*N],
                        op0=AL.is_ge, op1=AL.add)

        # ---------------- phase E: MLP ----------------
        mgs = []
        po0_pool = tc.alloc_tile_pool(name="po0", bufs=3, space="PSUM")
        po0 = [po0_pool.tile([P, 512], F32, name=f"po0_{mt}", tag="po")
               for mt in range(3)]
        with tc.tile_pool(name="dwp", bufs=2, space="PSUM") as dwp:
            dwd_load(0)
            for ch in range(8):
                if ch + 1 < 8:
                    dwd_load(ch + 1)
                xh = xh_p.tile([P, 2048], F32, name=f"xh{ch}", tag="t")
                for half in range(2):
                    mth = ch + 8 * half
                    for tp in range(2):
                        ph = ps_m.tile([P, 512], F32, name=f"ph{ch}{half}{tp}", tag="pm")
                        for kt in range(3):
                            nc.tensor.matmul(ph, _r(f1f[kt][:, mth*P:(mth+1)*P]),
                                             _r(xs_kt[kt][:, tp*512:(tp+1)*512]),
                                             start=(kt == 0), stop=(kt == 2))
                        for ti in range(2):
                            t = tp * 2 + ti
                            c = mth * 4 + t
                            nc.scalar.activation(
                                xh[:, half*1024 + t*N: half*1024 + (t+1)*N],
                                ph[:, ti*N:(ti+1)*N], AF.Identity,
                                bias=b_h[:, c:c+1], scale=a_h[:, c:c+1])
                # h-LIF: adds on GPSIMD, spikes via saturated sigmoid on ScalarE,
                # resets on DVE
                m_h = mh_p.tile([P, 512], F32, name=f"m_h{ch}", tag="t")
                x1p = x1p_ab[ch % 2]
                x1p4 = x1p[:, 20:1316].rearrange("p (t h w) -> p t h w", t=4, h=18)
                x2 = x2_p.tile([P, 1024], BF16, name=f"x2_{ch}", tag="t")
                xh3 = xh.rearrange("p (h q) -> p h q", h=2)
                mh3 = m_h.rearrange("p (h q) -> p h q", h=2)
                for t in range(T):
                    thr = float(2.0 ** t)
                    U3 = xh3[:, :, t*N:(t+1)*N]
                    if t > 0:
                        VE.tensor_tensor(U3, mh3, U3, AL.add)
                    nc.scalar.activation(
                        x1p4[:, t, 1:17, 1:17], xh[:, t*N:(t+1)*N],
                        AF.Sigmoid, bias=sgb[:, t:t+1], scale=K_SG)
                    VE.tensor_single_scalar(x2[:, t*N:(t+1)*N],
                                            xh[:, 1024 + t*N: 1024 + (t+1)*N],
                                            thr, AL.is_ge)
                    if t < T - 1:
                        VE.scalar_tensor_tensor(out=mh3, in0=U3, scalar=thr, in1=U3,
                                                op0=AL.is_lt, op1=AL.mult)
                # depthwise 3x3: per t-plane, 9 shifted diagonal bf16
                # matmuls on the padded plane (zero pads give exact boundary
                # handling, no fixups); one PSUM bank per t-plane chunk
                acc = acc_p.tile([P, 1024], F32, name=f"acc{ch}", tag="t")
                for t in range(T):
                    pacc = dwp.tile([P, 324], F32, name=f"pacc{ch}_{t}", tag="pacc")
                    for dp in range(9):
                        dy, dx = dp // 3, dp % 3
                        dlt = 18 * (dy - 1) + (dx - 1)
                        nc.tensor.matmul(
                            pacc, dwd[ch][:, dp*P:(dp+1)*P],
                            x1p[:, 20 + dlt + t*324: 20 + dlt + (t+1)*324],
                            start=(dp == 0), stop=(dp == 8))
                    pacc3 = pacc.rearrange("p (h w) -> p h w", h=18)
                    nc.scalar.activation(
                        acc[:, t*N:(t+1)*N], pacc3[:, 1:17, 1:17],
                        AF.Identity, bias=bdw_t[:, ch*4+t:ch*4+t+1],
                        scale=float(2.0 ** t))
                # dw-LIF + gate -> mg (bf16 exact {0,1})
                m_dw = mdw_p.tile([P, N], F32, name=f"m_dw{ch}", tag="t")
                mg = mg_p.tile([P, 1024], BF16, name=f"mg{ch}", tag="t")
                for t in range(T):
                    thr = float(2.0 ** t)
                    U = acc[:, t*N:(t+1)*N]
                    if t > 0:
                        VE.tensor_tensor(U, m_dw, U, AL.add)
                    VE.scalar_tensor_tensor(out=mg[:, t*N:(t+1)*N], in0=U, scalar=thr,
                                            in1=x2[:, t*N:(t+1)*N],
                                            op0=AL.is_ge, op1=AL.mult)
                    if t < T - 1:
                        VE.scalar_tensor_tensor(out=m_dw, in0=U, scalar=thr, in1=U,
                                                op0=AL.is_lt, op1=AL.mult)
                mgs.append(mg)
                for mt in range(3):
                    nc.tensor.matmul(po0[mt], f2h[ch][:, mt*P:(mt+1)*P],
                                     mg[:, 0:512], start=(ch == 0), stop=False,
                                     skip_group_check=True)
                    nc.tensor.matmul(po0[mt], f2l[ch][:, mt*P:(mt+1)*P],
                                     mg[:, 0:512], start=False, stop=(ch == 7),
                                     skip_group_check=True)

        # fc2 tp1 (bf16 2-term): ch order reversed so the first PSUM write
        # waits on mgs[7], i.e. after every pacc bank is drained
        with tc.tile_pool(name="ps_o", bufs=3, space="PSUM") as ps_o:
            po1 = [ps_o.tile([P, 512], F32, name=f"po1_{mt}", tag="po")
                   for mt in range(3)]
            po = [po0, po1]
            for ch in range(7, -1, -1):
                for mt in range(3):
                    nc.tensor.matmul(po1[mt], f2h[ch][:, mt*P:(mt+1)*P],
                                     mgs[ch][:, 512:1024],
                                     start=(ch == 7), stop=False,
                                     skip_group_check=True)
                    nc.tensor.matmul(po1[mt], f2l[ch][:, mt*P:(mt+1)*P],
                                     mgs[ch][:, 512:1024],
                                     start=False, stop=(ch == 0),
                                     skip_group_check=True)

            # fc2 evict + final LIF + residual + store
            xo_t = [xev_p.tile([P, 768], F32, name=f"xo{t}", tag="t") for t in range(T)]
            for t in range(T):
                for mt in range(3):
                    c = mt * 4 + t
                    nc.scalar.activation(xo_t[t][:, mt*N:(mt+1)*N],
                                         po[t // 2][mt][:, (t % 2)*N:(t % 2+1)*N],
                                         AF.Identity, bias=b_o[:, c:c+1],
                                         scale=a_o[:, c:c+1])
            for t in range(T):
                thr = float(2.0 ** t)
                for mt in range(3):
                    U = xo_t[t][:, mt*N:(mt+1)*N]
                    if t > 0:
                        GE.tensor_add(U, m_o[mt], U)
                    if t < T - 1:
                        VE.scalar_tensor_tensor(out=m_o[mt], in0=U, scalar=thr, in1=U,
                                                op0=AL.is_lt, op1=AL.mult)
                    # final out in place over xo (reset already consumed U)
                    VE.scalar_tensor_tensor(
                        out=U, in0=U, scalar=thr,
                        in1=xs_kt[mt][:, t*N:(t+1)*N], op0=AL.is_ge, op1=AL.add)
                    nc.sync.dma_start(out=d['out'][t*C + mt*P: t*C + (mt+1)*P, :],
                                      in_=U)
        po0_pool.release()


def _build():
    nc = bacc.Bacc()
    with tile.TileContext(nc) as tc:
        with tc.tile_pool(name="dram", bufs=1, space="DRAM") as dram:
            def din(name, shape, dt=F32):
                return dram.tile(shape, dt, kind="ExternalInput", name=name,
                                 uniquify=False)
            d = {
                'xin': din('xin', [C, 4 * N]),
                'out': dram.tile([T * C, N], F32, kind="ExternalOutput",
                                 name='out', uniquify=False),
                'kq_wf': din('kq_wf', [384, 484]),
                's_kq': din('s_kq', [100, 40]),
                'v_wf': din('v_wf', [384, 384]),
                'pj_w2': din('pj_w2', [384, 768], BF16),
                's_po': din('s_po', [128, 48]),
                'f1_wf': din('f1_wf', [384, 2048]),
                's_h': din('s_h', [128, 132]),
                's_dw': din('s_dw', [128, 32]),
                'f2_w2': din('f2_w2', [1024, 768], BF16),
                'dw_diag': din('dw_diag', [1024, 1152], BF16),
                'ident': din('ident', [128, 128], BF16),
            }
            _body(nc, tc, d)
    nc.finalize()
    return nc


_NC_CACHE = {}


def _get_nc():
    if 'nc' not in _NC_CACHE:
        _NC_CACHE['nc'] = _build()
    return _NC_CACHE['nc']


def _tcols(a):
    rows, k = a.shape
    out = np.empty((rows, k * 4), np.float32)
    for u in range(k):
        for t in range(4):
            out[:, u * 4 + t] = a[:, u] * (2.0 ** t)
    return out


def _split(w):
    hi = w.astype(ml_dtypes.bfloat16)
    lo = (w - hi.astype(np.float32)).astype(ml_dtypes.bfloat16)
    return hi, lo


def _cat2(w):
    hi, lo = _split(w)
    return np.concatenate([hi, lo], axis=1)


def _prep_common(inputs):
    inp = {k: np.asarray(v, np.float32) for k, v in inputs.items()}
    k_wT = inp['k_w'].T
    exp_wT = np.concatenate([inp['exp_w'][e].T for e in range(NE)], axis=1)
    r_wTs = inp['router_w'].T * (inp['router_g'] * S * 0.5)[None, :]
    kq_wT = np.concatenate([k_wT, r_wTs, exp_wT], axis=1)
    a_kq = np.zeros((100, 5), np.float32)
    b_kq = np.zeros((100, 5), np.float32)
    a_kq[0:96, 0] = 0.5
    a_kq[96:100, 0] = 1.0
    b_kq[96:100, 0] = 0.5 * (inp['router_b'] * inp['router_g'] * S
                             + inp['router_be'])
    for e in range(NE):
        a_kq[0:96, 1 + e] = 0.5 * inp['exp_g'][e] * S
        b_kq[0:96, 1 + e] = 0.5 * inp['exp_b'][e]
    taps = inp['dw_w'][:, 0] * (0.5 * inp['dw_g'] * S)[:, None, None]
    tap2 = taps.reshape(8, 128, 9).transpose(1, 0, 2).reshape(128, 72)
    tap_bf = tap2.astype(ml_dtypes.bfloat16)
    dw_diag = np.zeros((8, 128, 9, 128), dtype=ml_dtypes.bfloat16)
    idx = np.arange(128)
    for ch in range(8):
        for dp in range(9):
            dw_diag[ch, idx, dp, idx] = tap_bf[idx, ch * 9 + dp]
    dw_diag = dw_diag.reshape(1024, 1152)
    bdw_base = (0.5 * (inp['dw_b'] * inp['dw_g'] * S
                       + inp['dw_be'])).reshape(8, 128).T
    bdw_t = np.empty((128, 32), np.float32)
    for ch in range(8):
        for t in range(4):
            bdw_t[:, ch * 4 + t] = bdw_base[:, ch] * (2.0 ** t)
    sgb = np.tile((-(2.0 ** 20) * (2.0 ** np.arange(4, dtype=np.float32)))[None, :],
                  (128, 1)).astype(np.float32)

    com = {
        'kq_wf': kq_wT,
        's_kq': np.concatenate([_tcols(a_kq), _tcols(b_kq)], axis=1),
        'v_wf': inp['v_w'].T,
        'pj_w2': _cat2(inp['proj_w'].T),
        's_po': np.concatenate([
            _tcols((0.5 * inp['proj_g'] * S).reshape(3, 128).T),
            _tcols((0.5 * (inp['proj_b'] * inp['proj_g'] * S
                           + inp['proj_be'])).reshape(3, 128).T),
            _tcols((0.5 * inp['fc2_g'] * S).reshape(3, 128).T),
            _tcols((0.5 * (inp['fc2_b'] * inp['fc2_g'] * S
                           + inp['fc2_be'])).reshape(3, 128).T)], axis=1),
        'f1_wf': inp['fc1_w'].T,
        's_h': np.concatenate([
            _tcols((0.5 * inp['fc1_g'] * S).reshape(16, 128).T),
            _tcols((0.5 * (inp['fc1_b'] * inp['fc1_g'] * S
                           + inp['fc1_be'])).reshape(16, 128).T),
            sgb], axis=1),
        's_dw': bdw_t,
        'f2_w2': _cat2(inp['fc2_w'].T),
        'dw_diag': dw_diag,
        'ident': np.eye(128, dtype=ml_dtypes.bfloat16),
    }
    return {k: np.ascontiguousarray(v) for k, v in com.items()}


def run(inputs, trace=False, tmpdir=None):
    com = _prep_common(inputs)
    x = np.asarray(inputs['x'], np.float32).reshape(T, B, C, N)
    in_maps = []
    for b in range(B):
        m = dict(com)
        m['xin'] = np.ascontiguousarray(x[:, b].transpose(1, 0, 2).reshape(C, T * N))
        in_maps.append(m)
    res = run_bass_kernel_spmd(_get_nc(), in_maps, list(range(B)),
                               trace=trace, tmpdir=tmpdir)
    out = np.empty((T, B, C, N), np.float32)
    for b in range(B):
        out[:, b] = res.results[b]['out'].reshape(T, C, N)
    return out.reshape(T * B, C, 16, 16), res.exec_time_ns


def kernel(**inputs):
    out, _ = run(inputs)
    return out


# revision 20
# speedup vs baseline: 1.0240x; 1.0240x over previous
"""Trainium2 Bass kernel for nn_Block_31954556682442 (spiking MoE-SSA block).

Sharding: pure data-parallel over batch B=8 -> one sample (4 LIF time steps)
per NeuronCore, zero collectives. v3 design:
  - weight matmuls in fp32r (PE rounds inputs to ~11 mantissa bits, exact
    fp32 accumulate, full bf16 throughput) -> no hi/lo splits needed
  - time steps batched into matmul free dims (N=512 covers 2 steps)
  - bf16 exact-integer attention core (spikes are {0,1})
  - LIF scans in 2^t-scaled form: membrane update = tensor_add on GPSIMD,
    spike/reset = tensor_scalar/scalar_tensor_tensor on DVE (threshold 2^t)
  - depthwise 3x3 conv t-batched: 9 shifted per-partition-scalar MACs over
    (128, 4*256) tiles on DVE, 2^t applied at the LIF add
  - PSUM evicts fused with BN scale+bias (+2^t*0.5) on ScalarE
Self-contained: hardcodes all shapes; no sibling imports.
"""
import numpy as np
import ml_dtypes

import concourse.bacc as bacc
import concourse.mybir as mybir
import concourse.tile as tile
from concourse.bass_utils import run_bass_kernel_spmd

F32 = mybir.dt.float32
F32R = mybir.dt.float32r
BF16 = mybir.dt.bfloat16
AL = mybir.AluOpType
AF = mybir.ActivationFunctionType

T, B, C, N = 4, 8, 384, 256
ED = 96
NE = 4
NU = 5
HID, HH = 2048, 1024
S = float(1.0 / np.sqrt(1.0 + 1e-5))
P = 128


def _r(ap):
    return ap.bitcast(F32R)


def _body(nc, tc, d):
    from contextlib import ExitStack
    VE = nc.vector
    GE = nc.gpsimd
    K_SG = float(2.0 ** 20)

    with ExitStack() as ctx:
        def pool(name, bufs, space="SBUF"):
            return ctx.enter_context(tc.tile_pool(name=name, bufs=bufs, space=space))

        wp = pool("wp", 1)
        mp = pool("mp", 1)
        ps_m = pool("ps_m", 2, "PSUM")
        xs_p = pool("xs_p", 3)       # (128,1024) f32, doubles as x_new
        xkq_p = pool("xkq_p", 2)     # (96,1280) f32
        xev_p = pool("xev_p", 4)     # (128,768) f32 evict/LIF targets
        sp_p = pool("sp_p", 4)       # (96,1280) bf16 kq spikes
        vsp_p = pool("vsp_p", 4)     # (128,768) bf16
        wsp_p = pool("wsp_p", 4)     # (128,8) f32
        at_p = pool("at_p", 3)       # (128,256) bf16
        rsp_p = pool("rsp_p", 2)     # (128,768) bf16
        y_p = pool("y_p", 8)         # (128,384) bf16
        ydn_p = pool("ydn_p", 3)     # (128,1024) bf16
        xh_p = pool("xh_p", 2)       # (128,2048) f32
        x2_p = pool("x2_p", 2)       # (128,1024) bf16
        acc_p = pool("acc_p", 2)     # (128,1024) f32
        mg_p = pool("mg_p", 8)       # (128,1024) bf16, all 8 persist for fc2
        mh_p = pool("mh_p", 1)       # (128,512) f32
        mdw_p = pool("mdw_p", 1)     # (128,256) f32

        # ---------------- weight loads ----------------
        def wload(name, shape, dt=F32, src=None, rr=False):
            w = wp.tile(shape, dt, name=name, tag=name)
            s_ = d[name] if src is None else src
            nc.sync.dma_start(out=_r(w) if rr else w, in_=_r(s_) if rr else s_)
            return w

        ident = wload('ident', [P, P], BF16)
        # PE warmup: ~60 dummy matmuls to ramp the PE pstate before phase A
        pwarm = ps_m.tile([P, P], F32, name="pwarm", tag="pm")
        for wi in range(60):
            nc.tensor.matmul(pwarm, ident, ident, start=True, stop=True)
        warm_sink = wp.tile([P, 1], F32, name="warm_sink", tag="warm_sink")
        nc.scalar.activation(warm_sink, pwarm[:, 0:1], AF.Copy)

        # xs first (A-phase starts on these)
        xs_kt = []
        for kt in range(3):
            x_ = xs_p.tile([P, 4 * N], F32, name=f"xs{kt}", tag="t")
            xs_kt.append(x_)
        for kt in range(3):
            nc.sync.dma_start(out=_r(xs_kt[kt]), in_=_r(d['xin'][kt*P:(kt+1)*P, :]))
        kqf, vf, f1f = [], [], []
        for kt in range(3):
            kqf.append(wload(f'kqf_{kt}', [P, 484], F32, d['kq_wf'][kt*P:(kt+1)*P, :], rr=True))
        skq = wload('s_kq', [100, 40])
        a_kq, b_kq = skq[:, 0:20], skq[:, 20:40]
        for kt in range(3):
            vf.append(wload(f'vf_{kt}', [P, 384], F32, d['v_wf'][kt*P:(kt+1)*P, :], rr=True))
        pj2 = []
        for kt in range(3):
            pj2.append(wload(f'pj2_{kt}', [P, 768], BF16, d['pj_w2'][kt*P:(kt+1)*P, :]))
        pjh = [w[:, 0:384] for w in pj2]; pjl = [w[:, 384:768] for w in pj2]
        spo = wload('s_po', [P, 48])
        a_p, b_p = spo[:, 0:12], spo[:, 12:24]
        a_o, b_o = spo[:, 24:36], spo[:, 36:48]
        for kt in range(3):
            f1f.append(wload(f'f1f_{kt}', [P, 2048], F32, d['f1_wf'][kt*P:(kt+1)*P, :], rr=True))
        sh = wload('s_h', [P, 132])
        a_h, b_h, sgb = sh[:, 0:64], sh[:, 64:128], sh[:, 128:132]
        sdw = wload('s_dw', [P, 32])
        bdw_t = sdw[:, 0:32]
        f22 = []
        for ch in range(8):
            f22.append(wload(f'f22_{ch}', [P, 768], BF16, d['f2_w2'][ch*P:(ch+1)*P, :]))
        f2h = [w[:, 0:384] for w in f22]; f2l = [w[:, 384:768] for w in f22]
        dwd_p = pool("dwd_p", 3)     # (128,1152) bf16 streamed diag taps
        dwd = {}

        def dwd_load(ch):
            w = dwd_p.tile([P, 1152], BF16, name=f"dwd{ch}", tag="t")
            nc.sync.dma_start(out=w, in_=d['dw_diag'][ch*P:(ch+1)*P, :])
            dwd[ch] = w
        # padded x1 spike tiles: [20 guard | 4 x (18x18) planes | 20 guard]
        x1p_ab = []
        for i in range(2):
            xp_ = wp.tile([P, 1336], BF16, name=f"x1p{i}", tag=f"x1p{i}")
            GE.memzero(xp_)
            x1p_ab.append(xp_)

        # ---------------- phase A: kq / v / router matmuls + evicts ----------------
        m_kq = mp.tile([100, 5 * N], F32, name="m_kq", tag="m_kq")
        m_vt = mp.tile([P, 768], F32, name="m_vt", tag="m_vt")
        m_p = mp.tile([P, 768], F32, name="m_p", tag="m_p")
        m_o = [mp.tile([P, N], F32, name=f"m_o{i}", tag=f"m_o{i}") for i in range(3)]

        xkq_t = [xkq_p.tile([100, 5 * N], F32, name=f"xkq{t}", tag="t") for t in range(T)]
        xvt_t = [xev_p.tile([P, 768], F32, name=f"xvt{t}", tag="t") for t in range(T)]

        for tp in range(2):
            for u in range(NU):
                # u0 block is 100 wide: 96 k-head rows + 4 router rows
                rw = 100 if u == 0 else 96
                w0 = 0 if u == 0 else 100 + 96 * (u - 1)
                pt = ps_m.tile([rw, 512], F32, name=f"pkq{u}_{tp}", tag="pm")
                for kt in range(3):
                    nc.tensor.matmul(pt, _r(kqf[kt][:, w0:w0+rw]),
                                     _r(xs_kt[kt][:, tp*512:(tp+1)*512]),
                                     start=(kt == 0), stop=(kt == 2))
                for ti in range(2):
                    t = tp * 2 + ti
                    c = u * 4 + t
                    nc.scalar.activation(xkq_t[t][0:rw, u*N:(u+1)*N], pt[:, ti*N:(ti+1)*N],
                                         AF.Identity, bias=b_kq[0:rw, c:c+1],
                                         scale=a_kq[0:rw, c:c+1])
        for t in range(T):
            for mt in range(2):
                pv = ps_m.tile([P, 384], F32, name=f"pvt{t}_{mt}", tag="pm")
                for kt in range(3):
                    nc.tensor.matmul(pv, _r(xs_kt[kt][:, t*N + mt*P: t*N + (mt+1)*P]),
                                     _r(vf[kt]), start=(kt == 0), stop=(kt == 2))
                nc.scalar.activation(xvt_t[t][:, mt*384:(mt+1)*384], pv, AF.Copy,
                                     bias=0.0, scale=0.5 * float(2.0 ** t))

        # ---------------- phase B: LIF scans for kq / v / r ----------------
        sp_t, v_sp, w_sp = [], [], []
        for t in range(T):
            thr = float(2.0 ** t)
            U = xkq_t[t]
            if t > 0:
                GE.tensor_add(U, m_kq, U)
            sp = sp_p.tile([100, 5 * N], BF16, name=f"sp{t}", tag="t")
            VE.tensor_single_scalar(sp, U, thr, AL.is_ge)
            if t < T - 1:
                VE.scalar_tensor_tensor(out=m_kq, in0=U, scalar=thr, in1=U,
                                        op0=AL.is_lt, op1=AL.mult)
            sp_t.append(sp)

            U = xvt_t[t]
            if t > 0:
                GE.tensor_add(U, m_vt, U)
            vs = vsp_p.tile([P, 768], BF16, name=f"vsp{t}", tag="t")
            VE.tensor_single_scalar(vs, U, thr, AL.is_ge)
            if t < T - 1:
                VE.scalar_tensor_tensor(out=m_vt, in0=U, scalar=thr, in1=U,
                                        op0=AL.is_lt, op1=AL.mult)
            v_sp.append(vs)

            # router spikes ride in sp rows 96:100 of the u0 block; transpose
            # each position-half to (pos, 4) for the per-partition y weights
            ws = wsp_p.tile([P, 8], F32, name=f"wsp{t}", tag="t")
            ws4 = wsp_p.tile([4, N], BF16, name=f"ws4_{t}", tag="t")
            nc.sync.dma_start(out=ws4, in_=sp[96:100, 0:N])
            for mt in range(2):
                ptw = ps_m.tile([P, 4], BF16, name=f"ptw{t}{mt}", tag="pm")
                nc.tensor.transpose(ptw, ws4[:, mt*P:(mt+1)*P], ident[0:4, 0:4])
                nc.scalar.activation(ws[:, mt*4:(mt+1)*4], ptw, AF.Copy)
            w_sp.append(ws)

        # ---------------- phase C: experts ----------------
        y = [[None] * 2 for _ in range(T)]
        m_res_e = [mp.tile([P, 768], F32, name=f"m_res{e}", tag=f"m_res{e}")
                   for e in range(NE)]
        for t in range(T):
            thr = float(2.0 ** t)
            for e in range(NE):
                m_res = m_res_e[e]
                at_sb = []
                for mt in range(2):
                    pa = ps_m.tile([P, N], F32, name=f"pat{e}{t}{mt}", tag="pm")
                    nc.tensor.matmul(pa, sp_t[t][0:96, mt*P:(mt+1)*P],
                                     sp_t[t][0:96, (1+e)*N:(2+e)*N], start=True, stop=True)
                    ats = at_p.tile([P, N], BF16, name=f"at{e}{t}{mt}", tag="t")
                    nc.scalar.activation(ats, pa, AF.Copy)
                    at_sb.append(ats)
                xr = xev_p.tile([P, 768], F32, name=f"xres{e}{t}", tag="t")
                for mt in range(2):
                    pr_ = ps_m.tile([P, 384], F32, name=f"pres{e}{t}{mt}", tag="pm")
                    for mk in range(2):
                        nc.tensor.matmul(pr_, at_sb[mk][:, mt*P:(mt+1)*P],
                                         v_sp[t][:, mk*384:(mk+1)*384],
                                         start=(mk == 0), stop=(mk == 1))
                    nc.scalar.activation(xr[:, mt*384:(mt+1)*384], pr_, AF.Copy,
                                         bias=0.0, scale=0.5 * thr)
                U = xr
                if t > 0:
                    GE.tensor_add(U, m_res, U)
                rs = rsp_p.tile([P, 768], BF16, name=f"rsp{e}{t}", tag="t")
                VE.tensor_single_scalar(rs, U, thr, AL.is_ge)
                if t < T - 1:
                    VE.scalar_tensor_tensor(out=m_res, in0=U, scalar=thr, in1=U,
                                            op0=AL.is_lt, op1=AL.mult)
                for mt in range(2):
                    if e == 0:
                        yt = y_p.tile([P, 384], BF16, name=f"y{t}_{mt}", tag="t")
                        VE.scalar_tensor_tensor(
                            out=yt, in0=rs[:, mt*384:(mt+1)*384],
                            scalar=w_sp[t][:, mt*4:mt*4+1],
                            in1=rs[:, mt*384:(mt+1)*384], op0=AL.mult, op1=AL.bypass)
                        y[t][mt] = yt
                    else:
                        VE.scalar_tensor_tensor(
                            out=y[t][mt], in0=rs[:, mt*384:(mt+1)*384],
                            scalar=w_sp[t][:, mt*4+e:mt*4+e+1],
                            in1=y[t][mt], op0=AL.mult, op1=AL.add)

        # ---------------- phase D: transpose y, proj, LIF, residual ----------------
        ydn = [ydn_p.tile([P, 4 * N], BF16, name=f"ydn{dt}", tag="t") for dt in range(3)]
        xp_t = [xev_p.tile([P, 768], F32, name=f"xp{t}", tag="t") for t in range(T)]
        for tp in range(2):
            for t in (tp * 2, tp * 2 + 1):
                for mt in range(2):
                    for dt in range(3):
                        ptr = ps_m.tile([P, P], BF16, name=f"ptr{t}{mt}{dt}", tag="pm")
                        nc.tensor.transpose(ptr, y[t][mt][:, dt*P:(dt+1)*P], ident)
                        nc.scalar.activation(ydn[dt][:, t*N + mt*P: t*N + (mt+1)*P],
                                             ptr, AF.Copy)
            for mt in range(3):
                pp = ps_m.tile([P, 512], F32, name=f"pp{mt}_{tp}", tag="pm")
                first = True
                for kt in range(3):
                    r_ = ydn[kt][:, tp*512:(tp+1)*512]
                    nc.tensor.matmul(pp, pjh[kt][:, mt*P:(mt+1)*P], r_,
                                     start=first, stop=False)
                    first = False
                    nc.tensor.matmul(pp, pjl[kt][:, mt*P:(mt+1)*P], r_,
                                     start=False, stop=(kt == 2))
                for ti in range(2):
                    t = tp * 2 + ti
                    c = mt * 4 + t
                    nc.scalar.activation(xp_t[t][:, mt*N:(mt+1)*N], pp[:, ti*N:(ti+1)*N],
                                         AF.Identity, bias=b_p[:, c:c+1], scale=a_p[:, c:c+1])
            for t in (tp * 2, tp * 2 + 1):
                thr = float(2.0 ** t)
                U = xp_t[t]
                if t > 0:
                    GE.tensor_add(U, m_p, U)
                if t < T - 1:
                    VE.scalar_tensor_tensor(out=m_p, in0=U, scalar=thr, in1=U,
                                            op0=AL.is_lt, op1=AL.mult)
                for mt in range(3):
                    # x_new overwrites xs in place (residual add)
                    VE.scalar_tensor_tensor(
                        out=_r(xs_kt[mt][:, t*N:(t+1)*N]), in0=U[:, mt*N:(mt+1)*N],
                        scalar=thr, in1=xs_kt[mt][:, t*N:(t+1)*N],
                        op0=AL.is_ge, op1=AL.add)

        # ---------------- phase E: MLP ----------------
        mgs = []
        po0_pool = tc.alloc_tile_pool(name="po0", bufs=3, space="PSUM")
        po0 = [po0_pool.tile([P, 512], F32, name=f"po0_{mt}", tag="po")
               for mt in range(3)]
        with tc.tile_pool(name="dwp", bufs=2, space="PSUM") as dwp:
            dwd_load(0)
            for ch in range(8):
                if ch + 1 < 8:
                    dwd_load(ch + 1)
                xh = xh_p.tile([P, 2048], F32, name=f"xh{ch}", tag="t")
                for half in range(2):
                    mth = ch + 8 * half
                    for tp in range(2):
                        ph = ps_m.tile([P, 512], F32, name=f"ph{ch}{half}{tp}", tag="pm")
                        for kt in range(3):
                            nc.tensor.matmul(ph, _r(f1f[kt][:, mth*P:(mth+1)*P]),
                                             _r(xs_kt[kt][:, tp*512:(tp+1)*512]),
                                             start=(kt == 0), stop=(kt == 2))
                        for ti in range(2):
                            t = tp * 2 + ti
                            c = mth * 4 + t
                            nc.scalar.activation(
                                xh[:, half*1024 + t*N: half*1024 + (t+1)*N],
                                ph[:, ti*N:(ti+1)*N], AF.Identity,
                                bias=b_h[:, c:c+1], scale=a_h[:, c:c+1])
                # h-LIF: adds on GPSIMD, spikes via saturated sigmoid on ScalarE,
                # resets on DVE
                m_h = mh_p.tile([P, 512], F32, name=f"m_h{ch}", tag="t")
                x1p = x1p_ab[ch % 2]
                x1p4 = x1p[:, 20:1316].rearrange("p (t h w) -> p t h w", t=4, h=18)
                x2 = x2_p.tile([P, 1024], BF16, name=f"x2_{ch}", tag="t")
                xh3 = xh.rearrange("p (h q) -> p h q", h=2)
                mh3 = m_h.rearrange("p (h q) -> p h q", h=2)
                for t in range(T):
                    thr = float(2.0 ** t)
                    U3 = xh3[:, :, t*N:(t+1)*N]
                    if t > 0:
                        VE.tensor_tensor(U3, mh3, U3, AL.add)
                    nc.scalar.activation(
                        x1p4[:, t, 1:17, 1:17], xh[:, t*N:(t+1)*N],
                        AF.Sigmoid, bias=sgb[:, t:t+1], scale=K_SG)
                    VE.tensor_single_scalar(x2[:, t*N:(t+1)*N],
                                            xh[:, 1024 + t*N: 1024 + (t+1)*N],
                                            thr, AL.is_ge)
                    if t < T - 1:
                        VE.scalar_tensor_tensor(out=mh3, in0=U3, scalar=thr, in1=U3,
                                                op0=AL.is_lt, op1=AL.mult)
                # depthwise 3x3: per t-plane, 9 shifted diagonal bf16
                # matmuls on the padded plane (zero pads give exact boundary
                # handling, no fixups); one PSUM bank per t-plane chunk
                acc = acc_p.tile([P, 1024], F32, name=f"acc{ch}", tag="t")
                for t in range(T):
                    pacc = dwp.tile([P, 324], F32, name=f"pacc{ch}_{t}", tag="pacc")
                    for dp in range(9):
                        dy, dx = dp // 3, dp % 3
                        dlt = 18 * (dy - 1) + (dx - 1)
                        nc.tensor.matmul(
                            pacc, dwd[ch][:, dp*P:(dp+1)*P],
                            x1p[:, 20 + dlt + t*324: 20 + dlt + (t+1)*324],
                            start=(dp == 0), stop=(dp == 8))
                    pacc3 = pacc.rearrange("p (h w) -> p h w", h=18)
                    nc.scalar.activation(
                        acc[:, t*N:(t+1)*N], pacc3[:, 1:17, 1:17],
                        AF.Identity, bias=bdw_t[:, ch*4+t:ch*4+t+1],
                        scale=float(2.0 ** t))
                # dw-LIF + gate -> mg (bf16 exact {0,1})
                m_dw = mdw_p.tile([P, N], F32, name=f"m_dw{ch}", tag="t")
                mg = mg_p.tile([P, 1024], BF16, name=f"mg{ch}", tag="t")
                for t in range(T):
                    thr = float(2.0 ** t)
                    U = acc[:, t*N:(t+1)*N]
                    if t > 0:
                        VE.tensor_tensor(U, m_dw, U, AL.add)
                    VE.scalar_tensor_tensor(out=mg[:, t*N:(t+1)*N], in0=U, scalar=thr,
                                            in1=x2[:, t*N:(t+1)*N],
                                            op0=AL.is_ge, op1=AL.mult)
                    if t < T - 1:
                        VE.scalar_tensor_tensor(out=m_dw, in0=U, scalar=thr, in1=U,
                                                op0=AL.is_lt, op1=AL.mult)
                mgs.append(mg)
                for mt in range(3):
                    nc.tensor.matmul(po0[mt], f2h[ch][:, mt*P:(mt+1)*P],
                                     mg[:, 0:512], start=(ch == 0), stop=False,
                                     skip_group_check=True)
                    nc.tensor.matmul(po0[mt], f2l[ch][:, mt*P:(mt+1)*P],
                                     mg[:, 0:512], start=False, stop=(ch == 7),
                                     skip_group_check=True)

        # fc2 tp1 (bf16 2-term): ch order reversed so the first PSUM write
        # waits on mgs[7], i.e. after every pacc bank is drained
        with tc.tile_pool(name="ps_o", bufs=3, space="PSUM") as ps_o:
            po1 = [ps_o.tile([P, 512], F32, name=f"po1_{mt}", tag="po")
                   for mt in range(3)]
            po = [po0, po1]
            for ch in range(7, -1, -1):
                for mt in range(3):
                    nc.tensor.matmul(po1[mt], f2h[ch][:, mt*P:(mt+1)*P],
                                     mgs[ch][:, 512:1024],
                                     start=(ch == 7), stop=False,
                                     skip_group_check=True)
                    nc.tensor.matmul(po1[mt], f2l[ch][:, mt*P:(mt+1)*P],
                                     mgs[ch][:, 512:1024],
                                     start=False, stop=(ch == 0),
                                     skip_group_check=True)

            # fc2 evict + final LIF + residual + store
            xo_t = [xev_p.tile([P, 768], F32, name=f"xo{t}", tag="t") for t in range(T)]
            for t in range(T):
                for mt in range(3):
                    c = mt * 4 + t
                    nc.scalar.activation(xo_t[t][:, mt*N:(mt+1)*N],
                                         po[t // 2][mt][:, (t % 2)*N:(t % 2+1)*N],
                                         AF.Identity, bias=b_o[:, c:c+1],
                                         scale=a_o[:, c:c+1])
            for t in range(T):
                thr = float(2.0 ** t)
                for mt in range(3):
                    U = xo_t[t][:, mt*N:(mt+1)*N]
                    if t > 0:
                        GE.tensor_add(U, m_o[mt], U)
                    if t < T - 1:
                        VE.scalar_tensor_tensor(out=m_o[mt], in0=U, scalar=thr, in1=U,
                                                op0=AL.is_lt, op1=AL.mult)
                    # final out in place over xo (reset already consumed U)
                    VE.scalar_tensor_tensor(
                        out=U, in0=U, scalar=thr,
                        in1=xs_kt[mt][:, t*N:(t+1)*N], op0=AL.is_ge, op1=AL.add)
                    nc.sync.dma_start(out=d['out'][t*C + mt*P: t*C + (mt+1)*P, :],
                                      in_=U)
        po0_pool.release()


def _build():
    nc = bacc.Bacc()
    with tile.TileContext(nc) as tc:
        with tc.tile_pool(name="dram", bufs=1, space="DRAM") as dram:
            def din(name, shape, dt=F32):
                return dram.tile(shape, dt, kind="ExternalInput", name=name,
                                 uniquify=False)
            d = {
                'xin': din('xin', [C, 4 * N]),
                'out': dram.tile([T * C, N], F32, kind="ExternalOutput",
                                 name='out', uniquify=False),
                'kq_wf': din('kq_wf', [384, 484]),
                's_kq': din('s_kq', [100, 40]),
                'v_wf': din('v_wf', [384, 384]),
                'pj_w2': din('pj_w2', [384, 768], BF16),
                's_po': din('s_po', [128, 48]),
                'f1_wf': din('f1_wf', [384, 2048]),
                's_h': din('s_h', [128, 132]),
                's_dw': din('s_dw', [128, 32]),
                'f2_w2': din('f2_w2', [1024, 768], BF16),
                'dw_diag': din('dw_diag', [1024, 1152], BF16),
                'ident': din('ident', [128, 128], BF16),
            }
            _body(nc, tc, d)
    nc.finalize()
    return nc


_NC_CACHE = {}


def _get_nc():
    if 'nc' not in _NC_CACHE:
        _NC_CACHE['nc'] = _build()
    return _NC_CACHE['nc']


def _tcols(a):
    rows, k = a.shape
    out = np.empty((rows, k * 4), np.float32)
    for u in range(k):
        for t in range(4):
            out[:, u * 4 + t] = a[:, u] * (2.0 ** t)
    return out


def _split(w):
    hi = w.astype(ml_dtypes.bfloat16)
    lo = (w - hi.astype(np.float32)).astype(ml_dtypes.bfloat16)
    return hi, lo


def _cat2(w):
    hi, lo = _split(w)
    return np.concatenate([hi, lo], axis=1)


def _prep_common(inputs):
    inp = {k: np.asarray(v, np.float32) for k, v in inputs.items()}
    k_wT = inp['k_w'].T
    exp_wT = np.concatenate([inp['exp_w'][e].T for e in range(NE)], axis=1)
    r_wTs = inp['router_w'].T * (inp['router_g'] * S * 0.5)[None, :]
    kq_wT = np.concatenate([k_wT, r_wTs, exp_wT], axis=1)
    a_kq = np.zeros((100, 5), np.float32)
    b_kq = np.zeros((100, 5), np.float32)
    a_kq[0:96, 0] = 0.5
    a_kq[96:100, 0] = 1.0
    b_kq[96:100, 0] = 0.5 * (inp['router_b'] * inp['router_g'] * S
                             + inp['router_be'])
    for e in range(NE):
        a_kq[0:96, 1 + e] = 0.5 * inp['exp_g'][e] * S
        b_kq[0:96, 1 + e] = 0.5 * inp['exp_b'][e]
    taps = inp['dw_w'][:, 0] * (0.5 * inp['dw_g'] * S)[:, None, None]
    tap2 = taps.reshape(8, 128, 9).transpose(1, 0, 2).reshape(128, 72)
    tap_bf = tap2.astype(ml_dtypes.bfloat16)
    dw_diag = np.zeros((8, 128, 9, 128), dtype=ml_dtypes.bfloat16)
    idx = np.arange(128)
    for ch in range(8):
        for dp in range(9):
            dw_diag[ch, idx, dp, idx] = tap_bf[idx, ch * 9 + dp]
    dw_diag = dw_diag.reshape(1024, 1152)
    bdw_base = (0.5 * (inp['dw_b'] * inp['dw_g'] * S
                       + inp['dw_be'])).reshape(8, 128).T
    bdw_t = np.empty((128, 32), np.float32)
    for ch in range(8):
        for t in range(4):
            bdw_t[:, ch * 4 + t] = bdw_base[:, ch] * (2.0 ** t)
    sgb = np.tile((-(2.0 ** 20) * (2.0 ** np.arange(4, dtype=np.float32)))[None, :],
                  (128, 1)).astype(np.float32)

    com = {
        'kq_wf': kq_wT,
        's_kq': np.concatenate([_tcols(a_kq), _tcols(b_kq)], axis=1),
        'v_wf': inp['v_w'].T,
        'pj_w2': _cat2(inp['proj_w'].T),
        's_po': np.concatenate([
            _tcols((0.5 * inp['proj_g'] * S).reshape(3, 128).T),
            _tcols((0.5 * (inp['proj_b'] * inp['proj_g'] * S
                           + inp['proj_be'])).reshape(3, 128).T),
            _tcols((0.5 * inp['fc2_g'] * S).reshape(3, 128).T),
            _tcols((0.5 * (inp['fc2_b'] * inp['fc2_g'] * S
                           + inp['fc2_be'])).reshape(3, 128).T)], axis=1),
        'f1_wf': inp['fc1_w'].T,
        's_h': np.concatenate([
            _tcols((0.5 * inp['fc1_g'] * S).reshape(16, 128).T),
            _tcols((0.5 * (inp['fc1_b'] * inp['fc1_g'] * S
                           + inp['fc1_be'])).reshape(16, 128).T),
            sgb], axis=1),
        's_dw': bdw_t,
        'f2_w2': _cat2(inp['fc2_w'].T),
        'dw_diag': dw_diag,
        'ident': np.eye(128, dtype=ml_dtypes.bfloat16),
    }
    return {k: np.ascontiguousarray(v) for k, v in com.items()}


def run(inputs, trace=False, tmpdir=None):
    com = _prep_common(inputs)
    x = np.asarray(inputs['x'], np.float32).reshape(T, B, C, N)
    in_maps = []
    for b in range(B):
        m = dict(com)
        m['xin'] = np.ascontiguousarray(x[:, b].transpose(1, 0, 2).reshape(C, T * N))
        in_maps.append(m)
    res = run_bass_kernel_spmd(_get_nc(), in_maps, list(range(B)),
                               trace=trace, tmpdir=tmpdir)
    out = np.empty((T, B, C, N), np.float32)
    for b in range(B):
        out[:, b] = res.results[b]['out'].reshape(T, C, N)
    return out.reshape(T * B, C, 16, 16), res.exec_time_ns


def kernel(**inputs):
    out, _ = run(inputs)
    return out


# revision 21
# speedup vs baseline: 1.0748x; 1.0496x over previous
"""Trainium2 Bass kernel for nn_Block_31954556682442 (spiking MoE-SSA block).

Sharding: pure data-parallel over batch B=8 -> one sample (4 LIF time steps)
per NeuronCore, zero collectives. v3 design:
  - weight matmuls in fp32r (PE rounds inputs to ~11 mantissa bits, exact
    fp32 accumulate, full bf16 throughput) -> no hi/lo splits needed
  - time steps batched into matmul free dims (N=512 covers 2 steps)
  - bf16 exact-integer attention core (spikes are {0,1})
  - LIF scans in 2^t-scaled form: membrane update = tensor_add on GPSIMD,
    spike/reset = tensor_scalar/scalar_tensor_tensor on DVE (threshold 2^t)
  - depthwise 3x3 conv t-batched: 9 shifted per-partition-scalar MACs over
    (128, 4*256) tiles on DVE, 2^t applied at the LIF add
  - PSUM evicts fused with BN scale+bias (+2^t*0.5) on ScalarE
Self-contained: hardcodes all shapes; no sibling imports.
"""
import numpy as np
import ml_dtypes

import concourse.bacc as bacc
import concourse.mybir as mybir
import concourse.tile as tile
from concourse.bass_utils import run_bass_kernel_spmd

F32 = mybir.dt.float32
F32R = mybir.dt.float32r
BF16 = mybir.dt.bfloat16
AL = mybir.AluOpType
AF = mybir.ActivationFunctionType

T, B, C, N = 4, 8, 384, 256
ED = 96
NE = 4
NU = 5
HID, HH = 2048, 1024
S = float(1.0 / np.sqrt(1.0 + 1e-5))
P = 128


def _r(ap):
    return ap.bitcast(F32R)


def _body(nc, tc, d):
    from contextlib import ExitStack
    VE = nc.vector
    GE = nc.gpsimd
    K_SG = float(2.0 ** 20)

    with ExitStack() as ctx:
        def pool(name, bufs, space="SBUF"):
            return ctx.enter_context(tc.tile_pool(name=name, bufs=bufs, space=space))

        wp = pool("wp", 1)
        mp = pool("mp", 1)
        ps_m = pool("ps_m", 2, "PSUM")
        xs_p = pool("xs_p", 3)       # (128,1024) f32, doubles as x_new
        xkq_p = pool("xkq_p", 2)     # (96,1280) f32
        xev_p = pool("xev_p", 4)     # (128,768) f32 evict/LIF targets
        sp_p = pool("sp_p", 4)       # (96,1280) bf16 kq spikes
        vsp_p = pool("vsp_p", 4)     # (128,768) bf16
        wsp_p = pool("wsp_p", 4)     # (128,8) f32
        at_p = pool("at_p", 3)       # (128,256) bf16
        rsp_p = pool("rsp_p", 2)     # (128,768) bf16
        y_p = pool("y_p", 8)         # (128,384) bf16
        ydn_p = pool("ydn_p", 3)     # (128,1024) bf16
        xh_p = pool("xh_p", 2)       # (128,2048) f32
        x2_p = pool("x2_p", 2)       # (128,1024) bf16
        acc_p = pool("acc_p", 2)     # (128,1024) f32
        mg_p = pool("mg_p", 8)       # (128,1024) bf16, all 8 persist for fc2
        mh_p = pool("mh_p", 1)       # (128,512) f32
        mdw_p = pool("mdw_p", 1)     # (128,256) f32

        # ---------------- weight loads ----------------
        def wload(name, shape, dt=F32, src=None, rr=False):
            w = wp.tile(shape, dt, name=name, tag=name)
            s_ = d[name] if src is None else src
            nc.sync.dma_start(out=_r(w) if rr else w, in_=_r(s_) if rr else s_)
            return w

        ident = wload('ident', [P, P], BF16)
        # PE warmup: ~60 dummy matmuls to ramp the PE pstate before phase A
        pwarm = ps_m.tile([P, P], F32, name="pwarm", tag="pm")
        for wi in range(60):
            nc.tensor.matmul(pwarm, ident, ident, start=True, stop=True)
        warm_sink = wp.tile([P, 1], F32, name="warm_sink", tag="warm_sink")
        nc.scalar.activation(warm_sink, pwarm[:, 0:1], AF.Copy)

        # xs first (A-phase starts on these)
        xs_kt = []
        for kt in range(3):
            x_ = xs_p.tile([P, 4 * N], F32, name=f"xs{kt}", tag="t")
            xs_kt.append(x_)
        for kt in range(3):
            nc.sync.dma_start(out=_r(xs_kt[kt]), in_=_r(d['xin'][kt*P:(kt+1)*P, :]))
        kqf, vf, f1f = [], [], []
        for kt in range(3):
            kqf.append(wload(f'kqf_{kt}', [P, 484], F32, d['kq_wf'][kt*P:(kt+1)*P, :], rr=True))
        skq = wload('s_kq', [100, 40])
        a_kq, b_kq = skq[:, 0:20], skq[:, 20:40]
        for kt in range(3):
            vf.append(wload(f'vf_{kt}', [P, 384], F32, d['v_wf'][kt*P:(kt+1)*P, :], rr=True))
        pj2 = []
        for kt in range(3):
            pj2.append(wload(f'pj2_{kt}', [P, 768], BF16, d['pj_w2'][kt*P:(kt+1)*P, :]))
        pjh = [w[:, 0:384] for w in pj2]; pjl = [w[:, 384:768] for w in pj2]
        spo = wload('s_po', [P, 48])
        a_p, b_p = spo[:, 0:12], spo[:, 12:24]
        a_o, b_o = spo[:, 24:36], spo[:, 36:48]
        for kt in range(3):
            f1f.append(wload(f'f1f_{kt}', [P, 2048], F32, d['f1_wf'][kt*P:(kt+1)*P, :], rr=True))
        sh = wload('s_h', [P, 132])
        a_h, b_h, sgb = sh[:, 0:64], sh[:, 64:128], sh[:, 128:132]
        sdw = wload('s_dw', [P, 32])
        bdw_t = sdw[:, 0:32]
        f22 = []
        for ch in range(8):
            f22.append(wload(f'f22_{ch}', [P, 768], BF16, d['f2_w2'][ch*P:(ch+1)*P, :]))
        f2h = [w[:, 0:384] for w in f22]; f2l = [w[:, 384:768] for w in f22]
        dwd_p = pool("dwd_p", 3)     # (128,1152) bf16 streamed diag taps
        dwd = {}

        def dwd_load(ch):
            w = dwd_p.tile([P, 1152], BF16, name=f"dwd{ch}", tag="t")
            nc.sync.dma_start(out=w, in_=d['dw_diag'][ch*P:(ch+1)*P, :])
            dwd[ch] = w
        # padded x1 spike tiles: [20 guard | 4 x (18x18) planes | 20 guard]
        x1p_ab = []
        for i in range(2):
            xp_ = wp.tile([P, 1336], BF16, name=f"x1p{i}", tag=f"x1p{i}")
            GE.memzero(xp_)
            x1p_ab.append(xp_)

        # ---------------- phase A: kq / v / router matmuls + evicts ----------------
        psA = tc.alloc_tile_pool(name="psA", bufs=2, space="PSUM")
        _ps_i = [0]

        def psab():
            _ps_i[0] += 1
            return psA if _ps_i[0] % 2 else ps_m
        m_kq = mp.tile([100, 5 * N], F32, name="m_kq", tag="m_kq")
        m_vt = mp.tile([P, 768], F32, name="m_vt", tag="m_vt")
        m_p = mp.tile([P, 768], F32, name="m_p", tag="m_p")
        m_o = [mp.tile([P, N], F32, name=f"m_o{i}", tag=f"m_o{i}") for i in range(3)]

        xkq_t = [xkq_p.tile([100, 5 * N], F32, name=f"xkq{t}", tag="t") for t in range(T)]
        xvt_t = [xev_p.tile([P, 768], F32, name=f"xvt{t}", tag="t") for t in range(T)]

        for tp in range(2):
            for u in range(NU):
                # u0 block is 100 wide: 96 k-head rows + 4 router rows
                rw = 100 if u == 0 else 96
                w0 = 0 if u == 0 else 100 + 96 * (u - 1)
                pt = psab().tile([rw, 512], F32, name=f"pkq{u}_{tp}", tag="pm")
                for kt in range(3):
                    nc.tensor.matmul(pt, _r(kqf[kt][:, w0:w0+rw]),
                                     _r(xs_kt[kt][:, tp*512:(tp+1)*512]),
                                     start=(kt == 0), stop=(kt == 2))
                for ti in range(2):
                    t = tp * 2 + ti
                    c = u * 4 + t
                    nc.scalar.activation(xkq_t[t][0:rw, u*N:(u+1)*N], pt[:, ti*N:(ti+1)*N],
                                         AF.Identity, bias=b_kq[0:rw, c:c+1],
                                         scale=a_kq[0:rw, c:c+1])
        for t in range(T):
            for mt in range(2):
                pv = psab().tile([P, 384], F32, name=f"pvt{t}_{mt}", tag="pm")
                for kt in range(3):
                    nc.tensor.matmul(pv, _r(xs_kt[kt][:, t*N + mt*P: t*N + (mt+1)*P]),
                                     _r(vf[kt]), start=(kt == 0), stop=(kt == 2))
                nc.scalar.activation(xvt_t[t][:, mt*384:(mt+1)*384], pv, AF.Copy,
                                     bias=0.0, scale=0.5 * float(2.0 ** t))

        # ---------------- phase B: LIF scans for kq / v / r ----------------
        sp_t, v_sp, w_sp = [], [], []
        for t in range(T):
            thr = float(2.0 ** t)
            U = xkq_t[t]
            if t > 0:
                GE.tensor_add(U, m_kq, U)
            sp = sp_p.tile([100, 5 * N], BF16, name=f"sp{t}", tag="t")
            VE.tensor_single_scalar(sp, U, thr, AL.is_ge)
            if t < T - 1:
                VE.scalar_tensor_tensor(out=m_kq, in0=U, scalar=thr, in1=U,
                                        op0=AL.is_lt, op1=AL.mult)
            sp_t.append(sp)

            U = xvt_t[t]
            if t > 0:
                GE.tensor_add(U, m_vt, U)
            vs = vsp_p.tile([P, 768], BF16, name=f"vsp{t}", tag="t")
            VE.tensor_single_scalar(vs, U, thr, AL.is_ge)
            if t < T - 1:
                VE.scalar_tensor_tensor(out=m_vt, in0=U, scalar=thr, in1=U,
                                        op0=AL.is_lt, op1=AL.mult)
            v_sp.append(vs)

            # router spikes ride in sp rows 96:100 of the u0 block; transpose
            # each position-half to (pos, 4) for the per-partition y weights
            ws = wsp_p.tile([P, 8], F32, name=f"wsp{t}", tag="t")
            ws4 = wsp_p.tile([4, N], BF16, name=f"ws4_{t}", tag="t")
            nc.sync.dma_start(out=ws4, in_=sp[96:100, 0:N])
            for mt in range(2):
                ptw = ps_m.tile([P, 4], BF16, name=f"ptw{t}{mt}", tag="pm")
                nc.tensor.transpose(ptw, ws4[:, mt*P:(mt+1)*P], ident[0:4, 0:4])
                nc.scalar.activation(ws[:, mt*4:(mt+1)*4], ptw, AF.Copy)
            w_sp.append(ws)

        # ---------------- phase C: experts ----------------
        y = [[None] * 2 for _ in range(T)]
        m_res_e = [mp.tile([P, 768], F32, name=f"m_res{e}", tag=f"m_res{e}")
                   for e in range(NE)]
        for t in range(T):
            thr = float(2.0 ** t)
            for e in range(NE):
                m_res = m_res_e[e]
                at_sb = []
                for mt in range(2):
                    pa = psab().tile([P, N], F32, name=f"pat{e}{t}{mt}", tag="pm")
                    nc.tensor.matmul(pa, sp_t[t][0:96, mt*P:(mt+1)*P],
                                     sp_t[t][0:96, (1+e)*N:(2+e)*N], start=True, stop=True)
                    ats = at_p.tile([P, N], BF16, name=f"at{e}{t}{mt}", tag="t")
                    nc.scalar.activation(ats, pa, AF.Copy)
                    at_sb.append(ats)
                xr = xev_p.tile([P, 768], F32, name=f"xres{e}{t}", tag="t")
                for mt in range(2):
                    pr_ = psab().tile([P, 384], F32, name=f"pres{e}{t}{mt}", tag="pm")
                    for mk in range(2):
                        nc.tensor.matmul(pr_, at_sb[mk][:, mt*P:(mt+1)*P],
                                         v_sp[t][:, mk*384:(mk+1)*384],
                                         start=(mk == 0), stop=(mk == 1))
                    nc.scalar.activation(xr[:, mt*384:(mt+1)*384], pr_, AF.Copy,
                                         bias=0.0, scale=0.5 * thr)
                U = xr
                if t > 0:
                    GE.tensor_add(U, m_res, U)
                rs = rsp_p.tile([P, 768], BF16, name=f"rsp{e}{t}", tag="t")
                VE.tensor_single_scalar(rs, U, thr, AL.is_ge)
                if t < T - 1:
                    VE.scalar_tensor_tensor(out=m_res, in0=U, scalar=thr, in1=U,
                                            op0=AL.is_lt, op1=AL.mult)
                for mt in range(2):
                    if e == 0:
                        yt = y_p.tile([P, 384], BF16, name=f"y{t}_{mt}", tag="t")
                        VE.scalar_tensor_tensor(
                            out=yt, in0=rs[:, mt*384:(mt+1)*384],
                            scalar=w_sp[t][:, mt*4:mt*4+1],
                            in1=rs[:, mt*384:(mt+1)*384], op0=AL.mult, op1=AL.bypass)
                        y[t][mt] = yt
                    else:
                        VE.scalar_tensor_tensor(
                            out=y[t][mt], in0=rs[:, mt*384:(mt+1)*384],
                            scalar=w_sp[t][:, mt*4+e:mt*4+e+1],
                            in1=y[t][mt], op0=AL.mult, op1=AL.add)

        # ---------------- phase D: transpose y, proj, LIF, residual ----------------
        ydn = [ydn_p.tile([P, 4 * N], BF16, name=f"ydn{dt}", tag="t") for dt in range(3)]
        xp_t = [xev_p.tile([P, 768], F32, name=f"xp{t}", tag="t") for t in range(T)]
        for tp in range(2):
            for t in (tp * 2, tp * 2 + 1):
                for mt in range(2):
                    for dt in range(3):
                        ptr = psab().tile([P, P], BF16, name=f"ptr{t}{mt}{dt}", tag="pm")
                        nc.tensor.transpose(ptr, y[t][mt][:, dt*P:(dt+1)*P], ident)
                        nc.scalar.activation(ydn[dt][:, t*N + mt*P: t*N + (mt+1)*P],
                                             ptr, AF.Copy)
            for mt in range(3):
                pp = psab().tile([P, 512], F32, name=f"pp{mt}_{tp}", tag="pm")
                first = True
                for kt in range(3):
                    r_ = ydn[kt][:, tp*512:(tp+1)*512]
                    nc.tensor.matmul(pp, pjh[kt][:, mt*P:(mt+1)*P], r_,
                                     start=first, stop=False)
                    first = False
                    nc.tensor.matmul(pp, pjl[kt][:, mt*P:(mt+1)*P], r_,
                                     start=False, stop=(kt == 2))
                for ti in range(2):
                    t = tp * 2 + ti
                    c = mt * 4 + t
                    nc.scalar.activation(xp_t[t][:, mt*N:(mt+1)*N], pp[:, ti*N:(ti+1)*N],
                                         AF.Identity, bias=b_p[:, c:c+1], scale=a_p[:, c:c+1])
            for t in (tp * 2, tp * 2 + 1):
                thr = float(2.0 ** t)
                U = xp_t[t]
                if t > 0:
                    GE.tensor_add(U, m_p, U)
                if t < T - 1:
                    VE.scalar_tensor_tensor(out=m_p, in0=U, scalar=thr, in1=U,
                                            op0=AL.is_lt, op1=AL.mult)
                for mt in range(3):
                    # x_new overwrites xs in place (residual add)
                    VE.scalar_tensor_tensor(
                        out=_r(xs_kt[mt][:, t*N:(t+1)*N]), in0=U[:, mt*N:(mt+1)*N],
                        scalar=thr, in1=xs_kt[mt][:, t*N:(t+1)*N],
                        op0=AL.is_ge, op1=AL.add)

        psA.release()
        # ---------------- phase E: MLP ----------------
        mgs = []
        po0_pool = tc.alloc_tile_pool(name="po0", bufs=3, space="PSUM")
        po0 = [po0_pool.tile([P, 512], F32, name=f"po0_{mt}", tag="po")
               for mt in range(3)]
        with tc.tile_pool(name="dwp", bufs=2, space="PSUM") as dwp:
            dwd_load(0)
            for ch in range(8):
                if ch + 1 < 8:
                    dwd_load(ch + 1)
                xh = xh_p.tile([P, 2048], F32, name=f"xh{ch}", tag="t")
                for half in range(2):
                    mth = ch + 8 * half
                    for tp in range(2):
                        ph = ps_m.tile([P, 512], F32, name=f"ph{ch}{half}{tp}", tag="pm")
                        for kt in range(3):
                            nc.tensor.matmul(ph, _r(f1f[kt][:, mth*P:(mth+1)*P]),
                                             _r(xs_kt[kt][:, tp*512:(tp+1)*512]),
                                             start=(kt == 0), stop=(kt == 2))
                        for ti in range(2):
                            t = tp * 2 + ti
                            c = mth * 4 + t
                            nc.scalar.activation(
                                xh[:, half*1024 + t*N: half*1024 + (t+1)*N],
                                ph[:, ti*N:(ti+1)*N], AF.Identity,
                                bias=b_h[:, c:c+1], scale=a_h[:, c:c+1])
                # h-LIF: adds on GPSIMD, spikes via saturated sigmoid on ScalarE,
                # resets on DVE
                m_h = mh_p.tile([P, 512], F32, name=f"m_h{ch}", tag="t")
                x1p = x1p_ab[ch % 2]
                x1p4 = x1p[:, 20:1316].rearrange("p (t h w) -> p t h w", t=4, h=18)
                x2 = x2_p.tile([P, 1024], BF16, name=f"x2_{ch}", tag="t")
                xh3 = xh.rearrange("p (h q) -> p h q", h=2)
                mh3 = m_h.rearrange("p (h q) -> p h q", h=2)
                for t in range(T):
                    thr = float(2.0 ** t)
                    U3 = xh3[:, :, t*N:(t+1)*N]
                    if t > 0:
                        VE.tensor_tensor(U3, mh3, U3, AL.add)
                    nc.scalar.activation(
                        x1p4[:, t, 1:17, 1:17], xh[:, t*N:(t+1)*N],
                        AF.Sigmoid, bias=sgb[:, t:t+1], scale=K_SG)
                    VE.tensor_single_scalar(x2[:, t*N:(t+1)*N],
                                            xh[:, 1024 + t*N: 1024 + (t+1)*N],
                                            thr, AL.is_ge)
                    if t < T - 1:
                        VE.scalar_tensor_tensor(out=mh3, in0=U3, scalar=thr, in1=U3,
                                                op0=AL.is_lt, op1=AL.mult)
                # depthwise 3x3: per t-plane, 9 shifted diagonal bf16
                # matmuls on the padded plane (zero pads give exact boundary
                # handling, no fixups); one PSUM bank per t-plane chunk
                acc = acc_p.tile([P, 1024], F32, name=f"acc{ch}", tag="t")
                for t in range(T):
                    pacc = dwp.tile([P, 324], F32, name=f"pacc{ch}_{t}", tag="pacc")
                    for dp in range(9):
                        dy, dx = dp // 3, dp % 3
                        dlt = 18 * (dy - 1) + (dx - 1)
                        nc.tensor.matmul(
                            pacc, dwd[ch][:, dp*P:(dp+1)*P],
                            x1p[:, 20 + dlt + t*324: 20 + dlt + (t+1)*324],
                            start=(dp == 0), stop=(dp == 8))
                    pacc3 = pacc.rearrange("p (h w) -> p h w", h=18)
                    nc.scalar.activation(
                        acc[:, t*N:(t+1)*N], pacc3[:, 1:17, 1:17],
                        AF.Identity, bias=bdw_t[:, ch*4+t:ch*4+t+1],
                        scale=float(2.0 ** t))
                # dw-LIF + gate -> mg (bf16 exact {0,1})
                m_dw = mdw_p.tile([P, N], F32, name=f"m_dw{ch}", tag="t")
                mg = mg_p.tile([P, 1024], BF16, name=f"mg{ch}", tag="t")
                for t in range(T):
                    thr = float(2.0 ** t)
                    U = acc[:, t*N:(t+1)*N]
                    if t > 0:
                        VE.tensor_tensor(U, m_dw, U, AL.add)
                    VE.scalar_tensor_tensor(out=mg[:, t*N:(t+1)*N], in0=U, scalar=thr,
                                            in1=x2[:, t*N:(t+1)*N],
                                            op0=AL.is_ge, op1=AL.mult)
                    if t < T - 1:
                        VE.scalar_tensor_tensor(out=m_dw, in0=U, scalar=thr, in1=U,
                                                op0=AL.is_lt, op1=AL.mult)
                mgs.append(mg)
                for mt in range(3):
                    nc.tensor.matmul(po0[mt], f2h[ch][:, mt*P:(mt+1)*P],
                                     mg[:, 0:512], start=(ch == 0), stop=False,
                                     skip_group_check=True)
                    nc.tensor.matmul(po0[mt], f2l[ch][:, mt*P:(mt+1)*P],
                                     mg[:, 0:512], start=False, stop=(ch == 7),
                                     skip_group_check=True)

        # fc2 tp1 (bf16 2-term): ch order reversed so the first PSUM write
        # waits on mgs[7], i.e. after every pacc bank is drained
        with tc.tile_pool(name="ps_o", bufs=3, space="PSUM") as ps_o:
            po1 = [ps_o.tile([P, 512], F32, name=f"po1_{mt}", tag="po")
                   for mt in range(3)]
            po = [po0, po1]
            for ch in range(7, -1, -1):
                for mt in range(3):
                    nc.tensor.matmul(po1[mt], f2h[ch][:, mt*P:(mt+1)*P],
                                     mgs[ch][:, 512:1024],
                                     start=(ch == 7), stop=False,
                                     skip_group_check=True)
                    nc.tensor.matmul(po1[mt], f2l[ch][:, mt*P:(mt+1)*P],
                                     mgs[ch][:, 512:1024],
                                     start=False, stop=(ch == 0),
                                     skip_group_check=True)

            # fc2 evict + final LIF + residual + store
            xo_t = [xev_p.tile([P, 768], F32, name=f"xo{t}", tag="t") for t in range(T)]
            for t in range(T):
                for mt in range(3):
                    c = mt * 4 + t
                    nc.scalar.activation(xo_t[t][:, mt*N:(mt+1)*N],
                                         po[t // 2][mt][:, (t % 2)*N:(t % 2+1)*N],
                                         AF.Identity, bias=b_o[:, c:c+1],
                                         scale=a_o[:, c:c+1])
            for t in range(T):
                thr = float(2.0 ** t)
                for mt in range(3):
                    U = xo_t[t][:, mt*N:(mt+1)*N]
                    if t > 0:
                        GE.tensor_add(U, m_o[mt], U)
                    if t < T - 1:
                        VE.scalar_tensor_tensor(out=m_o[mt], in0=U, scalar=thr, in1=U,
                                                op0=AL.is_lt, op1=AL.mult)
                    # final out in place over xo (reset already consumed U)
                    VE.scalar_tensor_tensor(
                        out=U, in0=U, scalar=thr,
                        in1=xs_kt[mt][:, t*N:(t+1)*N], op0=AL.is_ge, op1=AL.add)
                    nc.sync.dma_start(out=d['out'][t*C + mt*P: t*C + (mt+1)*P, :],
                                      in_=U)
        po0_pool.release()


def _build():
    nc = bacc.Bacc()
    with tile.TileContext(nc) as tc:
        with tc.tile_pool(name="dram", bufs=1, space="DRAM") as dram:
            def din(name, shape, dt=F32):
                return dram.tile(shape, dt, kind="ExternalInput", name=name,
                                 uniquify=False)
            d = {
                'xin': din('xin', [C, 4 * N]),
                'out': dram.tile([T * C, N], F32, kind="ExternalOutput",
                                 name='out', uniquify=False),
                'kq_wf': din('kq_wf', [384, 484]),
                's_kq': din('s_kq', [100, 40]),
                'v_wf': din('v_wf', [384, 384]),
                'pj_w2': din('pj_w2', [384, 768], BF16),
                's_po': din('s_po', [128, 48]),
                'f1_wf': din('f1_wf', [384, 2048]),
                's_h': din('s_h', [128, 132]),
                's_dw': din('s_dw', [128, 32]),
                'f2_w2': din('f2_w2', [1024, 768], BF16),
                'dw_diag': din('dw_diag', [1024, 1152], BF16),
                'ident': din('ident', [128, 128], BF16),
            }
            _body(nc, tc, d)
    nc.finalize()
    return nc


_NC_CACHE = {}


def _get_nc():
    if 'nc' not in _NC_CACHE:
        _NC_CACHE['nc'] = _build()
    return _NC_CACHE['nc']


def _tcols(a):
    rows, k = a.shape
    out = np.empty((rows, k * 4), np.float32)
    for u in range(k):
        for t in range(4):
            out[:, u * 4 + t] = a[:, u] * (2.0 ** t)
    return out


def _split(w):
    hi = w.astype(ml_dtypes.bfloat16)
    lo = (w - hi.astype(np.float32)).astype(ml_dtypes.bfloat16)
    return hi, lo


def _cat2(w):
    hi, lo = _split(w)
    return np.concatenate([hi, lo], axis=1)


def _prep_common(inputs):
    inp = {k: np.asarray(v, np.float32) for k, v in inputs.items()}
    k_wT = inp['k_w'].T
    exp_wT = np.concatenate([inp['exp_w'][e].T for e in range(NE)], axis=1)
    r_wTs = inp['router_w'].T * (inp['router_g'] * S * 0.5)[None, :]
    kq_wT = np.concatenate([k_wT, r_wTs, exp_wT], axis=1)
    a_kq = np.zeros((100, 5), np.float32)
    b_kq = np.zeros((100, 5), np.float32)
    a_kq[0:96, 0] = 0.5
    a_kq[96:100, 0] = 1.0
    b_kq[96:100, 0] = 0.5 * (inp['router_b'] * inp['router_g'] * S
                             + inp['router_be'])
    for e in range(NE):
        a_kq[0:96, 1 + e] = 0.5 * inp['exp_g'][e] * S
        b_kq[0:96, 1 + e] = 0.5 * inp['exp_b'][e]
    taps = inp['dw_w'][:, 0] * (0.5 * inp['dw_g'] * S)[:, None, None]
    tap2 = taps.reshape(8, 128, 9).transpose(1, 0, 2).reshape(128, 72)
    tap_bf = tap2.astype(ml_dtypes.bfloat16)
    dw_diag = np.zeros((8, 128, 9, 128), dtype=ml_dtypes.bfloat16)
    idx = np.arange(128)
    for ch in range(8):
        for dp in range(9):
            dw_diag[ch, idx, dp, idx] = tap_bf[idx, ch * 9 + dp]
    dw_diag = dw_diag.reshape(1024, 1152)
    bdw_base = (0.5 * (inp['dw_b'] * inp['dw_g'] * S
                       + inp['dw_be'])).reshape(8, 128).T
    bdw_t = np.empty((128, 32), np.float32)
    for ch in range(8):
        for t in range(4):
            bdw_t[:, ch * 4 + t] = bdw_base[:, ch] * (2.0 ** t)
    sgb = np.tile((-(2.0 ** 20) * (2.0 ** np.arange(4, dtype=np.float32)))[None, :],
                  (128, 1)).astype(np.float32)

    com = {
        'kq_wf': kq_wT,
        's_kq': np.concatenate([_tcols(a_kq), _tcols(b_kq)], axis=1),
        'v_wf': inp['v_w'].T,
        'pj_w2': _cat2(inp['proj_w'].T),
        's_po': np.concatenate([
            _tcols((0.5 * inp['proj_g'] * S).reshape(3, 128).T),
            _tcols((0.5 * (inp['proj_b'] * inp['proj_g'] * S
                           + inp['proj_be'])).reshape(3, 128).T),
            _tcols((0.5 * inp['fc2_g'] * S).reshape(3, 128).T),
            _tcols((0.5 * (inp['fc2_b'] * inp['fc2_g'] * S
                           + inp['fc2_be'])).reshape(3, 128).T)], axis=1),
        'f1_wf': inp['fc1_w'].T,
        's_h': np.concatenate([
            _tcols((0.5 * inp['fc1_g'] * S).reshape(16, 128).T),
            _tcols((0.5 * (inp['fc1_b'] * inp['fc1_g'] * S
                           + inp['fc1_be'])).reshape(16, 128).T),
            sgb], axis=1),
        's_dw': bdw_t,
        'f2_w2': _cat2(inp['fc2_w'].T),
        'dw_diag': dw_diag,
        'ident': np.eye(128, dtype=ml_dtypes.bfloat16),
    }
    return {k: np.ascontiguousarray(v) for k, v in com.items()}


def run(inputs, trace=False, tmpdir=None):
    com = _prep_common(inputs)
    x = np.asarray(inputs['x'], np.float32).reshape(T, B, C, N)
    in_maps = []
    for b in range(B):
        m = dict(com)
        m['xin'] = np.ascontiguousarray(x[:, b].transpose(1, 0, 2).reshape(C, T * N))
        in_maps.append(m)
    res = run_bass_kernel_spmd(_get_nc(), in_maps, list(range(B)),
                               trace=trace, tmpdir=tmpdir)
    out = np.empty((T, B, C, N), np.float32)
    for b in range(B):
        out[:, b] = res.results[b]['out'].reshape(T, C, N)
    return out.reshape(T * B, C, 16, 16), res.exec_time_ns


def kernel(**inputs):
    out, _ = run(inputs)
    return out


# revision 22
# speedup vs baseline: 1.0778x; 1.0028x over previous
"""Trainium2 Bass kernel for nn_Block_31954556682442 (spiking MoE-SSA block).

Sharding: pure data-parallel over batch B=8 -> one sample (4 LIF time steps)
per NeuronCore, zero collectives. v3 design:
  - weight matmuls in fp32r (PE rounds inputs to ~11 mantissa bits, exact
    fp32 accumulate, full bf16 throughput) -> no hi/lo splits needed
  - time steps batched into matmul free dims (N=512 covers 2 steps)
  - bf16 exact-integer attention core (spikes are {0,1})
  - LIF scans in 2^t-scaled form: membrane update = tensor_add on GPSIMD,
    spike/reset = tensor_scalar/scalar_tensor_tensor on DVE (threshold 2^t)
  - depthwise 3x3 conv t-batched: 9 shifted per-partition-scalar MACs over
    (128, 4*256) tiles on DVE, 2^t applied at the LIF add
  - PSUM evicts fused with BN scale+bias (+2^t*0.5) on ScalarE
Self-contained: hardcodes all shapes; no sibling imports.
"""
import numpy as np
import ml_dtypes

import concourse.bacc as bacc
import concourse.mybir as mybir
import concourse.tile as tile
from concourse.bass_utils import run_bass_kernel_spmd

F32 = mybir.dt.float32
F32R = mybir.dt.float32r
BF16 = mybir.dt.bfloat16
AL = mybir.AluOpType
AF = mybir.ActivationFunctionType

T, B, C, N = 4, 8, 384, 256
ED = 96
NE = 4
NU = 5
HID, HH = 2048, 1024
S = float(1.0 / np.sqrt(1.0 + 1e-5))
P = 128


def _r(ap):
    return ap.bitcast(F32R)


def _body(nc, tc, d):
    from contextlib import ExitStack
    VE = nc.vector
    GE = nc.gpsimd
    K_SG = float(2.0 ** 20)

    with ExitStack() as ctx:
        def pool(name, bufs, space="SBUF"):
            return ctx.enter_context(tc.tile_pool(name=name, bufs=bufs, space=space))

        wp = pool("wp", 1)
        mp = pool("mp", 1)
        ps_m = pool("ps_m", 2, "PSUM")
        xs_p = pool("xs_p", 3)       # (128,1024) f32, doubles as x_new
        xkq_p = pool("xkq_p", 2)     # (96,1280) f32
        xev_p = pool("xev_p", 4)     # (128,768) f32 evict/LIF targets
        sp_p = pool("sp_p", 4)       # (96,1280) bf16 kq spikes
        vsp_p = pool("vsp_p", 4)     # (128,768) bf16
        wsp_p = pool("wsp_p", 4)     # (128,8) f32
        at_p = pool("at_p", 3)       # (128,256) bf16
        rsp_p = pool("rsp_p", 2)     # (128,768) bf16
        y_p = pool("y_p", 8)         # (128,384) bf16
        ydn_p = pool("ydn_p", 3)     # (128,1024) bf16
        xh_p = pool("xh_p", 2)       # (128,2048) f32
        x2_p = pool("x2_p", 2)       # (128,1024) bf16
        acc_p = pool("acc_p", 2)     # (128,1024) f32
        mg_p = pool("mg_p", 8)       # (128,1024) bf16, all 8 persist for fc2
        mh_p = pool("mh_p", 1)       # (128,512) f32
        mdw_p = pool("mdw_p", 1)     # (128,256) f32

        # ---------------- weight loads ----------------
        def wload(name, shape, dt=F32, src=None, rr=False):
            w = wp.tile(shape, dt, name=name, tag=name)
            s_ = d[name] if src is None else src
            nc.sync.dma_start(out=_r(w) if rr else w, in_=_r(s_) if rr else s_)
            return w

        ident = wload('ident', [P, P], BF16)
        # PE warmup: ~60 dummy matmuls to ramp the PE pstate before phase A
        pwarm = ps_m.tile([P, P], F32, name="pwarm", tag="pm")
        for wi in range(60):
            nc.tensor.matmul(pwarm, ident, ident, start=True, stop=True)
        warm_sink = wp.tile([P, 1], F32, name="warm_sink", tag="warm_sink")
        nc.scalar.activation(warm_sink, pwarm[:, 0:1], AF.Copy)

        # xs first (A-phase starts on these)
        xs_kt = []
        for kt in range(3):
            x_ = xs_p.tile([P, 4 * N], F32, name=f"xs{kt}", tag="t")
            xs_kt.append(x_)
        for kt in range(3):
            nc.sync.dma_start(out=_r(xs_kt[kt]), in_=_r(d['xin'][kt*P:(kt+1)*P, :]))
        kqf, vf, f1f = [], [], []
        for kt in range(3):
            kqf.append(wload(f'kqf_{kt}', [P, 484], F32, d['kq_wf'][kt*P:(kt+1)*P, :], rr=True))
        skq = wload('s_kq', [100, 40])
        a_kq, b_kq = skq[:, 0:20], skq[:, 20:40]
        for kt in range(3):
            vf.append(wload(f'vf_{kt}', [P, 384], F32, d['v_wf'][kt*P:(kt+1)*P, :], rr=True))
        pj2 = []
        for kt in range(3):
            pj2.append(wload(f'pj2_{kt}', [P, 768], BF16, d['pj_w2'][kt*P:(kt+1)*P, :]))
        pjh = [w[:, 0:384] for w in pj2]; pjl = [w[:, 384:768] for w in pj2]
        spo = wload('s_po', [P, 48])
        a_p, b_p = spo[:, 0:12], spo[:, 12:24]
        a_o, b_o = spo[:, 24:36], spo[:, 36:48]
        for kt in range(3):
            f1f.append(wload(f'f1f_{kt}', [P, 2048], F32, d['f1_wf'][kt*P:(kt+1)*P, :], rr=True))
        sh = wload('s_h', [P, 132])
        a_h, b_h, sgb = sh[:, 0:64], sh[:, 64:128], sh[:, 128:132]
        sdw = wload('s_dw', [P, 32])
        bdw_t = sdw[:, 0:32]
        f22 = []
        for ch in range(8):
            f22.append(wload(f'f22_{ch}', [P, 768], BF16, d['f2_w2'][ch*P:(ch+1)*P, :]))
        f2h = [w[:, 0:384] for w in f22]; f2l = [w[:, 384:768] for w in f22]
        dwd_p = pool("dwd_p", 3)     # (128,1152) bf16 streamed diag taps
        dwd = {}

        def dwd_load(ch):
            w = dwd_p.tile([P, 1152], BF16, name=f"dwd{ch}", tag="t")
            nc.sync.dma_start(out=w, in_=d['dw_diag'][ch*P:(ch+1)*P, :])
            dwd[ch] = w
        # padded x1 spike tiles: [20 guard | 4 x (18x18) planes | 20 guard]
        x1p_ab = []
        for i in range(2):
            xp_ = wp.tile([P, 1336], BF16, name=f"x1p{i}", tag=f"x1p{i}")
            GE.memzero(xp_)
            x1p_ab.append(xp_)

        # ---------------- phase A: kq / v / router matmuls + evicts ----------------
        psA = tc.alloc_tile_pool(name="psA", bufs=4, space="PSUM")
        _ps_i = [0]

        def psab():
            _ps_i[0] += 1
            return psA if _ps_i[0] % 2 else ps_m
        m_kq = mp.tile([100, 5 * N], F32, name="m_kq", tag="m_kq")
        m_vt = mp.tile([P, 768], F32, name="m_vt", tag="m_vt")
        m_p = mp.tile([P, 768], F32, name="m_p", tag="m_p")
        m_o = [mp.tile([P, N], F32, name=f"m_o{i}", tag=f"m_o{i}") for i in range(3)]

        xkq_t = [xkq_p.tile([100, 5 * N], F32, name=f"xkq{t}", tag="t") for t in range(T)]
        xvt_t = [xev_p.tile([P, 768], F32, name=f"xvt{t}", tag="t") for t in range(T)]

        for tp in range(2):
            for u in range(NU):
                # u0 block is 100 wide: 96 k-head rows + 4 router rows
                rw = 100 if u == 0 else 96
                w0 = 0 if u == 0 else 100 + 96 * (u - 1)
                pt = psab().tile([rw, 512], F32, name=f"pkq{u}_{tp}", tag="pm")
                for kt in range(3):
                    nc.tensor.matmul(pt, _r(kqf[kt][:, w0:w0+rw]),
                                     _r(xs_kt[kt][:, tp*512:(tp+1)*512]),
                                     start=(kt == 0), stop=(kt == 2))
                for ti in range(2):
                    t = tp * 2 + ti
                    c = u * 4 + t
                    nc.scalar.activation(xkq_t[t][0:rw, u*N:(u+1)*N], pt[:, ti*N:(ti+1)*N],
                                         AF.Identity, bias=b_kq[0:rw, c:c+1],
                                         scale=a_kq[0:rw, c:c+1])
        for t in range(T):
            for mt in range(2):
                pv = psab().tile([P, 384], F32, name=f"pvt{t}_{mt}", tag="pm")
                for kt in range(3):
                    nc.tensor.matmul(pv, _r(xs_kt[kt][:, t*N + mt*P: t*N + (mt+1)*P]),
                                     _r(vf[kt]), start=(kt == 0), stop=(kt == 2))
                nc.scalar.activation(xvt_t[t][:, mt*384:(mt+1)*384], pv, AF.Copy,
                                     bias=0.0, scale=0.5 * float(2.0 ** t))

        # ---------------- phase B: LIF scans for kq / v / r ----------------
        sp_t, v_sp, w_sp = [], [], []
        for t in range(T):
            thr = float(2.0 ** t)
            U = xkq_t[t]
            if t > 0:
                GE.tensor_add(U, m_kq, U)
            sp = sp_p.tile([100, 5 * N], BF16, name=f"sp{t}", tag="t")
            VE.tensor_single_scalar(sp, U, thr, AL.is_ge)
            if t < T - 1:
                VE.scalar_tensor_tensor(out=m_kq, in0=U, scalar=thr, in1=U,
                                        op0=AL.is_lt, op1=AL.mult)
            sp_t.append(sp)

            U = xvt_t[t]
            if t > 0:
                GE.tensor_add(U, m_vt, U)
            vs = vsp_p.tile([P, 768], BF16, name=f"vsp{t}", tag="t")
            VE.tensor_single_scalar(vs, U, thr, AL.is_ge)
            if t < T - 1:
                VE.scalar_tensor_tensor(out=m_vt, in0=U, scalar=thr, in1=U,
                                        op0=AL.is_lt, op1=AL.mult)
            v_sp.append(vs)

            # router spikes ride in sp rows 96:100 of the u0 block; transpose
            # each position-half to (pos, 4) for the per-partition y weights
            ws = wsp_p.tile([P, 8], F32, name=f"wsp{t}", tag="t")
            ws4 = wsp_p.tile([4, N], BF16, name=f"ws4_{t}", tag="t")
            nc.sync.dma_start(out=ws4, in_=sp[96:100, 0:N])
            for mt in range(2):
                ptw = ps_m.tile([P, 4], BF16, name=f"ptw{t}{mt}", tag="pm")
                nc.tensor.transpose(ptw, ws4[:, mt*P:(mt+1)*P], ident[0:4, 0:4])
                nc.scalar.activation(ws[:, mt*4:(mt+1)*4], ptw, AF.Copy)
            w_sp.append(ws)

        # ---------------- phase C: experts ----------------
        y = [[None] * 2 for _ in range(T)]
        m_res_e = [mp.tile([P, 768], F32, name=f"m_res{e}", tag=f"m_res{e}")
                   for e in range(NE)]
        for t in range(T):
            thr = float(2.0 ** t)
            for e in range(NE):
                m_res = m_res_e[e]
                at_sb = []
                for mt in range(2):
                    pa = psab().tile([P, N], F32, name=f"pat{e}{t}{mt}", tag="pm")
                    nc.tensor.matmul(pa, sp_t[t][0:96, mt*P:(mt+1)*P],
                                     sp_t[t][0:96, (1+e)*N:(2+e)*N], start=True, stop=True)
                    ats = at_p.tile([P, N], BF16, name=f"at{e}{t}{mt}", tag="t")
                    nc.scalar.activation(ats, pa, AF.Copy)
                    at_sb.append(ats)
                xr = xev_p.tile([P, 768], F32, name=f"xres{e}{t}", tag="t")
                for mt in range(2):
                    pr_ = psab().tile([P, 384], F32, name=f"pres{e}{t}{mt}", tag="pm")
                    for mk in range(2):
                        nc.tensor.matmul(pr_, at_sb[mk][:, mt*P:(mt+1)*P],
                                         v_sp[t][:, mk*384:(mk+1)*384],
                                         start=(mk == 0), stop=(mk == 1))
                    nc.scalar.activation(xr[:, mt*384:(mt+1)*384], pr_, AF.Copy,
                                         bias=0.0, scale=0.5 * thr)
                U = xr
                if t > 0:
                    GE.tensor_add(U, m_res, U)
                rs = rsp_p.tile([P, 768], BF16, name=f"rsp{e}{t}", tag="t")
                VE.tensor_single_scalar(rs, U, thr, AL.is_ge)
                if t < T - 1:
                    VE.scalar_tensor_tensor(out=m_res, in0=U, scalar=thr, in1=U,
                                            op0=AL.is_lt, op1=AL.mult)
                for mt in range(2):
                    if e == 0:
                        yt = y_p.tile([P, 384], BF16, name=f"y{t}_{mt}", tag="t")
                        VE.scalar_tensor_tensor(
                            out=yt, in0=rs[:, mt*384:(mt+1)*384],
                            scalar=w_sp[t][:, mt*4:mt*4+1],
                            in1=rs[:, mt*384:(mt+1)*384], op0=AL.mult, op1=AL.bypass)
                        y[t][mt] = yt
                    else:
                        VE.scalar_tensor_tensor(
                            out=y[t][mt], in0=rs[:, mt*384:(mt+1)*384],
                            scalar=w_sp[t][:, mt*4+e:mt*4+e+1],
                            in1=y[t][mt], op0=AL.mult, op1=AL.add)

        # ---------------- phase D: transpose y, proj, LIF, residual ----------------
        ydn = [ydn_p.tile([P, 4 * N], BF16, name=f"ydn{dt}", tag="t") for dt in range(3)]
        xp_t = [xev_p.tile([P, 768], F32, name=f"xp{t}", tag="t") for t in range(T)]
        for tp in range(2):
            for t in (tp * 2, tp * 2 + 1):
                for mt in range(2):
                    for dt in range(3):
                        ptr = psab().tile([P, P], BF16, name=f"ptr{t}{mt}{dt}", tag="pm")
                        nc.tensor.transpose(ptr, y[t][mt][:, dt*P:(dt+1)*P], ident)
                        nc.scalar.activation(ydn[dt][:, t*N + mt*P: t*N + (mt+1)*P],
                                             ptr, AF.Copy)
            for mt in range(3):
                pp = psab().tile([P, 512], F32, name=f"pp{mt}_{tp}", tag="pm")
                first = True
                for kt in range(3):
                    r_ = ydn[kt][:, tp*512:(tp+1)*512]
                    nc.tensor.matmul(pp, pjh[kt][:, mt*P:(mt+1)*P], r_,
                                     start=first, stop=False)
                    first = False
                    nc.tensor.matmul(pp, pjl[kt][:, mt*P:(mt+1)*P], r_,
                                     start=False, stop=(kt == 2))
                for ti in range(2):
                    t = tp * 2 + ti
                    c = mt * 4 + t
                    nc.scalar.activation(xp_t[t][:, mt*N:(mt+1)*N], pp[:, ti*N:(ti+1)*N],
                                         AF.Identity, bias=b_p[:, c:c+1], scale=a_p[:, c:c+1])
            for t in (tp * 2, tp * 2 + 1):
                thr = float(2.0 ** t)
                U = xp_t[t]
                if t > 0:
                    GE.tensor_add(U, m_p, U)
                if t < T - 1:
                    VE.scalar_tensor_tensor(out=m_p, in0=U, scalar=thr, in1=U,
                                            op0=AL.is_lt, op1=AL.mult)
                for mt in range(3):
                    # x_new overwrites xs in place (residual add)
                    VE.scalar_tensor_tensor(
                        out=_r(xs_kt[mt][:, t*N:(t+1)*N]), in0=U[:, mt*N:(mt+1)*N],
                        scalar=thr, in1=xs_kt[mt][:, t*N:(t+1)*N],
                        op0=AL.is_ge, op1=AL.add)

        psA.release()
        # ---------------- phase E: MLP ----------------
        mgs = []
        po0_pool = tc.alloc_tile_pool(name="po0", bufs=3, space="PSUM")
        po0 = [po0_pool.tile([P, 512], F32, name=f"po0_{mt}", tag="po")
               for mt in range(3)]
        with tc.tile_pool(name="dwp", bufs=3, space="PSUM") as dwp:
            dwd_load(0)
            for ch in range(8):
                if ch + 1 < 8:
                    dwd_load(ch + 1)
                xh = xh_p.tile([P, 2048], F32, name=f"xh{ch}", tag="t")
                for half in range(2):
                    mth = ch + 8 * half
                    for tp in range(2):
                        ph = ps_m.tile([P, 512], F32, name=f"ph{ch}{half}{tp}", tag="pm")
                        for kt in range(3):
                            nc.tensor.matmul(ph, _r(f1f[kt][:, mth*P:(mth+1)*P]),
                                             _r(xs_kt[kt][:, tp*512:(tp+1)*512]),
                                             start=(kt == 0), stop=(kt == 2))
                        for ti in range(2):
                            t = tp * 2 + ti
                            c = mth * 4 + t
                            nc.scalar.activation(
                                xh[:, half*1024 + t*N: half*1024 + (t+1)*N],
                                ph[:, ti*N:(ti+1)*N], AF.Identity,
                                bias=b_h[:, c:c+1], scale=a_h[:, c:c+1])
                # h-LIF: adds on GPSIMD, spikes via saturated sigmoid on ScalarE,
                # resets on DVE
                m_h = mh_p.tile([P, 512], F32, name=f"m_h{ch}", tag="t")
                x1p = x1p_ab[ch % 2]
                x1p4 = x1p[:, 20:1316].rearrange("p (t h w) -> p t h w", t=4, h=18)
                x2 = x2_p.tile([P, 1024], BF16, name=f"x2_{ch}", tag="t")
                xh3 = xh.rearrange("p (h q) -> p h q", h=2)
                mh3 = m_h.rearrange("p (h q) -> p h q", h=2)
                for t in range(T):
                    thr = float(2.0 ** t)
                    U3 = xh3[:, :, t*N:(t+1)*N]
                    if t > 0:
                        VE.tensor_tensor(U3, mh3, U3, AL.add)
                    nc.scalar.activation(
                        x1p4[:, t, 1:17, 1:17], xh[:, t*N:(t+1)*N],
                        AF.Sigmoid, bias=sgb[:, t:t+1], scale=K_SG)
                    VE.tensor_single_scalar(x2[:, t*N:(t+1)*N],
                                            xh[:, 1024 + t*N: 1024 + (t+1)*N],
                                            thr, AL.is_ge)
                    if t < T - 1:
                        VE.scalar_tensor_tensor(out=mh3, in0=U3, scalar=thr, in1=U3,
                                                op0=AL.is_lt, op1=AL.mult)
                # depthwise 3x3: per t-plane, 9 shifted diagonal bf16
                # matmuls on the padded plane (zero pads give exact boundary
                # handling, no fixups); one PSUM bank per t-plane chunk
                acc = acc_p.tile([P, 1024], F32, name=f"acc{ch}", tag="t")
                for t in range(T):
                    pacc = dwp.tile([P, 324], F32, name=f"pacc{ch}_{t}", tag="pacc")
                    for dp in range(9):
                        dy, dx = dp // 3, dp % 3
                        dlt = 18 * (dy - 1) + (dx - 1)
                        nc.tensor.matmul(
                            pacc, dwd[ch][:, dp*P:(dp+1)*P],
                            x1p[:, 20 + dlt + t*324: 20 + dlt + (t+1)*324],
                            start=(dp == 0), stop=(dp == 8))
                    pacc3 = pacc.rearrange("p (h w) -> p h w", h=18)
                    nc.scalar.activation(
                        acc[:, t*N:(t+1)*N], pacc3[:, 1:17, 1:17],
                        AF.Identity, bias=bdw_t[:, ch*4+t:ch*4+t+1],
                        scale=float(2.0 ** t))
                # dw-LIF + gate -> mg (bf16 exact {0,1})
                m_dw = mdw_p.tile([P, N], F32, name=f"m_dw{ch}", tag="t")
                mg = mg_p.tile([P, 1024], BF16, name=f"mg{ch}", tag="t")
                for t in range(T):
                    thr = float(2.0 ** t)
                    U = acc[:, t*N:(t+1)*N]
                    if t > 0:
                        VE.tensor_tensor(U, m_dw, U, AL.add)
                    VE.scalar_tensor_tensor(out=mg[:, t*N:(t+1)*N], in0=U, scalar=thr,
                                            in1=x2[:, t*N:(t+1)*N],
                                            op0=AL.is_ge, op1=AL.mult)
                    if t < T - 1:
                        VE.scalar_tensor_tensor(out=m_dw, in0=U, scalar=thr, in1=U,
                                                op0=AL.is_lt, op1=AL.mult)
                mgs.append(mg)
                for mt in range(3):
                    nc.tensor.matmul(po0[mt], f2h[ch][:, mt*P:(mt+1)*P],
                                     mg[:, 0:512], start=(ch == 0), stop=False,
                                     skip_group_check=True)
                    nc.tensor.matmul(po0[mt], f2l[ch][:, mt*P:(mt+1)*P],
                                     mg[:, 0:512], start=False, stop=(ch == 7),
                                     skip_group_check=True)

        # fc2 tp1 (bf16 2-term): ch order reversed so the first PSUM write
        # waits on mgs[7], i.e. after every pacc bank is drained
        with tc.tile_pool(name="ps_o", bufs=3, space="PSUM") as ps_o:
            po1 = [ps_o.tile([P, 512], F32, name=f"po1_{mt}", tag="po")
                   for mt in range(3)]
            po = [po0, po1]
            for ch in range(7, -1, -1):
                for mt in range(3):
                    nc.tensor.matmul(po1[mt], f2h[ch][:, mt*P:(mt+1)*P],
                                     mgs[ch][:, 512:1024],
                                     start=(ch == 7), stop=False,
                                     skip_group_check=True)
                    nc.tensor.matmul(po1[mt], f2l[ch][:, mt*P:(mt+1)*P],
                                     mgs[ch][:, 512:1024],
                                     start=False, stop=(ch == 0),
                                     skip_group_check=True)

            # fc2 evict + final LIF + residual + store
            xo_t = [xev_p.tile([P, 768], F32, name=f"xo{t}", tag="t") for t in range(T)]
            for t in range(T):
                for mt in range(3):
                    c = mt * 4 + t
                    nc.scalar.activation(xo_t[t][:, mt*N:(mt+1)*N],
                                         po[t // 2][mt][:, (t % 2)*N:(t % 2+1)*N],
                                         AF.Identity, bias=b_o[:, c:c+1],
                                         scale=a_o[:, c:c+1])
            for t in range(T):
                thr = float(2.0 ** t)
                for mt in range(3):
                    U = xo_t[t][:, mt*N:(mt+1)*N]
                    if t > 0:
                        GE.tensor_add(U, m_o[mt], U)
                    if t < T - 1:
                        VE.scalar_tensor_tensor(out=m_o[mt], in0=U, scalar=thr, in1=U,
                                                op0=AL.is_lt, op1=AL.mult)
                    # final out in place over xo (reset already consumed U)
                    VE.scalar_tensor_tensor(
                        out=U, in0=U, scalar=thr,
                        in1=xs_kt[mt][:, t*N:(t+1)*N], op0=AL.is_ge, op1=AL.add)
                    nc.sync.dma_start(out=d['out'][t*C + mt*P: t*C + (mt+1)*P, :],
                                      in_=U)
        po0_pool.release()


def _build():
    nc = bacc.Bacc()
    with tile.TileContext(nc) as tc:
        with tc.tile_pool(name="dram", bufs=1, space="DRAM") as dram:
            def din(name, shape, dt=F32):
                return dram.tile(shape, dt, kind="ExternalInput", name=name,
                                 uniquify=False)
            d = {
                'xin': din('xin', [C, 4 * N]),
                'out': dram.tile([T * C, N], F32, kind="ExternalOutput",
                                 name='out', uniquify=False),
                'kq_wf': din('kq_wf', [384, 484]),
                's_kq': din('s_kq', [100, 40]),
                'v_wf': din('v_wf', [384, 384]),
                'pj_w2': din('pj_w2', [384, 768], BF16),
                's_po': din('s_po', [128, 48]),
                'f1_wf': din('f1_wf', [384, 2048]),
                's_h': din('s_h', [128, 132]),
                's_dw': din('s_dw', [128, 32]),
                'f2_w2': din('f2_w2', [1024, 768], BF16),
                'dw_diag': din('dw_diag', [1024, 1152], BF16),
                'ident': din('ident', [128, 128], BF16),
            }
            _body(nc, tc, d)
    nc.finalize()
    return nc


_NC_CACHE = {}


def _get_nc():
    if 'nc' not in _NC_CACHE:
        _NC_CACHE['nc'] = _build()
    return _NC_CACHE['nc']


def _tcols(a):
    rows, k = a.shape
    out = np.empty((rows, k * 4), np.float32)
    for u in range(k):
        for t in range(4):
            out[:, u * 4 + t] = a[:, u] * (2.0 ** t)
    return out


def _split(w):
    hi = w.astype(ml_dtypes.bfloat16)
    lo = (w - hi.astype(np.float32)).astype(ml_dtypes.bfloat16)
    return hi, lo


def _cat2(w):
    hi, lo = _split(w)
    return np.concatenate([hi, lo], axis=1)


def _prep_common(inputs):
    inp = {k: np.asarray(v, np.float32) for k, v in inputs.items()}
    k_wT = inp['k_w'].T
    exp_wT = np.concatenate([inp['exp_w'][e].T for e in range(NE)], axis=1)
    r_wTs = inp['router_w'].T * (inp['router_g'] * S * 0.5)[None, :]
    kq_wT = np.concatenate([k_wT, r_wTs, exp_wT], axis=1)
    a_kq = np.zeros((100, 5), np.float32)
    b_kq = np.zeros((100, 5), np.float32)
    a_kq[0:96, 0] = 0.5
    a_kq[96:100, 0] = 1.0
    b_kq[96:100, 0] = 0.5 * (inp['router_b'] * inp['router_g'] * S
                             + inp['router_be'])
    for e in range(NE):
        a_kq[0:96, 1 + e] = 0.5 * inp['exp_g'][e] * S
        b_kq[0:96, 1 + e] = 0.5 * inp['exp_b'][e]
    taps = inp['dw_w'][:, 0] * (0.5 * inp['dw_g'] * S)[:, None, None]
    tap2 = taps.reshape(8, 128, 9).transpose(1, 0, 2).reshape(128, 72)
    tap_bf = tap2.astype(ml_dtypes.bfloat16)
    dw_diag = np.zeros((8, 128, 9, 128), dtype=ml_dtypes.bfloat16)
    idx = np.arange(128)
    for ch in range(8):
        for dp in range(9):
            dw_diag[ch, idx, dp, idx] = tap_bf[idx, ch * 9 + dp]
    dw_diag = dw_diag.reshape(1024, 1152)
    bdw_base = (0.5 * (inp['dw_b'] * inp['dw_g'] * S
                       + inp['dw_be'])).reshape(8, 128).T
    bdw_t = np.empty((128, 32), np.float32)
    for ch in range(8):
        for t in range(4):
            bdw_t[:, ch * 4 + t] = bdw_base[:, ch] * (2.0 ** t)
    sgb = np.tile((-(2.0 ** 20) * (2.0 ** np.arange(4, dtype=np.float32)))[None, :],
                  (128, 1)).astype(np.float32)

    com = {
        'kq_wf': kq_wT,
        's_kq': np.concatenate([_tcols(a_kq), _tcols(b_kq)], axis=1),
        'v_wf': inp['v_w'].T,
        'pj_w2': _cat2(inp['proj_w'].T),
        's_po': np.concatenate([
            _tcols((0.5 * inp['proj_g'] * S).reshape(3, 128).T),
            _tcols((0.5 * (inp['proj_b'] * inp['proj_g'] * S
                           + inp['proj_be'])).reshape(3, 128).T),
            _tcols((0.5 * inp['fc2_g'] * S).reshape(3, 128).T),
            _tcols((0.5 * (inp['fc2_b'] * inp['fc2_g'] * S
                           + inp['fc2_be'])).reshape(3, 128).T)], axis=1),
        'f1_wf': inp['fc1_w'].T,
        's_h': np.concatenate([
            _tcols((0.5 * inp['fc1_g'] * S).reshape(16, 128).T),
            _tcols((0.5 * (inp['fc1_b'] * inp['fc1_g'] * S
                           + inp['fc1_be'])).reshape(16, 128).T),
            sgb], axis=1),
        's_dw': bdw_t,
        'f2_w2': _cat2(inp['fc2_w'].T),
        'dw_diag': dw_diag,
        'ident': np.eye(128, dtype=ml_dtypes.bfloat16),
    }
    return {k: np.ascontiguousarray(v) for k, v in com.items()}


def run(inputs, trace=False, tmpdir=None):
    com = _prep_common(inputs)
    x = np.asarray(inputs['x'], np.float32).reshape(T, B, C, N)
    in_maps = []
    for b in range(B):
        m = dict(com)
        m['xin'] = np.ascontiguousarray(x[:, b].transpose(1, 0, 2).reshape(C, T * N))
        in_maps.append(m)
    res = run_bass_kernel_spmd(_get_nc(), in_maps, list(range(B)),
                               trace=trace, tmpdir=tmpdir)
    out = np.empty((T, B, C, N), np.float32)
    for b in range(B):
        out[:, b] = res.results[b]['out'].reshape(T, C, N)
    return out.reshape(T * B, C, 16, 16), res.exec_time_ns


def kernel(**inputs):
    out, _ = run(inputs)
    return out


# revision 23
# speedup vs baseline: 1.0937x; 1.0147x over previous
"""Trainium2 Bass kernel for nn_Block_31954556682442 (spiking MoE-SSA block).

Sharding: pure data-parallel over batch B=8 -> one sample (4 LIF time steps)
per NeuronCore, zero collectives. v3 design:
  - weight matmuls in fp32r (PE rounds inputs to ~11 mantissa bits, exact
    fp32 accumulate, full bf16 throughput) -> no hi/lo splits needed
  - time steps batched into matmul free dims (N=512 covers 2 steps)
  - bf16 exact-integer attention core (spikes are {0,1})
  - LIF scans in 2^t-scaled form: membrane update = tensor_add on GPSIMD,
    spike/reset = tensor_scalar/scalar_tensor_tensor on DVE (threshold 2^t)
  - depthwise 3x3 conv t-batched: 9 shifted per-partition-scalar MACs over
    (128, 4*256) tiles on DVE, 2^t applied at the LIF add
  - PSUM evicts fused with BN scale+bias (+2^t*0.5) on ScalarE
Self-contained: hardcodes all shapes; no sibling imports.
"""
import numpy as np
import ml_dtypes

import concourse.bacc as bacc
import concourse.mybir as mybir
import concourse.tile as tile
from concourse.bass_utils import run_bass_kernel_spmd

F32 = mybir.dt.float32
F32R = mybir.dt.float32r
BF16 = mybir.dt.bfloat16
AL = mybir.AluOpType
AF = mybir.ActivationFunctionType

T, B, C, N = 4, 8, 384, 256
ED = 96
NE = 4
NU = 5
HID, HH = 2048, 1024
S = float(1.0 / np.sqrt(1.0 + 1e-5))
P = 128


def _r(ap):
    return ap.bitcast(F32R)


def _body(nc, tc, d):
    from contextlib import ExitStack
    VE = nc.vector
    GE = nc.gpsimd
    K_SG = float(2.0 ** 20)

    with ExitStack() as ctx:
        def pool(name, bufs, space="SBUF"):
            return ctx.enter_context(tc.tile_pool(name=name, bufs=bufs, space=space))

        wp = pool("wp", 1)
        mp = pool("mp", 1)
        ps_m = pool("ps_m", 2, "PSUM")
        xs_p = pool("xs_p", 3)       # (128,1024) f32, doubles as x_new
        xkq_p = pool("xkq_p", 2)     # (96,1280) f32
        xev_p = pool("xev_p", 4)     # (128,768) f32 evict/LIF targets
        sp_p = pool("sp_p", 4)       # (96,1280) bf16 kq spikes
        vsp_p = pool("vsp_p", 4)     # (128,768) bf16
        wsp_p = pool("wsp_p", 4)     # (128,8) f32
        at_p = pool("at_p", 3)       # (128,256) bf16
        rsp_p = pool("rsp_p", 2)     # (128,768) bf16
        y_p = pool("y_p", 8)         # (128,384) bf16
        ydn_p = pool("ydn_p", 3)     # (128,1024) bf16
        xh_p = pool("xh_p", 2)       # (128,2048) f32
        x2_p = pool("x2_p", 2)       # (128,1024) bf16
        acc_p = pool("acc_p", 2)     # (128,1024) f32
        mg_p = pool("mg_p", 8)       # (128,1024) bf16, all 8 persist for fc2
        mh_p = pool("mh_p", 1)       # (128,512) f32
        mdw_p = pool("mdw_p", 1)     # (128,256) f32

        # ---------------- weight loads ----------------
        def wload(name, shape, dt=F32, src=None, rr=False):
            w = wp.tile(shape, dt, name=name, tag=name)
            s_ = d[name] if src is None else src
            nc.sync.dma_start(out=_r(w) if rr else w, in_=_r(s_) if rr else s_)
            return w

        ident = wload('ident', [P, P], BF16)
        # PE warmup: ~60 dummy matmuls to ramp the PE pstate before phase A
        pwarm = ps_m.tile([P, P], F32, name="pwarm", tag="pm")
        for wi in range(60):
            nc.tensor.matmul(pwarm, ident, ident, start=True, stop=True)
        warm_sink = wp.tile([P, 1], F32, name="warm_sink", tag="warm_sink")
        nc.scalar.activation(warm_sink, pwarm[:, 0:1], AF.Copy)

        # xs first (A-phase starts on these)
        xs_kt = []
        for kt in range(3):
            x_ = xs_p.tile([P, 4 * N], F32, name=f"xs{kt}", tag="t")
            xs_kt.append(x_)
        for kt in range(3):
            nc.sync.dma_start(out=_r(xs_kt[kt]), in_=_r(d['xin'][kt*P:(kt+1)*P, :]))
        kqf, vf, f1f = [], [], []
        for kt in range(3):
            kqf.append(wload(f'kqf_{kt}', [P, 484], F32, d['kq_wf'][kt*P:(kt+1)*P, :], rr=True))
        skq = wload('s_kq', [100, 40])
        a_kq, b_kq = skq[:, 0:20], skq[:, 20:40]
        for kt in range(3):
            vf.append(wload(f'vf_{kt}', [P, 384], F32, d['v_wf'][kt*P:(kt+1)*P, :], rr=True))
        pj2 = []
        for kt in range(3):
            pj2.append(wload(f'pj2_{kt}', [P, 768], BF16, d['pj_w2'][kt*P:(kt+1)*P, :]))
        pjh = [w[:, 0:384] for w in pj2]; pjl = [w[:, 384:768] for w in pj2]
        spo = wload('s_po', [P, 48])
        a_p, b_p = spo[:, 0:12], spo[:, 12:24]
        a_o, b_o = spo[:, 24:36], spo[:, 36:48]
        for kt in range(3):
            f1f.append(wload(f'f1f_{kt}', [P, 2048], F32, d['f1_wf'][kt*P:(kt+1)*P, :], rr=True))
        sh = wload('s_h', [P, 132])
        a_h, b_h, sgb = sh[:, 0:64], sh[:, 64:128], sh[:, 128:132]
        sdw = wload('s_dw', [P, 32])
        bdw_t = sdw[:, 0:32]
        f22 = []
        for ch in range(8):
            f22.append(wload(f'f22_{ch}', [P, 768], BF16, d['f2_w2'][ch*P:(ch+1)*P, :]))
        f2h = [w[:, 0:384] for w in f22]; f2l = [w[:, 384:768] for w in f22]
        dwd_p = pool("dwd_p", 3)     # (128,1152) bf16 streamed diag taps
        dwd = {}

        def dwd_load(ch):
            w = dwd_p.tile([P, 1152], BF16, name=f"dwd{ch}", tag="t")
            nc.sync.dma_start(out=w, in_=d['dw_diag'][ch*P:(ch+1)*P, :])
            dwd[ch] = w
        # padded x1 spike tiles: [20 guard | 4 x (18x18) planes | 20 guard]
        x1p_ab = []
        for i in range(2):
            xp_ = wp.tile([P, 1336], BF16, name=f"x1p{i}", tag=f"x1p{i}")
            GE.memzero(xp_)
            x1p_ab.append(xp_)

        # ---------------- phase A: kq / v / router matmuls + evicts ----------------
        psA = tc.alloc_tile_pool(name="psA", bufs=4, space="PSUM")
        _ps_i = [0]

        def psab():
            _ps_i[0] += 1
            return psA if _ps_i[0] % 2 else ps_m
        m_kq = mp.tile([100, 5 * N], F32, name="m_kq", tag="m_kq")
        m_vt = mp.tile([P, 768], F32, name="m_vt", tag="m_vt")
        m_p = mp.tile([P, 768], F32, name="m_p", tag="m_p")
        m_o = [mp.tile([P, N], F32, name=f"m_o{i}", tag=f"m_o{i}") for i in range(3)]

        xkq_t = [xkq_p.tile([100, 5 * N], F32, name=f"xkq{t}", tag="t") for t in range(T)]
        xvt_t = [xev_p.tile([P, 768], F32, name=f"xvt{t}", tag="t") for t in range(T)]

        for tp in range(2):
            for u in range(NU):
                # u0 block is 100 wide: 96 k-head rows + 4 router rows
                rw = 100 if u == 0 else 96
                w0 = 0 if u == 0 else 100 + 96 * (u - 1)
                pt = psab().tile([rw, 512], F32, name=f"pkq{u}_{tp}", tag="pm")
                for kt in range(3):
                    nc.tensor.matmul(pt, _r(kqf[kt][:, w0:w0+rw]),
                                     _r(xs_kt[kt][:, tp*512:(tp+1)*512]),
                                     start=(kt == 0), stop=(kt == 2))
                for ti in range(2):
                    t = tp * 2 + ti
                    c = u * 4 + t
                    nc.scalar.activation(xkq_t[t][0:rw, u*N:(u+1)*N], pt[:, ti*N:(ti+1)*N],
                                         AF.Identity, bias=b_kq[0:rw, c:c+1],
                                         scale=a_kq[0:rw, c:c+1])
        for t in range(T):
            for mt in range(2):
                pv = psab().tile([P, 384], F32, name=f"pvt{t}_{mt}", tag="pm")
                for kt in range(3):
                    nc.tensor.matmul(pv, _r(xs_kt[kt][:, t*N + mt*P: t*N + (mt+1)*P]),
                                     _r(vf[kt]), start=(kt == 0), stop=(kt == 2))
                nc.scalar.activation(xvt_t[t][:, mt*384:(mt+1)*384], pv, AF.Copy,
                                     bias=0.0, scale=0.5 * float(2.0 ** t))

        # ---------------- phase B: LIF scans for kq / v / r ----------------
        sp_t, v_sp, w_sp = [], [], []
        for t in range(T):
            thr = float(2.0 ** t)
            U = xkq_t[t]
            if t > 0:
                VE.tensor_tensor(U, m_kq, U, AL.add)
            sp = sp_p.tile([100, 5 * N], BF16, name=f"sp{t}", tag="t")
            VE.tensor_single_scalar(sp, U, thr, AL.is_ge)
            if t < T - 1:
                VE.scalar_tensor_tensor(out=m_kq, in0=U, scalar=thr, in1=U,
                                        op0=AL.is_lt, op1=AL.mult)
            sp_t.append(sp)

            U = xvt_t[t]
            if t > 0:
                GE.tensor_add(U, m_vt, U)
            vs = vsp_p.tile([P, 768], BF16, name=f"vsp{t}", tag="t")
            VE.tensor_single_scalar(vs, U, thr, AL.is_ge)
            if t < T - 1:
                VE.scalar_tensor_tensor(out=m_vt, in0=U, scalar=thr, in1=U,
                                        op0=AL.is_lt, op1=AL.mult)
            v_sp.append(vs)

            # router spikes ride in sp rows 96:100 of the u0 block; transpose
            # each position-half to (pos, 4) for the per-partition y weights
            ws = wsp_p.tile([P, 8], F32, name=f"wsp{t}", tag="t")
            ws4 = wsp_p.tile([4, N], BF16, name=f"ws4_{t}", tag="t")
            nc.sync.dma_start(out=ws4, in_=sp[96:100, 0:N])
            for mt in range(2):
                ptw = ps_m.tile([P, 4], BF16, name=f"ptw{t}{mt}", tag="pm")
                nc.tensor.transpose(ptw, ws4[:, mt*P:(mt+1)*P], ident[0:4, 0:4])
                nc.scalar.activation(ws[:, mt*4:(mt+1)*4], ptw, AF.Copy)
            w_sp.append(ws)

        # ---------------- phase C: experts ----------------
        y = [[None] * 2 for _ in range(T)]
        m_res_e = [mp.tile([P, 768], F32, name=f"m_res{e}", tag=f"m_res{e}")
                   for e in range(NE)]
        for t in range(T):
            thr = float(2.0 ** t)
            for e in range(NE):
                m_res = m_res_e[e]
                at_sb = []
                for mt in range(2):
                    pa = psab().tile([P, N], F32, name=f"pat{e}{t}{mt}", tag="pm")
                    nc.tensor.matmul(pa, sp_t[t][0:96, mt*P:(mt+1)*P],
                                     sp_t[t][0:96, (1+e)*N:(2+e)*N], start=True, stop=True)
                    ats = at_p.tile([P, N], BF16, name=f"at{e}{t}{mt}", tag="t")
                    nc.scalar.activation(ats, pa, AF.Copy)
                    at_sb.append(ats)
                xr = xev_p.tile([P, 768], F32, name=f"xres{e}{t}", tag="t")
                for mt in range(2):
                    pr_ = psab().tile([P, 384], F32, name=f"pres{e}{t}{mt}", tag="pm")
                    for mk in range(2):
                        nc.tensor.matmul(pr_, at_sb[mk][:, mt*P:(mt+1)*P],
                                         v_sp[t][:, mk*384:(mk+1)*384],
                                         start=(mk == 0), stop=(mk == 1))
                    nc.scalar.activation(xr[:, mt*384:(mt+1)*384], pr_, AF.Copy,
                                         bias=0.0, scale=0.5 * thr)
                U = xr
                if t > 0:
                    GE.tensor_add(U, m_res, U)
                rs = rsp_p.tile([P, 768], BF16, name=f"rsp{e}{t}", tag="t")
                VE.tensor_single_scalar(rs, U, thr, AL.is_ge)
                if t < T - 1:
                    VE.scalar_tensor_tensor(out=m_res, in0=U, scalar=thr, in1=U,
                                            op0=AL.is_lt, op1=AL.mult)
                for mt in range(2):
                    if e == 0:
                        yt = y_p.tile([P, 384], BF16, name=f"y{t}_{mt}", tag="t")
                        VE.scalar_tensor_tensor(
                            out=yt, in0=rs[:, mt*384:(mt+1)*384],
                            scalar=w_sp[t][:, mt*4:mt*4+1],
                            in1=rs[:, mt*384:(mt+1)*384], op0=AL.mult, op1=AL.bypass)
                        y[t][mt] = yt
                    else:
                        VE.scalar_tensor_tensor(
                            out=y[t][mt], in0=rs[:, mt*384:(mt+1)*384],
                            scalar=w_sp[t][:, mt*4+e:mt*4+e+1],
                            in1=y[t][mt], op0=AL.mult, op1=AL.add)

        # ---------------- phase D: transpose y, proj, LIF, residual ----------------
        ydn = [ydn_p.tile([P, 4 * N], BF16, name=f"ydn{dt}", tag="t") for dt in range(3)]
        xp_t = [xev_p.tile([P, 768], F32, name=f"xp{t}", tag="t") for t in range(T)]
        for tp in range(2):
            for t in (tp * 2, tp * 2 + 1):
                for mt in range(2):
                    for dt in range(3):
                        ptr = psab().tile([P, P], BF16, name=f"ptr{t}{mt}{dt}", tag="pm")
                        nc.tensor.transpose(ptr, y[t][mt][:, dt*P:(dt+1)*P], ident)
                        nc.scalar.activation(ydn[dt][:, t*N + mt*P: t*N + (mt+1)*P],
                                             ptr, AF.Copy)
            for mt in range(3):
                pp = psab().tile([P, 512], F32, name=f"pp{mt}_{tp}", tag="pm")
                first = True
                for kt in range(3):
                    r_ = ydn[kt][:, tp*512:(tp+1)*512]
                    nc.tensor.matmul(pp, pjh[kt][:, mt*P:(mt+1)*P], r_,
                                     start=first, stop=False)
                    first = False
                    nc.tensor.matmul(pp, pjl[kt][:, mt*P:(mt+1)*P], r_,
                                     start=False, stop=(kt == 2))
                for ti in range(2):
                    t = tp * 2 + ti
                    c = mt * 4 + t
                    nc.scalar.activation(xp_t[t][:, mt*N:(mt+1)*N], pp[:, ti*N:(ti+1)*N],
                                         AF.Identity, bias=b_p[:, c:c+1], scale=a_p[:, c:c+1])
            for t in (tp * 2, tp * 2 + 1):
                thr = float(2.0 ** t)
                U = xp_t[t]
                if t > 0:
                    GE.tensor_add(U, m_p, U)
                if t < T - 1:
                    VE.scalar_tensor_tensor(out=m_p, in0=U, scalar=thr, in1=U,
                                            op0=AL.is_lt, op1=AL.mult)
                for mt in range(3):
                    # x_new overwrites xs in place (residual add)
                    VE.scalar_tensor_tensor(
                        out=_r(xs_kt[mt][:, t*N:(t+1)*N]), in0=U[:, mt*N:(mt+1)*N],
                        scalar=thr, in1=xs_kt[mt][:, t*N:(t+1)*N],
                        op0=AL.is_ge, op1=AL.add)

        psA.release()
        # ---------------- phase E: MLP ----------------
        mgs = []
        po0_pool = tc.alloc_tile_pool(name="po0", bufs=3, space="PSUM")
        po0 = [po0_pool.tile([P, 512], F32, name=f"po0_{mt}", tag="po")
               for mt in range(3)]
        with tc.tile_pool(name="dwp", bufs=3, space="PSUM") as dwp:
            dwd_load(0)
            for ch in range(8):
                if ch + 1 < 8:
                    dwd_load(ch + 1)
                xh = xh_p.tile([P, 2048], F32, name=f"xh{ch}", tag="t")
                for half in range(2):
                    mth = ch + 8 * half
                    for tp in range(2):
                        ph = ps_m.tile([P, 512], F32, name=f"ph{ch}{half}{tp}", tag="pm")
                        for kt in range(3):
                            nc.tensor.matmul(ph, _r(f1f[kt][:, mth*P:(mth+1)*P]),
                                             _r(xs_kt[kt][:, tp*512:(tp+1)*512]),
                                             start=(kt == 0), stop=(kt == 2))
                        for ti in range(2):
                            t = tp * 2 + ti
                            c = mth * 4 + t
                            nc.scalar.activation(
                                xh[:, half*1024 + t*N: half*1024 + (t+1)*N],
                                ph[:, ti*N:(ti+1)*N], AF.Identity,
                                bias=b_h[:, c:c+1], scale=a_h[:, c:c+1])
                # h-LIF: adds on GPSIMD, spikes via saturated sigmoid on ScalarE,
                # resets on DVE
                m_h = mh_p.tile([P, 512], F32, name=f"m_h{ch}", tag="t")
                x1p = x1p_ab[ch % 2]
                x1p4 = x1p[:, 20:1316].rearrange("p (t h w) -> p t h w", t=4, h=18)
                x2 = x2_p.tile([P, 1024], BF16, name=f"x2_{ch}", tag="t")
                xh3 = xh.rearrange("p (h q) -> p h q", h=2)
                mh3 = m_h.rearrange("p (h q) -> p h q", h=2)
                for t in range(T):
                    thr = float(2.0 ** t)
                    U3 = xh3[:, :, t*N:(t+1)*N]
                    if t > 0:
                        VE.tensor_tensor(U3, mh3, U3, AL.add)
                    nc.scalar.activation(
                        x1p4[:, t, 1:17, 1:17], xh[:, t*N:(t+1)*N],
                        AF.Sigmoid, bias=sgb[:, t:t+1], scale=K_SG)
                    VE.tensor_single_scalar(x2[:, t*N:(t+1)*N],
                                            xh[:, 1024 + t*N: 1024 + (t+1)*N],
                                            thr, AL.is_ge)
                    if t < T - 1:
                        VE.scalar_tensor_tensor(out=mh3, in0=U3, scalar=thr, in1=U3,
                                                op0=AL.is_lt, op1=AL.mult)
                # depthwise 3x3: per t-plane, 9 shifted diagonal bf16
                # matmuls on the padded plane (zero pads give exact boundary
                # handling, no fixups); one PSUM bank per t-plane chunk
                acc = acc_p.tile([P, 1024], F32, name=f"acc{ch}", tag="t")
                for t in range(T):
                    pacc = dwp.tile([P, 324], F32, name=f"pacc{ch}_{t}", tag="pacc")
                    for dp in range(9):
                        dy, dx = dp // 3, dp % 3
                        dlt = 18 * (dy - 1) + (dx - 1)
                        nc.tensor.matmul(
                            pacc, dwd[ch][:, dp*P:(dp+1)*P],
                            x1p[:, 20 + dlt + t*324: 20 + dlt + (t+1)*324],
                            start=(dp == 0), stop=(dp == 8))
                    pacc3 = pacc.rearrange("p (h w) -> p h w", h=18)
                    nc.scalar.activation(
                        acc[:, t*N:(t+1)*N], pacc3[:, 1:17, 1:17],
                        AF.Identity, bias=bdw_t[:, ch*4+t:ch*4+t+1],
                        scale=float(2.0 ** t))
                # dw-LIF + gate -> mg (bf16 exact {0,1})
                m_dw = mdw_p.tile([P, N], F32, name=f"m_dw{ch}", tag="t")
                mg = mg_p.tile([P, 1024], BF16, name=f"mg{ch}", tag="t")
                for t in range(T):
                    thr = float(2.0 ** t)
                    U = acc[:, t*N:(t+1)*N]
                    if t > 0:
                        VE.tensor_tensor(U, m_dw, U, AL.add)
                    VE.scalar_tensor_tensor(out=mg[:, t*N:(t+1)*N], in0=U, scalar=thr,
                                            in1=x2[:, t*N:(t+1)*N],
                                            op0=AL.is_ge, op1=AL.mult)
                    if t < T - 1:
                        VE.scalar_tensor_tensor(out=m_dw, in0=U, scalar=thr, in1=U,
                                                op0=AL.is_lt, op1=AL.mult)
                mgs.append(mg)
                for mt in range(3):
                    nc.tensor.matmul(po0[mt], f2h[ch][:, mt*P:(mt+1)*P],
                                     mg[:, 0:512], start=(ch == 0), stop=False,
                                     skip_group_check=True)
                    nc.tensor.matmul(po0[mt], f2l[ch][:, mt*P:(mt+1)*P],
                                     mg[:, 0:512], start=False, stop=(ch == 7),
                                     skip_group_check=True)

        # fc2 tp1 (bf16 2-term): ch order reversed so the first PSUM write
        # waits on mgs[7], i.e. after every pacc bank is drained
        with tc.tile_pool(name="ps_o", bufs=3, space="PSUM") as ps_o:
            po1 = [ps_o.tile([P, 512], F32, name=f"po1_{mt}", tag="po")
                   for mt in range(3)]
            po = [po0, po1]
            for ch in range(7, -1, -1):
                for mt in range(3):
                    nc.tensor.matmul(po1[mt], f2h[ch][:, mt*P:(mt+1)*P],
                                     mgs[ch][:, 512:1024],
                                     start=(ch == 7), stop=False,
                                     skip_group_check=True)
                    nc.tensor.matmul(po1[mt], f2l[ch][:, mt*P:(mt+1)*P],
                                     mgs[ch][:, 512:1024],
                                     start=False, stop=(ch == 0),
                                     skip_group_check=True)

            # fc2 evict + final LIF + residual + store
            xo_t = [xev_p.tile([P, 768], F32, name=f"xo{t}", tag="t") for t in range(T)]
            for t in range(T):
                for mt in range(3):
                    c = mt * 4 + t
                    nc.scalar.activation(xo_t[t][:, mt*N:(mt+1)*N],
                                         po[t // 2][mt][:, (t % 2)*N:(t % 2+1)*N],
                                         AF.Identity, bias=b_o[:, c:c+1],
                                         scale=a_o[:, c:c+1])
            for t in range(T):
                thr = float(2.0 ** t)
                for mt in range(3):
                    U = xo_t[t][:, mt*N:(mt+1)*N]
                    if t > 0:
                        GE.tensor_add(U, m_o[mt], U)
                    if t < T - 1:
                        VE.scalar_tensor_tensor(out=m_o[mt], in0=U, scalar=thr, in1=U,
                                                op0=AL.is_lt, op1=AL.mult)
                    # final out in place over xo (reset already consumed U)
                    VE.scalar_tensor_tensor(
                        out=U, in0=U, scalar=thr,
                        in1=xs_kt[mt][:, t*N:(t+1)*N], op0=AL.is_ge, op1=AL.add)
                    nc.sync.dma_start(out=d['out'][t*C + mt*P: t*C + (mt+1)*P, :],
                                      in_=U)
        po0_pool.release()


def _build():
    nc = bacc.Bacc()
    with tile.TileContext(nc) as tc:
        with tc.tile_pool(name="dram", bufs=1, space="DRAM") as dram:
            def din(name, shape, dt=F32):
                return dram.tile(shape, dt, kind="ExternalInput", name=name,
                                 uniquify=False)
            d = {
                'xin': din('xin', [C, 4 * N]),
                'out': dram.tile([T * C, N], F32, kind="ExternalOutput",
                                 name='out', uniquify=False),
                'kq_wf': din('kq_wf', [384, 484]),
                's_kq': din('s_kq', [100, 40]),
                'v_wf': din('v_wf', [384, 384]),
                'pj_w2': din('pj_w2', [384, 768], BF16),
                's_po': din('s_po', [128, 48]),
                'f1_wf': din('f1_wf', [384, 2048]),
                's_h': din('s_h', [128, 132]),
                's_dw': din('s_dw', [128, 32]),
                'f2_w2': din('f2_w2', [1024, 768], BF16),
                'dw_diag': din('dw_diag', [1024, 1152], BF16),
                'ident': din('ident', [128, 128], BF16),
            }
            _body(nc, tc, d)
    nc.finalize()
    return nc


_NC_CACHE = {}


def _get_nc():
    if 'nc' not in _NC_CACHE:
        _NC_CACHE['nc'] = _build()
    return _NC_CACHE['nc']


def _tcols(a):
    rows, k = a.shape
    out = np.empty((rows, k * 4), np.float32)
    for u in range(k):
        for t in range(4):
            out[:, u * 4 + t] = a[:, u] * (2.0 ** t)
    return out


def _split(w):
    hi = w.astype(ml_dtypes.bfloat16)
    lo = (w - hi.astype(np.float32)).astype(ml_dtypes.bfloat16)
    return hi, lo


def _cat2(w):
    hi, lo = _split(w)
    return np.concatenate([hi, lo], axis=1)


def _prep_common(inputs):
    inp = {k: np.asarray(v, np.float32) for k, v in inputs.items()}
    k_wT = inp['k_w'].T
    exp_wT = np.concatenate([inp['exp_w'][e].T for e in range(NE)], axis=1)
    r_wTs = inp['router_w'].T * (inp['router_g'] * S * 0.5)[None, :]
    kq_wT = np.concatenate([k_wT, r_wTs, exp_wT], axis=1)
    a_kq = np.zeros((100, 5), np.float32)
    b_kq = np.zeros((100, 5), np.float32)
    a_kq[0:96, 0] = 0.5
    a_kq[96:100, 0] = 1.0
    b_kq[96:100, 0] = 0.5 * (inp['router_b'] * inp['router_g'] * S
                             + inp['router_be'])
    for e in range(NE):
        a_kq[0:96, 1 + e] = 0.5 * inp['exp_g'][e] * S
        b_kq[0:96, 1 + e] = 0.5 * inp['exp_b'][e]
    taps = inp['dw_w'][:, 0] * (0.5 * inp['dw_g'] * S)[:, None, None]
    tap2 = taps.reshape(8, 128, 9).transpose(1, 0, 2).reshape(128, 72)
    tap_bf = tap2.astype(ml_dtypes.bfloat16)
    dw_diag = np.zeros((8, 128, 9, 128), dtype=ml_dtypes.bfloat16)
    idx = np.arange(128)
    for ch in range(8):
        for dp in range(9):
            dw_diag[ch, idx, dp, idx] = tap_bf[idx, ch * 9 + dp]
    dw_diag = dw_diag.reshape(1024, 1152)
    bdw_base = (0.5 * (inp['dw_b'] * inp['dw_g'] * S
                       + inp['dw_be'])).reshape(8, 128).T
    bdw_t = np.empty((128, 32), np.float32)
    for ch in range(8):
        for t in range(4):
            bdw_t[:, ch * 4 + t] = bdw_base[:, ch] * (2.0 ** t)
    sgb = np.tile((-(2.0 ** 20) * (2.0 ** np.arange(4, dtype=np.float32)))[None, :],
                  (128, 1)).astype(np.float32)

    com = {
        'kq_wf': kq_wT,
        's_kq': np.concatenate([_tcols(a_kq), _tcols(b_kq)], axis=1),
        'v_wf': inp['v_w'].T,
        'pj_w2': _cat2(inp['proj_w'].T),
        's_po': np.concatenate([
            _tcols((0.5 * inp['proj_g'] * S).reshape(3, 128).T),
            _tcols((0.5 * (inp['proj_b'] * inp['proj_g'] * S
                           + inp['proj_be'])).reshape(3, 128).T),
            _tcols((0.5 * inp['fc2_g'] * S).reshape(3, 128).T),
            _tcols((0.5 * (inp['fc2_b'] * inp['fc2_g'] * S
                           + inp['fc2_be'])).reshape(3, 128).T)], axis=1),
        'f1_wf': inp['fc1_w'].T,
        's_h': np.concatenate([
            _tcols((0.5 * inp['fc1_g'] * S).reshape(16, 128).T),
            _tcols((0.5 * (inp['fc1_b'] * inp['fc1_g'] * S
                           + inp['fc1_be'])).reshape(16, 128).T),
            sgb], axis=1),
        's_dw': bdw_t,
        'f2_w2': _cat2(inp['fc2_w'].T),
        'dw_diag': dw_diag,
        'ident': np.eye(128, dtype=ml_dtypes.bfloat16),
    }
    return {k: np.ascontiguousarray(v) for k, v in com.items()}


def run(inputs, trace=False, tmpdir=None):
    com = _prep_common(inputs)
    x = np.asarray(inputs['x'], np.float32).reshape(T, B, C, N)
    in_maps = []
    for b in range(B):
        m = dict(com)
        m['xin'] = np.ascontiguousarray(x[:, b].transpose(1, 0, 2).reshape(C, T * N))
        in_maps.append(m)
    res = run_bass_kernel_spmd(_get_nc(), in_maps, list(range(B)),
                               trace=trace, tmpdir=tmpdir)
    out = np.empty((T, B, C, N), np.float32)
    for b in range(B):
        out[:, b] = res.results[b]['out'].reshape(T, C, N)
    return out.reshape(T * B, C, 16, 16), res.exec_time_ns


def kernel(**inputs):
    out, _ = run(inputs)
    return out


# revision 24
# speedup vs baseline: 1.1021x; 1.0077x over previous
"""Trainium2 Bass kernel for nn_Block_31954556682442 (spiking MoE-SSA block).

Sharding: pure data-parallel over batch B=8 -> one sample (4 LIF time steps)
per NeuronCore, zero collectives. v3 design:
  - weight matmuls in fp32r (PE rounds inputs to ~11 mantissa bits, exact
    fp32 accumulate, full bf16 throughput) -> no hi/lo splits needed
  - time steps batched into matmul free dims (N=512 covers 2 steps)
  - bf16 exact-integer attention core (spikes are {0,1})
  - LIF scans in 2^t-scaled form: membrane update = tensor_add on GPSIMD,
    spike/reset = tensor_scalar/scalar_tensor_tensor on DVE (threshold 2^t)
  - depthwise 3x3 conv t-batched: 9 shifted per-partition-scalar MACs over
    (128, 4*256) tiles on DVE, 2^t applied at the LIF add
  - PSUM evicts fused with BN scale+bias (+2^t*0.5) on ScalarE
Self-contained: hardcodes all shapes; no sibling imports.
"""
import numpy as np
import ml_dtypes

import concourse.bacc as bacc
import concourse.mybir as mybir
import concourse.tile as tile
from concourse.bass_utils import run_bass_kernel_spmd

F32 = mybir.dt.float32
F32R = mybir.dt.float32r
BF16 = mybir.dt.bfloat16
AL = mybir.AluOpType
AF = mybir.ActivationFunctionType

T, B, C, N = 4, 8, 384, 256
ED = 96
NE = 4
NU = 5
HID, HH = 2048, 1024
S = float(1.0 / np.sqrt(1.0 + 1e-5))
P = 128


def _r(ap):
    return ap.bitcast(F32R)


def _body(nc, tc, d):
    from contextlib import ExitStack
    VE = nc.vector
    GE = nc.gpsimd
    K_SG = float(2.0 ** 20)

    with ExitStack() as ctx:
        def pool(name, bufs, space="SBUF"):
            return ctx.enter_context(tc.tile_pool(name=name, bufs=bufs, space=space))

        wp = pool("wp", 1)
        mp = pool("mp", 1)
        ps_m = pool("ps_m", 2, "PSUM")
        xs_p = pool("xs_p", 3)       # (128,1024) f32, doubles as x_new
        xkq_p = pool("xkq_p", 2)     # (96,1280) f32
        xev_p = pool("xev_p", 4)     # (128,768) f32 evict/LIF targets
        sp_p = pool("sp_p", 4)       # (96,1280) bf16 kq spikes
        vsp_p = pool("vsp_p", 4)     # (128,768) bf16
        wsp_p = pool("wsp_p", 4)     # (128,8) f32
        at_p = pool("at_p", 3)       # (128,256) bf16
        rsp_p = pool("rsp_p", 2)     # (128,768) bf16
        y_p = pool("y_p", 8)         # (128,384) bf16
        ydn_p = pool("ydn_p", 3)     # (128,1024) bf16
        xh_p = pool("xh_p", 2)       # (128,2048) f32
        x2_p = pool("x2_p", 2)       # (128,1024) bf16
        acc_p = pool("acc_p", 2)     # (128,1024) f32
        mg_p = pool("mg_p", 8)       # (128,1024) bf16, all 8 persist for fc2
        mh_p = pool("mh_p", 1)       # (128,512) f32
        mdw_p = pool("mdw_p", 1)     # (128,256) f32

        # ---------------- weight loads ----------------
        def wload(name, shape, dt=F32, src=None, rr=False):
            w = wp.tile(shape, dt, name=name, tag=name)
            s_ = d[name] if src is None else src
            nc.sync.dma_start(out=_r(w) if rr else w, in_=_r(s_) if rr else s_)
            return w

        ident = wload('ident', [P, P], BF16)
        # PE warmup: ~60 dummy matmuls to ramp the PE pstate before phase A
        pwarm = ps_m.tile([P, P], F32, name="pwarm", tag="pm")
        for wi in range(60):
            nc.tensor.matmul(pwarm, ident, ident, start=True, stop=True)
        warm_sink = wp.tile([P, 1], F32, name="warm_sink", tag="warm_sink")
        nc.scalar.activation(warm_sink, pwarm[:, 0:1], AF.Copy)

        # xs first (A-phase starts on these)
        xs_kt = []
        for kt in range(3):
            x_ = xs_p.tile([P, 4 * N], F32, name=f"xs{kt}", tag="t")
            xs_kt.append(x_)
        for kt in range(3):
            nc.sync.dma_start(out=_r(xs_kt[kt]), in_=_r(d['xin'][kt*P:(kt+1)*P, :]))
        kqf, vf, f1f = [], [], []
        for kt in range(3):
            kqf.append(wload(f'kqf_{kt}', [P, 484], F32, d['kq_wf'][kt*P:(kt+1)*P, :], rr=True))
        skq = wload('s_kq', [100, 40])
        a_kq, b_kq = skq[:, 0:20], skq[:, 20:40]
        for kt in range(3):
            vf.append(wload(f'vf_{kt}', [P, 384], F32, d['v_wf'][kt*P:(kt+1)*P, :], rr=True))
        pj2 = []
        for kt in range(3):
            pj2.append(wload(f'pj2_{kt}', [P, 768], BF16, d['pj_w2'][kt*P:(kt+1)*P, :]))
        pjh = [w[:, 0:384] for w in pj2]; pjl = [w[:, 384:768] for w in pj2]
        spo = wload('s_po', [P, 48])
        a_p, b_p = spo[:, 0:12], spo[:, 12:24]
        a_o, b_o = spo[:, 24:36], spo[:, 36:48]
        for kt in range(3):
            f1f.append(wload(f'f1f_{kt}', [P, 2048], F32, d['f1_wf'][kt*P:(kt+1)*P, :], rr=True))
        sh = wload('s_h', [P, 132])
        a_h, b_h, sgb = sh[:, 0:64], sh[:, 64:128], sh[:, 128:132]
        sdw = wload('s_dw', [P, 32])
        bdw_t = sdw[:, 0:32]
        f22 = []
        for ch in range(8):
            f22.append(wload(f'f22_{ch}', [P, 768], BF16, d['f2_w2'][ch*P:(ch+1)*P, :]))
        f2h = [w[:, 0:384] for w in f22]; f2l = [w[:, 384:768] for w in f22]
        dwd_p = pool("dwd_p", 3)     # (128,1152) bf16 streamed diag taps
        dwd = {}

        def dwd_load(ch):
            w = dwd_p.tile([P, 1152], BF16, name=f"dwd{ch}", tag="t")
            nc.sync.dma_start(out=w, in_=d['dw_diag'][ch*P:(ch+1)*P, :])
            dwd[ch] = w
        # padded x1 spike tiles: [20 guard | 4 x (18x18) planes | 20 guard]
        x1p_ab = []
        for i in range(2):
            xp_ = wp.tile([P, 1336], BF16, name=f"x1p{i}", tag=f"x1p{i}")
            GE.memzero(xp_)
            x1p_ab.append(xp_)

        # ---------------- phase A: kq / v / router matmuls + evicts ----------------
        psA = tc.alloc_tile_pool(name="psA", bufs=4, space="PSUM")
        _ps_i = [0]

        def psab():
            _ps_i[0] += 1
            return psA if _ps_i[0] % 2 else ps_m
        m_kq = mp.tile([100, 5 * N], F32, name="m_kq", tag="m_kq")
        m_vt = mp.tile([P, 768], F32, name="m_vt", tag="m_vt")
        m_p = mp.tile([P, 768], F32, name="m_p", tag="m_p")
        m_o = [mp.tile([P, N], F32, name=f"m_o{i}", tag=f"m_o{i}") for i in range(3)]

        xkq_t = [xkq_p.tile([100, 5 * N], F32, name=f"xkq{t}", tag="t") for t in range(T)]
        xvt_t = [xev_p.tile([P, 768], F32, name=f"xvt{t}", tag="t") for t in range(T)]

        for tp in range(2):
            for u in range(NU):
                # u0 block is 100 wide: 96 k-head rows + 4 router rows
                rw = 100 if u == 0 else 96
                w0 = 0 if u == 0 else 100 + 96 * (u - 1)
                pt = psab().tile([rw, 512], F32, name=f"pkq{u}_{tp}", tag="pm")
                for kt in range(3):
                    nc.tensor.matmul(pt, _r(kqf[kt][:, w0:w0+rw]),
                                     _r(xs_kt[kt][:, tp*512:(tp+1)*512]),
                                     start=(kt == 0), stop=(kt == 2))
                for ti in range(2):
                    t = tp * 2 + ti
                    c = u * 4 + t
                    nc.scalar.activation(xkq_t[t][0:rw, u*N:(u+1)*N], pt[:, ti*N:(ti+1)*N],
                                         AF.Identity, bias=b_kq[0:rw, c:c+1],
                                         scale=a_kq[0:rw, c:c+1])
        for t in range(T):
            for mt in range(2):
                pv = psab().tile([P, 384], F32, name=f"pvt{t}_{mt}", tag="pm")
                for kt in range(3):
                    nc.tensor.matmul(pv, _r(xs_kt[kt][:, t*N + mt*P: t*N + (mt+1)*P]),
                                     _r(vf[kt]), start=(kt == 0), stop=(kt == 2))
                nc.scalar.activation(xvt_t[t][:, mt*384:(mt+1)*384], pv, AF.Copy,
                                     bias=0.0, scale=0.5 * float(2.0 ** t))

        # ---------------- phase B: LIF scans for kq / v / r ----------------
        sp_t, v_sp, w_sp = [], [], []
        for t in range(T):
            thr = float(2.0 ** t)
            U = xkq_t[t]
            if t > 0:
                VE.tensor_tensor(U, m_kq, U, AL.add)
            sp = sp_p.tile([100, 5 * N], BF16, name=f"sp{t}", tag="t")
            VE.tensor_single_scalar(sp, U, thr, AL.is_ge)
            if t < T - 1:
                VE.scalar_tensor_tensor(out=m_kq, in0=U, scalar=thr, in1=U,
                                        op0=AL.is_lt, op1=AL.mult)
            sp_t.append(sp)

            U = xvt_t[t]
            if t > 0:
                VE.tensor_tensor(U, m_vt, U, AL.add)
            vs = vsp_p.tile([P, 768], BF16, name=f"vsp{t}", tag="t")
            VE.tensor_single_scalar(vs, U, thr, AL.is_ge)
            if t < T - 1:
                VE.scalar_tensor_tensor(out=m_vt, in0=U, scalar=thr, in1=U,
                                        op0=AL.is_lt, op1=AL.mult)
            v_sp.append(vs)

            # router spikes ride in sp rows 96:100 of the u0 block; transpose
            # each position-half to (pos, 4) for the per-partition y weights
            ws = wsp_p.tile([P, 8], F32, name=f"wsp{t}", tag="t")
            ws4 = wsp_p.tile([4, N], BF16, name=f"ws4_{t}", tag="t")
            nc.sync.dma_start(out=ws4, in_=sp[96:100, 0:N])
            for mt in range(2):
                ptw = ps_m.tile([P, 4], BF16, name=f"ptw{t}{mt}", tag="pm")
                nc.tensor.transpose(ptw, ws4[:, mt*P:(mt+1)*P], ident[0:4, 0:4])
                nc.scalar.activation(ws[:, mt*4:(mt+1)*4], ptw, AF.Copy)
            w_sp.append(ws)

        # ---------------- phase C: experts ----------------
        y = [[None] * 2 for _ in range(T)]
        m_res_e = [mp.tile([P, 768], F32, name=f"m_res{e}", tag=f"m_res{e}")
                   for e in range(NE)]
        for t in range(T):
            thr = float(2.0 ** t)
            for e in range(NE):
                m_res = m_res_e[e]
                at_sb = []
                for mt in range(2):
                    pa = psab().tile([P, N], F32, name=f"pat{e}{t}{mt}", tag="pm")
                    nc.tensor.matmul(pa, sp_t[t][0:96, mt*P:(mt+1)*P],
                                     sp_t[t][0:96, (1+e)*N:(2+e)*N], start=True, stop=True)
                    ats = at_p.tile([P, N], BF16, name=f"at{e}{t}{mt}", tag="t")
                    nc.scalar.activation(ats, pa, AF.Copy)
                    at_sb.append(ats)
                xr = xev_p.tile([P, 768], F32, name=f"xres{e}{t}", tag="t")
                for mt in range(2):
                    pr_ = psab().tile([P, 384], F32, name=f"pres{e}{t}{mt}", tag="pm")
                    for mk in range(2):
                        nc.tensor.matmul(pr_, at_sb[mk][:, mt*P:(mt+1)*P],
                                         v_sp[t][:, mk*384:(mk+1)*384],
                                         start=(mk == 0), stop=(mk == 1))
                    nc.scalar.activation(xr[:, mt*384:(mt+1)*384], pr_, AF.Copy,
                                         bias=0.0, scale=0.5 * thr)
                U = xr
                if t > 0:
                    GE.tensor_add(U, m_res, U)
                rs = rsp_p.tile([P, 768], BF16, name=f"rsp{e}{t}", tag="t")
                VE.tensor_single_scalar(rs, U, thr, AL.is_ge)
                if t < T - 1:
                    VE.scalar_tensor_tensor(out=m_res, in0=U, scalar=thr, in1=U,
                                            op0=AL.is_lt, op1=AL.mult)
                for mt in range(2):
                    if e == 0:
                        yt = y_p.tile([P, 384], BF16, name=f"y{t}_{mt}", tag="t")
                        VE.scalar_tensor_tensor(
                            out=yt, in0=rs[:, mt*384:(mt+1)*384],
                            scalar=w_sp[t][:, mt*4:mt*4+1],
                            in1=rs[:, mt*384:(mt+1)*384], op0=AL.mult, op1=AL.bypass)
                        y[t][mt] = yt
                    else:
                        VE.scalar_tensor_tensor(
                            out=y[t][mt], in0=rs[:, mt*384:(mt+1)*384],
                            scalar=w_sp[t][:, mt*4+e:mt*4+e+1],
                            in1=y[t][mt], op0=AL.mult, op1=AL.add)

        # ---------------- phase D: transpose y, proj, LIF, residual ----------------
        ydn = [ydn_p.tile([P, 4 * N], BF16, name=f"ydn{dt}", tag="t") for dt in range(3)]
        xp_t = [xev_p.tile([P, 768], F32, name=f"xp{t}", tag="t") for t in range(T)]
        for tp in range(2):
            for t in (tp * 2, tp * 2 + 1):
                for mt in range(2):
                    for dt in range(3):
                        ptr = psab().tile([P, P], BF16, name=f"ptr{t}{mt}{dt}", tag="pm")
                        nc.tensor.transpose(ptr, y[t][mt][:, dt*P:(dt+1)*P], ident)
                        nc.scalar.activation(ydn[dt][:, t*N + mt*P: t*N + (mt+1)*P],
                                             ptr, AF.Copy)
            for mt in range(3):
                pp = psab().tile([P, 512], F32, name=f"pp{mt}_{tp}", tag="pm")
                first = True
                for kt in range(3):
                    r_ = ydn[kt][:, tp*512:(tp+1)*512]
                    nc.tensor.matmul(pp, pjh[kt][:, mt*P:(mt+1)*P], r_,
                                     start=first, stop=False)
                    first = False
                    nc.tensor.matmul(pp, pjl[kt][:, mt*P:(mt+1)*P], r_,
                                     start=False, stop=(kt == 2))
                for ti in range(2):
                    t = tp * 2 + ti
                    c = mt * 4 + t
                    nc.scalar.activation(xp_t[t][:, mt*N:(mt+1)*N], pp[:, ti*N:(ti+1)*N],
                                         AF.Identity, bias=b_p[:, c:c+1], scale=a_p[:, c:c+1])
            for t in (tp * 2, tp * 2 + 1):
                thr = float(2.0 ** t)
                U = xp_t[t]
                if t > 0:
                    GE.tensor_add(U, m_p, U)
                if t < T - 1:
                    VE.scalar_tensor_tensor(out=m_p, in0=U, scalar=thr, in1=U,
                                            op0=AL.is_lt, op1=AL.mult)
                for mt in range(3):
                    # x_new overwrites xs in place (residual add)
                    VE.scalar_tensor_tensor(
                        out=_r(xs_kt[mt][:, t*N:(t+1)*N]), in0=U[:, mt*N:(mt+1)*N],
                        scalar=thr, in1=xs_kt[mt][:, t*N:(t+1)*N],
                        op0=AL.is_ge, op1=AL.add)

        psA.release()
        # ---------------- phase E: MLP ----------------
        mgs = []
        po0_pool = tc.alloc_tile_pool(name="po0", bufs=3, space="PSUM")
        po0 = [po0_pool.tile([P, 512], F32, name=f"po0_{mt}", tag="po")
               for mt in range(3)]
        with tc.tile_pool(name="dwp", bufs=3, space="PSUM") as dwp:
            dwd_load(0)
            for ch in range(8):
                if ch + 1 < 8:
                    dwd_load(ch + 1)
                xh = xh_p.tile([P, 2048], F32, name=f"xh{ch}", tag="t")
                for half in range(2):
                    mth = ch + 8 * half
                    for tp in range(2):
                        ph = ps_m.tile([P, 512], F32, name=f"ph{ch}{half}{tp}", tag="pm")
                        for kt in range(3):
                            nc.tensor.matmul(ph, _r(f1f[kt][:, mth*P:(mth+1)*P]),
                                             _r(xs_kt[kt][:, tp*512:(tp+1)*512]),
                                             start=(kt == 0), stop=(kt == 2))
                        for ti in range(2):
                            t = tp * 2 + ti
                            c = mth * 4 + t
                            nc.scalar.activation(
                                xh[:, half*1024 + t*N: half*1024 + (t+1)*N],
                                ph[:, ti*N:(ti+1)*N], AF.Identity,
                                bias=b_h[:, c:c+1], scale=a_h[:, c:c+1])
                # h-LIF: adds on GPSIMD, spikes via saturated sigmoid on ScalarE,
                # resets on DVE
                m_h = mh_p.tile([P, 512], F32, name=f"m_h{ch}", tag="t")
                x1p = x1p_ab[ch % 2]
                x1p4 = x1p[:, 20:1316].rearrange("p (t h w) -> p t h w", t=4, h=18)
                x2 = x2_p.tile([P, 1024], BF16, name=f"x2_{ch}", tag="t")
                xh3 = xh.rearrange("p (h q) -> p h q", h=2)
                mh3 = m_h.rearrange("p (h q) -> p h q", h=2)
                for t in range(T):
                    thr = float(2.0 ** t)
                    U3 = xh3[:, :, t*N:(t+1)*N]
                    if t > 0:
                        VE.tensor_tensor(U3, mh3, U3, AL.add)
                    nc.scalar.activation(
                        x1p4[:, t, 1:17, 1:17], xh[:, t*N:(t+1)*N],
                        AF.Sigmoid, bias=sgb[:, t:t+1], scale=K_SG)
                    VE.tensor_single_scalar(x2[:, t*N:(t+1)*N],
                                            xh[:, 1024 + t*N: 1024 + (t+1)*N],
                                            thr, AL.is_ge)
                    if t < T - 1:
                        VE.scalar_tensor_tensor(out=mh3, in0=U3, scalar=thr, in1=U3,
                                                op0=AL.is_lt, op1=AL.mult)
                # depthwise 3x3: per t-plane, 9 shifted diagonal bf16
                # matmuls on the padded plane (zero pads give exact boundary
                # handling, no fixups); one PSUM bank per t-plane chunk
                acc = acc_p.tile([P, 1024], F32, name=f"acc{ch}", tag="t")
                for t in range(T):
                    pacc = dwp.tile([P, 324], F32, name=f"pacc{ch}_{t}", tag="pacc")
                    for dp in range(9):
                        dy, dx = dp // 3, dp % 3
                        dlt = 18 * (dy - 1) + (dx - 1)
                        nc.tensor.matmul(
                            pacc, dwd[ch][:, dp*P:(dp+1)*P],
                            x1p[:, 20 + dlt + t*324: 20 + dlt + (t+1)*324],
                            start=(dp == 0), stop=(dp == 8))
                    pacc3 = pacc.rearrange("p (h w) -> p h w", h=18)
                    nc.scalar.activation(
                        acc[:, t*N:(t+1)*N], pacc3[:, 1:17, 1:17],
                        AF.Identity, bias=bdw_t[:, ch*4+t:ch*4+t+1],
                        scale=float(2.0 ** t))
                # dw-LIF + gate -> mg (bf16 exact {0,1})
                m_dw = mdw_p.tile([P, N], F32, name=f"m_dw{ch}", tag="t")
                mg = mg_p.tile([P, 1024], BF16, name=f"mg{ch}", tag="t")
                for t in range(T):
                    thr = float(2.0 ** t)
                    U = acc[:, t*N:(t+1)*N]
                    if t > 0:
                        VE.tensor_tensor(U, m_dw, U, AL.add)
                    VE.scalar_tensor_tensor(out=mg[:, t*N:(t+1)*N], in0=U, scalar=thr,
                                            in1=x2[:, t*N:(t+1)*N],
                                            op0=AL.is_ge, op1=AL.mult)
                    if t < T - 1:
                        VE.scalar_tensor_tensor(out=m_dw, in0=U, scalar=thr, in1=U,
                                                op0=AL.is_lt, op1=AL.mult)
                mgs.append(mg)
                for mt in range(3):
                    nc.tensor.matmul(po0[mt], f2h[ch][:, mt*P:(mt+1)*P],
                                     mg[:, 0:512], start=(ch == 0), stop=False,
                                     skip_group_check=True)
                    nc.tensor.matmul(po0[mt], f2l[ch][:, mt*P:(mt+1)*P],
                                     mg[:, 0:512], start=False, stop=(ch == 7),
                                     skip_group_check=True)

        # fc2 tp1 (bf16 2-term): ch order reversed so the first PSUM write
        # waits on mgs[7], i.e. after every pacc bank is drained
        with tc.tile_pool(name="ps_o", bufs=3, space="PSUM") as ps_o:
            po1 = [ps_o.tile([P, 512], F32, name=f"po1_{mt}", tag="po")
                   for mt in range(3)]
            po = [po0, po1]
            for ch in range(7, -1, -1):
                for mt in range(3):
                    nc.tensor.matmul(po1[mt], f2h[ch][:, mt*P:(mt+1)*P],
                                     mgs[ch][:, 512:1024],
                                     start=(ch == 7), stop=False,
                                     skip_group_check=True)
                    nc.tensor.matmul(po1[mt], f2l[ch][:, mt*P:(mt+1)*P],
                                     mgs[ch][:, 512:1024],
                                     start=False, stop=(ch == 0),
                                     skip_group_check=True)

            # fc2 evict + final LIF + residual + store
            xo_t = [xev_p.tile([P, 768], F32, name=f"xo{t}", tag="t") for t in range(T)]
            for t in range(T):
                for mt in range(3):
                    c = mt * 4 + t
                    nc.scalar.activation(xo_t[t][:, mt*N:(mt+1)*N],
                                         po[t // 2][mt][:, (t % 2)*N:(t % 2+1)*N],
                                         AF.Identity, bias=b_o[:, c:c+1],
                                         scale=a_o[:, c:c+1])
            for t in range(T):
                thr = float(2.0 ** t)
                for mt in range(3):
                    U = xo_t[t][:, mt*N:(mt+1)*N]
                    if t > 0:
                        GE.tensor_add(U, m_o[mt], U)
                    if t < T - 1:
                        VE.scalar_tensor_tensor(out=m_o[mt], in0=U, scalar=thr, in1=U,
                                                op0=AL.is_lt, op1=AL.mult)
                    # final out in place over xo (reset already consumed U)
                    VE.scalar_tensor_tensor(
                        out=U, in0=U, scalar=thr,
                        in1=xs_kt[mt][:, t*N:(t+1)*N], op0=AL.is_ge, op1=AL.add)
                    nc.sync.dma_start(out=d['out'][t*C + mt*P: t*C + (mt+1)*P, :],
                                      in_=U)
        po0_pool.release()


def _build():
    nc = bacc.Bacc()
    with tile.TileContext(nc) as tc:
        with tc.tile_pool(name="dram", bufs=1, space="DRAM") as dram:
            def din(name, shape, dt=F32):
                return dram.tile(shape, dt, kind="ExternalInput", name=name,
                                 uniquify=False)
            d = {
                'xin': din('xin', [C, 4 * N]),
                'out': dram.tile([T * C, N], F32, kind="ExternalOutput",
                                 name='out', uniquify=False),
                'kq_wf': din('kq_wf', [384, 484]),
                's_kq': din('s_kq', [100, 40]),
                'v_wf': din('v_wf', [384, 384]),
                'pj_w2': din('pj_w2', [384, 768], BF16),
                's_po': din('s_po', [128, 48]),
                'f1_wf': din('f1_wf', [384, 2048]),
                's_h': din('s_h', [128, 132]),
                's_dw': din('s_dw', [128, 32]),
                'f2_w2': din('f2_w2', [1024, 768], BF16),
                'dw_diag': din('dw_diag', [1024, 1152], BF16),
                'ident': din('ident', [128, 128], BF16),
            }
            _body(nc, tc, d)
    nc.finalize()
    return nc


_NC_CACHE = {}


def _get_nc():
    if 'nc' not in _NC_CACHE:
        _NC_CACHE['nc'] = _build()
    return _NC_CACHE['nc']


def _tcols(a):
    rows, k = a.shape
    out = np.empty((rows, k * 4), np.float32)
    for u in range(k):
        for t in range(4):
            out[:, u * 4 + t] = a[:, u] * (2.0 ** t)
    return out


def _split(w):
    hi = w.astype(ml_dtypes.bfloat16)
    lo = (w - hi.astype(np.float32)).astype(ml_dtypes.bfloat16)
    return hi, lo


def _cat2(w):
    hi, lo = _split(w)
    return np.concatenate([hi, lo], axis=1)


def _prep_common(inputs):
    inp = {k: np.asarray(v, np.float32) for k, v in inputs.items()}
    k_wT = inp['k_w'].T
    exp_wT = np.concatenate([inp['exp_w'][e].T for e in range(NE)], axis=1)
    r_wTs = inp['router_w'].T * (inp['router_g'] * S * 0.5)[None, :]
    kq_wT = np.concatenate([k_wT, r_wTs, exp_wT], axis=1)
    a_kq = np.zeros((100, 5), np.float32)
    b_kq = np.zeros((100, 5), np.float32)
    a_kq[0:96, 0] = 0.5
    a_kq[96:100, 0] = 1.0
    b_kq[96:100, 0] = 0.5 * (inp['router_b'] * inp['router_g'] * S
                             + inp['router_be'])
    for e in range(NE):
        a_kq[0:96, 1 + e] = 0.5 * inp['exp_g'][e] * S
        b_kq[0:96, 1 + e] = 0.5 * inp['exp_b'][e]
    taps = inp['dw_w'][:, 0] * (0.5 * inp['dw_g'] * S)[:, None, None]
    tap2 = taps.reshape(8, 128, 9).transpose(1, 0, 2).reshape(128, 72)
    tap_bf = tap2.astype(ml_dtypes.bfloat16)
    dw_diag = np.zeros((8, 128, 9, 128), dtype=ml_dtypes.bfloat16)
    idx = np.arange(128)
    for ch in range(8):
        for dp in range(9):
            dw_diag[ch, idx, dp, idx] = tap_bf[idx, ch * 9 + dp]
    dw_diag = dw_diag.reshape(1024, 1152)
    bdw_base = (0.5 * (inp['dw_b'] * inp['dw_g'] * S
                       + inp['dw_be'])).reshape(8, 128).T
    bdw_t = np.empty((128, 32), np.float32)
    for ch in range(8):
        for t in range(4):
            bdw_t[:, ch * 4 + t] = bdw_base[:, ch] * (2.0 ** t)
    sgb = np.tile((-(2.0 ** 20) * (2.0 ** np.arange(4, dtype=np.float32)))[None, :],
                  (128, 1)).astype(np.float32)

    com = {
        'kq_wf': kq_wT,
        's_kq': np.concatenate([_tcols(a_kq), _tcols(b_kq)], axis=1),
        'v_wf': inp['v_w'].T,
        'pj_w2': _cat2(inp['proj_w'].T),
        's_po': np.concatenate([
            _tcols((0.5 * inp['proj_g'] * S).reshape(3, 128).T),
            _tcols((0.5 * (inp['proj_b'] * inp['proj_g'] * S
                           + inp['proj_be'])).reshape(3, 128).T),
            _tcols((0.5 * inp['fc2_g'] * S).reshape(3, 128).T),
            _tcols((0.5 * (inp['fc2_b'] * inp['fc2_g'] * S
                           + inp['fc2_be'])).reshape(3, 128).T)], axis=1),
        'f1_wf': inp['fc1_w'].T,
        's_h': np.concatenate([
            _tcols((0.5 * inp['fc1_g'] * S).reshape(16, 128).T),
            _tcols((0.5 * (inp['fc1_b'] * inp['fc1_g'] * S
                           + inp['fc1_be'])).reshape(16, 128).T),
            sgb], axis=1),
        's_dw': bdw_t,
        'f2_w2': _cat2(inp['fc2_w'].T),
        'dw_diag': dw_diag,
        'ident': np.eye(128, dtype=ml_dtypes.bfloat16),
    }
    return {k: np.ascontiguousarray(v) for k, v in com.items()}


def run(inputs, trace=False, tmpdir=None):
    com = _prep_common(inputs)
    x = np.asarray(inputs['x'], np.float32).reshape(T, B, C, N)
    in_maps = []
    for b in range(B):
        m = dict(com)
        m['xin'] = np.ascontiguousarray(x[:, b].transpose(1, 0, 2).reshape(C, T * N))
        in_maps.append(m)
    res = run_bass_kernel_spmd(_get_nc(), in_maps, list(range(B)),
                               trace=trace, tmpdir=tmpdir)
    out = np.empty((T, B, C, N), np.float32)
    for b in range(B):
        out[:, b] = res.results[b]['out'].reshape(T, C, N)
    return out.reshape(T * B, C, 16, 16), res.exec_time_ns


def kernel(**inputs):
    out, _ = run(inputs)
    return out


# revision 25
# speedup vs baseline: 1.1446x; 1.0386x over previous
"""Trainium2 Bass kernel for nn_Block_31954556682442 (spiking MoE-SSA block).

Sharding: pure data-parallel over batch B=8 -> one sample (4 LIF time steps)
per NeuronCore, zero collectives. v3 design:
  - weight matmuls in fp32r (PE rounds inputs to ~11 mantissa bits, exact
    fp32 accumulate, full bf16 throughput) -> no hi/lo splits needed
  - time steps batched into matmul free dims (N=512 covers 2 steps)
  - bf16 exact-integer attention core (spikes are {0,1})
  - LIF scans in 2^t-scaled form: membrane update = tensor_add on GPSIMD,
    spike/reset = tensor_scalar/scalar_tensor_tensor on DVE (threshold 2^t)
  - depthwise 3x3 conv t-batched: 9 shifted per-partition-scalar MACs over
    (128, 4*256) tiles on DVE, 2^t applied at the LIF add
  - PSUM evicts fused with BN scale+bias (+2^t*0.5) on ScalarE
Self-contained: hardcodes all shapes; no sibling imports.
"""
import numpy as np
import ml_dtypes

import concourse.bacc as bacc
import concourse.mybir as mybir
import concourse.tile as tile
from concourse.bass_utils import run_bass_kernel_spmd

F32 = mybir.dt.float32
F32R = mybir.dt.float32r
BF16 = mybir.dt.bfloat16
AL = mybir.AluOpType
AF = mybir.ActivationFunctionType

T, B, C, N = 4, 8, 384, 256
ED = 96
NE = 4
NU = 5
HID, HH = 2048, 1024
S = float(1.0 / np.sqrt(1.0 + 1e-5))
P = 128


def _r(ap):
    return ap.bitcast(F32R)


def _body(nc, tc, d):
    from contextlib import ExitStack
    VE = nc.vector
    GE = nc.gpsimd
    K_SG = float(2.0 ** 20)

    with ExitStack() as ctx:
        def pool(name, bufs, space="SBUF"):
            return ctx.enter_context(tc.tile_pool(name=name, bufs=bufs, space=space))

        wp = pool("wp", 1)
        mp = pool("mp", 1)
        ps_m = pool("ps_m", 2, "PSUM")
        xs_p = pool("xs_p", 3)       # (128,1024) f32, doubles as x_new
        xkq_p = pool("xkq_p", 2)     # (96,1280) f32
        xev_p = pool("xev_p", 4)     # (128,768) f32 evict/LIF targets
        sp_p = pool("sp_p", 4)       # (96,1280) bf16 kq spikes
        vsp_p = pool("vsp_p", 4)     # (128,768) bf16
        wsp_p = pool("wsp_p", 4)     # (128,8) f32
        at_p = pool("at_p", 3)       # (128,256) bf16
        rsp_p = pool("rsp_p", 2)     # (128,768) bf16
        y_p = pool("y_p", 8)         # (128,384) bf16
        ydn_p = pool("ydn_p", 3)     # (128,1024) bf16
        xh_p = pool("xh_p", 2)       # (128,2048) f32
        x2_p = pool("x2_p", 2)       # (128,1024) bf16
        acc_p = pool("acc_p", 2)     # (128,1024) f32
        mg_p = pool("mg_p", 8)       # (128,1024) bf16, all 8 persist for fc2
        mh_p = pool("mh_p", 1)       # (128,512) f32
        mdw_p = pool("mdw_p", 1)     # (128,256) f32

        # ---------------- weight loads ----------------
        def wload(name, shape, dt=F32, src=None, rr=False):
            w = wp.tile(shape, dt, name=name, tag=name)
            s_ = d[name] if src is None else src
            nc.sync.dma_start(out=_r(w) if rr else w, in_=_r(s_) if rr else s_)
            return w

        ident = wload('ident', [P, P], BF16)
        # PE warmup: ~60 dummy matmuls to ramp the PE pstate before phase A
        pwarm = ps_m.tile([P, P], F32, name="pwarm", tag="pm")
        for wi in range(60):
            nc.tensor.matmul(pwarm, ident, ident, start=True, stop=True)
        warm_sink = wp.tile([P, 1], F32, name="warm_sink", tag="warm_sink")
        nc.scalar.activation(warm_sink, pwarm[:, 0:1], AF.Copy)

        # xs first (A-phase starts on these)
        xs_kt = []
        for kt in range(3):
            x_ = xs_p.tile([P, 4 * N], F32, name=f"xs{kt}", tag="t")
            xs_kt.append(x_)
        for kt in range(3):
            nc.sync.dma_start(out=_r(xs_kt[kt]), in_=_r(d['xin'][kt*P:(kt+1)*P, :]))
        kqf, vf, f1f = [], [], []
        for kt in range(3):
            kqf.append(wload(f'kqf_{kt}', [P, 484], F32, d['kq_wf'][kt*P:(kt+1)*P, :], rr=True))
        skq = wload('s_kq', [100, 40])
        a_kq, b_kq = skq[:, 0:20], skq[:, 20:40]
        for kt in range(3):
            vf.append(wload(f'vf_{kt}', [P, 384], F32, d['v_wf'][kt*P:(kt+1)*P, :], rr=True))
        pj2 = []
        for kt in range(3):
            pj2.append(wload(f'pj2_{kt}', [P, 768], BF16, d['pj_w2'][kt*P:(kt+1)*P, :]))
        pjh = [w[:, 0:384] for w in pj2]; pjl = [w[:, 384:768] for w in pj2]
        spo = wload('s_po', [P, 48])
        a_p, b_p = spo[:, 0:12], spo[:, 12:24]
        a_o, b_o = spo[:, 24:36], spo[:, 36:48]
        for kt in range(3):
            f1f.append(wload(f'f1f_{kt}', [P, 2048], F32, d['f1_wf'][kt*P:(kt+1)*P, :], rr=True))
        sh = wload('s_h', [P, 132])
        a_h, b_h, sgb = sh[:, 0:64], sh[:, 64:128], sh[:, 128:132]
        sdw = wload('s_dw', [P, 32])
        bdw_t = sdw[:, 0:32]
        f22 = []
        for ch in range(8):
            f22.append(wload(f'f22_{ch}', [P, 768], BF16, d['f2_w2'][ch*P:(ch+1)*P, :]))
        f2h = [w[:, 0:384] for w in f22]; f2l = [w[:, 384:768] for w in f22]
        dwd_p = pool("dwd_p", 3)     # (128,1152) bf16 streamed diag taps
        dwd = {}

        def dwd_load(ch):
            w = dwd_p.tile([P, 1152], BF16, name=f"dwd{ch}", tag="t")
            nc.sync.dma_start(out=w, in_=d['dw_diag'][ch*P:(ch+1)*P, :])
            dwd[ch] = w
        # padded x1 spike tiles: [20 guard | 4 x (18x18) planes | 20 guard]
        x1p_ab = []
        for i in range(2):
            xp_ = wp.tile([P, 1336], BF16, name=f"x1p{i}", tag=f"x1p{i}")
            GE.memzero(xp_)
            x1p_ab.append(xp_)

        # ---------------- phase A: kq / v / router matmuls + evicts ----------------
        psA = tc.alloc_tile_pool(name="psA", bufs=4, space="PSUM")
        _ps_i = [0]

        def psab():
            _ps_i[0] += 1
            return psA if _ps_i[0] % 2 else ps_m
        m_kq = mp.tile([100, 5 * N], F32, name="m_kq", tag="m_kq")
        m_vt = mp.tile([P, 768], F32, name="m_vt", tag="m_vt")
        m_p = mp.tile([P, 768], F32, name="m_p", tag="m_p")
        m_o = [mp.tile([P, N], F32, name=f"m_o{i}", tag=f"m_o{i}") for i in range(3)]

        xkq_t = [xkq_p.tile([100, 5 * N], F32, name=f"xkq{t}", tag="t") for t in range(T)]
        xvt_t = [xev_p.tile([P, 768], F32, name=f"xvt{t}", tag="t") for t in range(T)]

        for tp in range(2):
            for u in range(NU):
                # u0 block is 100 wide: 96 k-head rows + 4 router rows
                rw = 100 if u == 0 else 96
                w0 = 0 if u == 0 else 100 + 96 * (u - 1)
                pt = psab().tile([rw, 512], F32, name=f"pkq{u}_{tp}", tag="pm")
                for kt in range(3):
                    nc.tensor.matmul(pt, _r(kqf[kt][:, w0:w0+rw]),
                                     _r(xs_kt[kt][:, tp*512:(tp+1)*512]),
                                     start=(kt == 0), stop=(kt == 2))
                for ti in range(2):
                    t = tp * 2 + ti
                    c = u * 4 + t
                    nc.scalar.activation(xkq_t[t][0:rw, u*N:(u+1)*N], pt[:, ti*N:(ti+1)*N],
                                         AF.Identity, bias=b_kq[0:rw, c:c+1],
                                         scale=a_kq[0:rw, c:c+1])
        for t in range(T):
            for mt in range(2):
                pv = psab().tile([P, 384], F32, name=f"pvt{t}_{mt}", tag="pm")
                for kt in range(3):
                    nc.tensor.matmul(pv, _r(xs_kt[kt][:, t*N + mt*P: t*N + (mt+1)*P]),
                                     _r(vf[kt]), start=(kt == 0), stop=(kt == 2))
                nc.scalar.activation(xvt_t[t][:, mt*384:(mt+1)*384], pv, AF.Copy,
                                     bias=0.0, scale=0.5 * float(2.0 ** t))

        # ---------------- phase B: LIF scans for kq / v / r ----------------
        sp_t, v_sp, w_sp = [], [], []
        for t in range(T):
            thr = float(2.0 ** t)
            U = xkq_t[t]
            if t > 0:
                VE.tensor_tensor(U, m_kq, U, AL.add)
            sp = sp_p.tile([100, 5 * N], BF16, name=f"sp{t}", tag="t")
            VE.tensor_single_scalar(sp, U, thr, AL.is_ge)
            if t < T - 1:
                VE.scalar_tensor_tensor(out=m_kq, in0=U, scalar=thr, in1=U,
                                        op0=AL.is_lt, op1=AL.mult)
            sp_t.append(sp)

            U = xvt_t[t]
            if t > 0:
                VE.tensor_tensor(U, m_vt, U, AL.add)
            vs = vsp_p.tile([P, 768], BF16, name=f"vsp{t}", tag="t")
            VE.tensor_single_scalar(vs, U, thr, AL.is_ge)
            if t < T - 1:
                VE.scalar_tensor_tensor(out=m_vt, in0=U, scalar=thr, in1=U,
                                        op0=AL.is_lt, op1=AL.mult)
            v_sp.append(vs)

            # router spikes ride in sp rows 96:100 of the u0 block; transpose
            # each position-half to (pos, 4) for the per-partition y weights
            ws = wsp_p.tile([P, 8], F32, name=f"wsp{t}", tag="t")
            ws4 = wsp_p.tile([4, N], BF16, name=f"ws4_{t}", tag="t")
            nc.sync.dma_start(out=ws4, in_=sp[96:100, 0:N])
            for mt in range(2):
                ptw = ps_m.tile([P, 4], BF16, name=f"ptw{t}{mt}", tag="pm")
                nc.tensor.transpose(ptw, ws4[:, mt*P:(mt+1)*P], ident[0:4, 0:4])
                nc.scalar.activation(ws[:, mt*4:(mt+1)*4], ptw, AF.Copy)
            w_sp.append(ws)

        # ---------------- phase C: experts ----------------
        y = [[None] * 2 for _ in range(T)]
        m_res_e = [mp.tile([P, 768], F32, name=f"m_res{e}", tag=f"m_res{e}")
                   for e in range(NE)]
        for t in range(T):
            thr = float(2.0 ** t)
            for e in range(NE):
                m_res = m_res_e[e]
                at_sb = []
                for mt in range(2):
                    pa = psab().tile([P, N], F32, name=f"pat{e}{t}{mt}", tag="pm")
                    nc.tensor.matmul(pa, sp_t[t][0:96, mt*P:(mt+1)*P],
                                     sp_t[t][0:96, (1+e)*N:(2+e)*N], start=True, stop=True)
                    ats = at_p.tile([P, N], BF16, name=f"at{e}{t}{mt}", tag="t")
                    nc.scalar.activation(ats, pa, AF.Copy)
                    at_sb.append(ats)
                xr = xev_p.tile([P, 768], F32, name=f"xres{e}{t}", tag="t")
                for mt in range(2):
                    pr_ = psab().tile([P, 384], F32, name=f"pres{e}{t}{mt}", tag="pm")
                    for mk in range(2):
                        nc.tensor.matmul(pr_, at_sb[mk][:, mt*P:(mt+1)*P],
                                         v_sp[t][:, mk*384:(mk+1)*384],
                                         start=(mk == 0), stop=(mk == 1))
                    nc.scalar.activation(xr[:, mt*384:(mt+1)*384], pr_, AF.Copy,
                                         bias=0.0, scale=0.5 * thr)
                U = xr
                if t > 0:
                    VE.tensor_tensor(U, m_res, U, AL.add)
                rs = rsp_p.tile([P, 768], BF16, name=f"rsp{e}{t}", tag="t")
                VE.tensor_single_scalar(rs, U, thr, AL.is_ge)
                if t < T - 1:
                    VE.scalar_tensor_tensor(out=m_res, in0=U, scalar=thr, in1=U,
                                            op0=AL.is_lt, op1=AL.mult)
                for mt in range(2):
                    if e == 0:
                        yt = y_p.tile([P, 384], BF16, name=f"y{t}_{mt}", tag="t")
                        VE.scalar_tensor_tensor(
                            out=yt, in0=rs[:, mt*384:(mt+1)*384],
                            scalar=w_sp[t][:, mt*4:mt*4+1],
                            in1=rs[:, mt*384:(mt+1)*384], op0=AL.mult, op1=AL.bypass)
                        y[t][mt] = yt
                    else:
                        VE.scalar_tensor_tensor(
                            out=y[t][mt], in0=rs[:, mt*384:(mt+1)*384],
                            scalar=w_sp[t][:, mt*4+e:mt*4+e+1],
                            in1=y[t][mt], op0=AL.mult, op1=AL.add)

        # ---------------- phase D: transpose y, proj, LIF, residual ----------------
        ydn = [ydn_p.tile([P, 4 * N], BF16, name=f"ydn{dt}", tag="t") for dt in range(3)]
        xp_t = [xev_p.tile([P, 768], F32, name=f"xp{t}", tag="t") for t in range(T)]
        for tp in range(2):
            for t in (tp * 2, tp * 2 + 1):
                for mt in range(2):
                    for dt in range(3):
                        ptr = psab().tile([P, P], BF16, name=f"ptr{t}{mt}{dt}", tag="pm")
                        nc.tensor.transpose(ptr, y[t][mt][:, dt*P:(dt+1)*P], ident)
                        nc.scalar.activation(ydn[dt][:, t*N + mt*P: t*N + (mt+1)*P],
                                             ptr, AF.Copy)
            for mt in range(3):
                pp = psab().tile([P, 512], F32, name=f"pp{mt}_{tp}", tag="pm")
                first = True
                for kt in range(3):
                    r_ = ydn[kt][:, tp*512:(tp+1)*512]
                    nc.tensor.matmul(pp, pjh[kt][:, mt*P:(mt+1)*P], r_,
                                     start=first, stop=False)
                    first = False
                    nc.tensor.matmul(pp, pjl[kt][:, mt*P:(mt+1)*P], r_,
                                     start=False, stop=(kt == 2))
                for ti in range(2):
                    t = tp * 2 + ti
                    c = mt * 4 + t
                    nc.scalar.activation(xp_t[t][:, mt*N:(mt+1)*N], pp[:, ti*N:(ti+1)*N],
                                         AF.Identity, bias=b_p[:, c:c+1], scale=a_p[:, c:c+1])
            for t in (tp * 2, tp * 2 + 1):
                thr = float(2.0 ** t)
                U = xp_t[t]
                if t > 0:
                    GE.tensor_add(U, m_p, U)
                if t < T - 1:
                    VE.scalar_tensor_tensor(out=m_p, in0=U, scalar=thr, in1=U,
                                            op0=AL.is_lt, op1=AL.mult)
                for mt in range(3):
                    # x_new overwrites xs in place (residual add)
                    VE.scalar_tensor_tensor(
                        out=_r(xs_kt[mt][:, t*N:(t+1)*N]), in0=U[:, mt*N:(mt+1)*N],
                        scalar=thr, in1=xs_kt[mt][:, t*N:(t+1)*N],
                        op0=AL.is_ge, op1=AL.add)

        psA.release()
        # ---------------- phase E: MLP ----------------
        mgs = []
        po0_pool = tc.alloc_tile_pool(name="po0", bufs=3, space="PSUM")
        po0 = [po0_pool.tile([P, 512], F32, name=f"po0_{mt}", tag="po")
               for mt in range(3)]
        with tc.tile_pool(name="dwp", bufs=3, space="PSUM") as dwp:
            dwd_load(0)
            for ch in range(8):
                if ch + 1 < 8:
                    dwd_load(ch + 1)
                xh = xh_p.tile([P, 2048], F32, name=f"xh{ch}", tag="t")
                for half in range(2):
                    mth = ch + 8 * half
                    for tp in range(2):
                        ph = ps_m.tile([P, 512], F32, name=f"ph{ch}{half}{tp}", tag="pm")
                        for kt in range(3):
                            nc.tensor.matmul(ph, _r(f1f[kt][:, mth*P:(mth+1)*P]),
                                             _r(xs_kt[kt][:, tp*512:(tp+1)*512]),
                                             start=(kt == 0), stop=(kt == 2))
                        for ti in range(2):
                            t = tp * 2 + ti
                            c = mth * 4 + t
                            nc.scalar.activation(
                                xh[:, half*1024 + t*N: half*1024 + (t+1)*N],
                                ph[:, ti*N:(ti+1)*N], AF.Identity,
                                bias=b_h[:, c:c+1], scale=a_h[:, c:c+1])
                # h-LIF: adds on GPSIMD, spikes via saturated sigmoid on ScalarE,
                # resets on DVE
                m_h = mh_p.tile([P, 512], F32, name=f"m_h{ch}", tag="t")
                x1p = x1p_ab[ch % 2]
                x1p4 = x1p[:, 20:1316].rearrange("p (t h w) -> p t h w", t=4, h=18)
                x2 = x2_p.tile([P, 1024], BF16, name=f"x2_{ch}", tag="t")
                xh3 = xh.rearrange("p (h q) -> p h q", h=2)
                mh3 = m_h.rearrange("p (h q) -> p h q", h=2)
                for t in range(T):
                    thr = float(2.0 ** t)
                    U3 = xh3[:, :, t*N:(t+1)*N]
                    if t > 0:
                        VE.tensor_tensor(U3, mh3, U3, AL.add)
                    nc.scalar.activation(
                        x1p4[:, t, 1:17, 1:17], xh[:, t*N:(t+1)*N],
                        AF.Sigmoid, bias=sgb[:, t:t+1], scale=K_SG)
                    VE.tensor_single_scalar(x2[:, t*N:(t+1)*N],
                                            xh[:, 1024 + t*N: 1024 + (t+1)*N],
                                            thr, AL.is_ge)
                    if t < T - 1:
                        VE.scalar_tensor_tensor(out=mh3, in0=U3, scalar=thr, in1=U3,
                                                op0=AL.is_lt, op1=AL.mult)
                # depthwise 3x3: per t-plane, 9 shifted diagonal bf16
                # matmuls on the padded plane (zero pads give exact boundary
                # handling, no fixups); one PSUM bank per t-plane chunk
                acc = acc_p.tile([P, 1024], F32, name=f"acc{ch}", tag="t")
                for t in range(T):
                    pacc = dwp.tile([P, 324], F32, name=f"pacc{ch}_{t}", tag="pacc")
                    for dp in range(9):
                        dy, dx = dp // 3, dp % 3
                        dlt = 18 * (dy - 1) + (dx - 1)
                        nc.tensor.matmul(
                            pacc, dwd[ch][:, dp*P:(dp+1)*P],
                            x1p[:, 20 + dlt + t*324: 20 + dlt + (t+1)*324],
                            start=(dp == 0), stop=(dp == 8))
                    pacc3 = pacc.rearrange("p (h w) -> p h w", h=18)
                    nc.scalar.activation(
                        acc[:, t*N:(t+1)*N], pacc3[:, 1:17, 1:17],
                        AF.Identity, bias=bdw_t[:, ch*4+t:ch*4+t+1],
                        scale=float(2.0 ** t))
                # dw-LIF + gate -> mg (bf16 exact {0,1})
                m_dw = mdw_p.tile([P, N], F32, name=f"m_dw{ch}", tag="t")
                mg = mg_p.tile([P, 1024], BF16, name=f"mg{ch}", tag="t")
                for t in range(T):
                    thr = float(2.0 ** t)
                    U = acc[:, t*N:(t+1)*N]
                    if t > 0:
                        VE.tensor_tensor(U, m_dw, U, AL.add)
                    VE.scalar_tensor_tensor(out=mg[:, t*N:(t+1)*N], in0=U, scalar=thr,
                                            in1=x2[:, t*N:(t+1)*N],
                                            op0=AL.is_ge, op1=AL.mult)
                    if t < T - 1:
                        VE.scalar_tensor_tensor(out=m_dw, in0=U, scalar=thr, in1=U,
                                                op0=AL.is_lt, op1=AL.mult)
                mgs.append(mg)
                for mt in range(3):
                    nc.tensor.matmul(po0[mt], f2h[ch][:, mt*P:(mt+1)*P],
                                     mg[:, 0:512], start=(ch == 0), stop=False,
                                     skip_group_check=True)
                    nc.tensor.matmul(po0[mt], f2l[ch][:, mt*P:(mt+1)*P],
                                     mg[:, 0:512], start=False, stop=(ch == 7),
                                     skip_group_check=True)

        # fc2 tp1 (bf16 2-term): ch order reversed so the first PSUM write
        # waits on mgs[7], i.e. after every pacc bank is drained
        with tc.tile_pool(name="ps_o", bufs=3, space="PSUM") as ps_o:
            po1 = [ps_o.tile([P, 512], F32, name=f"po1_{mt}", tag="po")
                   for mt in range(3)]
            po = [po0, po1]
            for ch in range(7, -1, -1):
                for mt in range(3):
                    nc.tensor.matmul(po1[mt], f2h[ch][:, mt*P:(mt+1)*P],
                                     mgs[ch][:, 512:1024],
                                     start=(ch == 7), stop=False,
                                     skip_group_check=True)
                    nc.tensor.matmul(po1[mt], f2l[ch][:, mt*P:(mt+1)*P],
                                     mgs[ch][:, 512:1024],
                                     start=False, stop=(ch == 0),
                                     skip_group_check=True)

            # fc2 evict + final LIF + residual + store
            xo_t = [xev_p.tile([P, 768], F32, name=f"xo{t}", tag="t") for t in range(T)]
            for t in range(T):
                for mt in range(3):
                    c = mt * 4 + t
                    nc.scalar.activation(xo_t[t][:, mt*N:(mt+1)*N],
                                         po[t // 2][mt][:, (t % 2)*N:(t % 2+1)*N],
                                         AF.Identity, bias=b_o[:, c:c+1],
                                         scale=a_o[:, c:c+1])
            for t in range(T):
                thr = float(2.0 ** t)
                for mt in range(3):
                    U = xo_t[t][:, mt*N:(mt+1)*N]
                    if t > 0:
                        GE.tensor_add(U, m_o[mt], U)
                    if t < T - 1:
                        VE.scalar_tensor_tensor(out=m_o[mt], in0=U, scalar=thr, in1=U,
                                                op0=AL.is_lt, op1=AL.mult)
                    # final out in place over xo (reset already consumed U)
                    VE.scalar_tensor_tensor(
                        out=U, in0=U, scalar=thr,
                        in1=xs_kt[mt][:, t*N:(t+1)*N], op0=AL.is_ge, op1=AL.add)
                    nc.sync.dma_start(out=d['out'][t*C + mt*P: t*C + (mt+1)*P, :],
                                      in_=U)
        po0_pool.release()


def _build():
    nc = bacc.Bacc()
    with tile.TileContext(nc) as tc:
        with tc.tile_pool(name="dram", bufs=1, space="DRAM") as dram:
            def din(name, shape, dt=F32):
                return dram.tile(shape, dt, kind="ExternalInput", name=name,
                                 uniquify=False)
            d = {
                'xin': din('xin', [C, 4 * N]),
                'out': dram.tile([T * C, N], F32, kind="ExternalOutput",
                                 name='out', uniquify=False),
                'kq_wf': din('kq_wf', [384, 484]),
                's_kq': din('s_kq', [100, 40]),
                'v_wf': din('v_wf', [384, 384]),
                'pj_w2': din('pj_w2', [384, 768], BF16),
                's_po': din('s_po', [128, 48]),
                'f1_wf': din('f1_wf', [384, 2048]),
                's_h': din('s_h', [128, 132]),
                's_dw': din('s_dw', [128, 32]),
                'f2_w2': din('f2_w2', [1024, 768], BF16),
                'dw_diag': din('dw_diag', [1024, 1152], BF16),
                'ident': din('ident', [128, 128], BF16),
            }
            _body(nc, tc, d)
    nc.finalize()
    return nc


_NC_CACHE = {}


def _get_nc():
    if 'nc' not in _NC_CACHE:
        _NC_CACHE['nc'] = _build()
    return _NC_CACHE['nc']


def _tcols(a):
    rows, k = a.shape
    out = np.empty((rows, k * 4), np.float32)
    for u in range(k):
        for t in range(4):
            out[:, u * 4 + t] = a[:, u] * (2.0 ** t)
    return out


def _split(w):
    hi = w.astype(ml_dtypes.bfloat16)
    lo = (w - hi.astype(np.float32)).astype(ml_dtypes.bfloat16)
    return hi, lo


def _cat2(w):
    hi, lo = _split(w)
    return np.concatenate([hi, lo], axis=1)


def _prep_common(inputs):
    inp = {k: np.asarray(v, np.float32) for k, v in inputs.items()}
    k_wT = inp['k_w'].T
    exp_wT = np.concatenate([inp['exp_w'][e].T for e in range(NE)], axis=1)
    r_wTs = inp['router_w'].T * (inp['router_g'] * S * 0.5)[None, :]
    kq_wT = np.concatenate([k_wT, r_wTs, exp_wT], axis=1)
    a_kq = np.zeros((100, 5), np.float32)
    b_kq = np.zeros((100, 5), np.float32)
    a_kq[0:96, 0] = 0.5
    a_kq[96:100, 0] = 1.0
    b_kq[96:100, 0] = 0.5 * (inp['router_b'] * inp['router_g'] * S
                             + inp['router_be'])
    for e in range(NE):
        a_kq[0:96, 1 + e] = 0.5 * inp['exp_g'][e] * S
        b_kq[0:96, 1 + e] = 0.5 * inp['exp_b'][e]
    taps = inp['dw_w'][:, 0] * (0.5 * inp['dw_g'] * S)[:, None, None]
    tap2 = taps.reshape(8, 128, 9).transpose(1, 0, 2).reshape(128, 72)
    tap_bf = tap2.astype(ml_dtypes.bfloat16)
    dw_diag = np.zeros((8, 128, 9, 128), dtype=ml_dtypes.bfloat16)
    idx = np.arange(128)
    for ch in range(8):
        for dp in range(9):
            dw_diag[ch, idx, dp, idx] = tap_bf[idx, ch * 9 + dp]
    dw_diag = dw_diag.reshape(1024, 1152)
    bdw_base = (0.5 * (inp['dw_b'] * inp['dw_g'] * S
                       + inp['dw_be'])).reshape(8, 128).T
    bdw_t = np.empty((128, 32), np.float32)
    for ch in range(8):
        for t in range(4):
            bdw_t[:, ch * 4 + t] = bdw_base[:, ch] * (2.0 ** t)
    sgb = np.tile((-(2.0 ** 20) * (2.0 ** np.arange(4, dtype=np.float32)))[None, :],
                  (128, 1)).astype(np.float32)

    com = {
        'kq_wf': kq_wT,
        's_kq': np.concatenate([_tcols(a_kq), _tcols(b_kq)], axis=1),
        'v_wf': inp['v_w'].T,
        'pj_w2': _cat2(inp['proj_w'].T),
        's_po': np.concatenate([
            _tcols((0.5 * inp['proj_g'] * S).reshape(3, 128).T),
            _tcols((0.5 * (inp['proj_b'] * inp['proj_g'] * S
                           + inp['proj_be'])).reshape(3, 128).T),
            _tcols((0.5 * inp['fc2_g'] * S).reshape(3, 128).T),
            _tcols((0.5 * (inp['fc2_b'] * inp['fc2_g'] * S
                           + inp['fc2_be'])).reshape(3, 128).T)], axis=1),
        'f1_wf': inp['fc1_w'].T,
        's_h': np.concatenate([
            _tcols((0.5 * inp['fc1_g'] * S).reshape(16, 128).T),
            _tcols((0.5 * (inp['fc1_b'] * inp['fc1_g'] * S
                           + inp['fc1_be'])).reshape(16, 128).T),
            sgb], axis=1),
        's_dw': bdw_t,
        'f2_w2': _cat2(inp['fc2_w'].T),
        'dw_diag': dw_diag,
        'ident': np.eye(128, dtype=ml_dtypes.bfloat16),
    }
    return {k: np.ascontiguousarray(v) for k, v in com.items()}


def run(inputs, trace=False, tmpdir=None):
    com = _prep_common(inputs)
    x = np.asarray(inputs['x'], np.float32).reshape(T, B, C, N)
    in_maps = []
    for b in range(B):
        m = dict(com)
        m['xin'] = np.ascontiguousarray(x[:, b].transpose(1, 0, 2).reshape(C, T * N))
        in_maps.append(m)
    res = run_bass_kernel_spmd(_get_nc(), in_maps, list(range(B)),
                               trace=trace, tmpdir=tmpdir)
    out = np.empty((T, B, C, N), np.float32)
    for b in range(B):
        out[:, b] = res.results[b]['out'].reshape(T, C, N)
    return out.reshape(T * B, C, 16, 16), res.exec_time_ns


def kernel(**inputs):
    out, _ = run(inputs)
    return out
